# revision 2
# baseline (speedup 1.0000x reference)
"""Multi-head causal attention on 8 TRN2 NeuronCores — v5.

Sharding: data-parallel over batch (2) x tensor-parallel over heads (4 groups
of 4 heads) = 8 cores. Host sums the 4 partial output projections per batch.

Engine plan (GPSIMD cannot touch PSUM on real HW):
  PE   S^T pairs, z accumulation, projections, stats samples, transposes
  ACT  exp (paired S blocks), row-max formatting (+its DMA queue)
  DVE  PSUM drains (q/k/v), diag masks, fused stats mask+min-reduce,
       denominator copy, z normalize divide
  Pool denominator partition-broadcast (SBUF only)
  SP   batched input DMAs, phase-3 PSUM->DRAM output (f32), row-max rows

One global software pipeline over every S-pair of every head: each step
emits [S-pair matmuls, diag masks, exp], one interleave closure (v chunk /
next head's 1024-wide projection slab / stats pack / phase-3 chunk), then
retires the z matmuls of the pair three steps back.
"""

import os
import sys

import numpy as np

for _p in ("/opt/trn_rl_repo", "/root/.axon_site/_ro/trn_rl_repo"):
    if os.path.isdir(_p) and _p not in sys.path:
        sys.path.insert(0, _p)

import concourse.bass as bass
from concourse import bacc
import concourse.tile as tile
from concourse import mybir
from concourse.masks import make_identity

F32 = mybir.dt.float32
F16 = mybir.dt.float16
BF16 = mybir.dt.bfloat16
AX = mybir.AxisListType
OP = mybir.AluOpType
AF = mybir.ActivationFunctionType

T = 2048
D = 1024
HPC = 4          # heads per core
DH = 64
NQB = T // 128   # 16
NSB = T // 512   # 4
NDC = D // 128   # 8
MARGIN = 40.0


def build_nc():
    nc = bacc.Bacc("TRN2", target_bir_lowering=False)
    xT = nc.dram_tensor("xT", [D, T], F16, kind="ExternalInput")
    wqk = nc.dram_tensor("wqk", [D, 128 * HPC], F16, kind="ExternalInput")
    wv = nc.dram_tensor("wv", [D, DH * HPC], F16, kind="ExternalInput")
    wo = nc.dram_tensor("wo", [2, 128, D], BF16, kind="ExternalInput")
    mskS = nc.dram_tensor("mskS", [128, 128], F32, kind="ExternalInput")
    mskW = nc.dram_tensor("mskW", [128, 512], F32, kind="ExternalInput")
    mskT = nc.dram_tensor("mskT", [128, 128], F32, kind="ExternalInput")
    ones = nc.dram_tensor("ones", [1, T], F16, kind="ExternalInput")
    outT = nc.dram_tensor("outT", [D, T], BF16, kind="ExternalOutput")

    with tile.TileContext(nc) as tc:
        with (
            tc.tile_pool(name="const", bufs=1) as constp,
            tc.tile_pool(name="big", bufs=1) as bigp,
            tc.tile_pool(name="sb", bufs=3) as sbp,
            tc.tile_pool(name="psB", bufs=3, space="PSUM") as psB,
            tc.tile_pool(name="psZ", bufs=2, space="PSUM") as psZ,
        ):
            # ---- persistent SBUF ----
            xT_sb = bigp.tile([128, NDC, T], F16, tag="xT")
            wqk_sb = bigp.tile([128, NDC, 128 * HPC], F16, tag="wqk")
            wv_sb = bigp.tile([128, NDC, DH * HPC], F16, tag="wv")
            wo_sb = bigp.tile([128, 2, D], BF16, tag="wo")
            mskS_sb = constp.tile([128, 128], F32, tag="mskS")
            mskW_sb = constp.tile([128, 512], F32, tag="mskW")
            mskT_sb = constp.tile([128, 128], F32, tag="mskT")
            ident = constp.tile([128, 128], F32, tag="ident")
            margin = constp.tile([NQB, 1], F32, tag="margin")
            q_sb = [bigp.tile([65, T], F16, tag=f"q{j}", name=f"q{j}") for j in range(HPC)]
            k_sb = [bigp.tile([65, T], F16, tag=f"k{j}", name=f"k{j}") for j in range(HPC)]
            v_sb = bigp.tile([128, NQB, HPC, DH + 1], BF16, tag="v")
            zT_sb = [bigp.tile([128, T], BF16, tag=f"zp{p}", name=f"zp{p}") for p in range(2)]

            xTr = xT.rearrange("(c p) t -> p c t", p=128)
            wqkr = wqk.rearrange("(c p) m -> p c m", p=128)
            nc.sync.dma_start(wqk_sb[:, :, 0:128], wqkr[:, :, 0:128])
            nc.sync.dma_start(xT_sb[:, :, 0:512], xTr[:, :, 0:512])
            nc.sync.dma_start(mskS_sb[:], mskS[:])
            nc.sync.dma_start(mskW_sb[:], mskW[:])
            nc.sync.dma_start(wv_sb[:], wv.rearrange("(c p) m -> p c m", p=128))
            nc.sync.dma_start(mskT_sb[:], mskT[:])
            nc.sync.dma_start(k_sb[0][64:65, :], ones[:])
            nc.sync.dma_start(wqk_sb[:, :, 128:512], wqkr[:, :, 128:512])
            for s in (1, 2, 3):
                nc.sync.dma_start(
                    xT_sb[:, :, s * 512 : (s + 1) * 512],
                    xTr[:, :, s * 512 : (s + 1) * 512],
                )
            for j in range(1, HPC):
                nc.sync.dma_start(k_sb[j][64:65, :], ones[:])
            nc.sync.dma_start(wo_sb[:], wo.rearrange("p k d -> k p d"))
            make_identity(nc, ident[:])
            nc.vector.memset(margin[:], -MARGIN)
            nc.vector.memset(v_sb[:], 1.0)

            # ---- phase 1a: qT/kT slabs. width = number of 512-superblocks
            def qk_slab(j, s0, width=1):
                W = 512 * width
                ps = psB.tile([128, 1024], F32, tag="mm")
                for h in range(width):
                    for c in range(NDC):
                        nc.tensor.matmul(
                            ps[:, h * 512 : (h + 1) * 512],
                            lhsT=(wqk_sb[:, c, j * 128 : (j + 1) * 128]),
                            rhs=(xT_sb[:, c, (s0 + h) * 512 : (s0 + h + 1) * 512]),
                            start=(c == 0),
                            stop=(c == NDC - 1),
                        )
                nc.vector.tensor_copy(
                    q_sb[j][0:64, s0 * 512 : s0 * 512 + W], ps[0:64, 0:W]
                )
                nc.vector.tensor_copy(
                    k_sb[j][0:64, s0 * 512 : s0 * 512 + W], ps[64:128, 0:W]
                )

            # ---- phase 1b: v natural [t, 4*65] (ones col 64 for denominators)
            def v_chunk(tb):
                ps = psB.tile([128, 512], F32, tag="mm")
                for c in range(NDC):
                    nc.tensor.matmul(
                        ps[:, 0 : DH * HPC],
                        lhsT=(xT_sb[:, c, tb * 128 : (tb + 1) * 128]),
                        rhs=(wv_sb[:, c, :]),
                        start=(c == 0),
                        stop=(c == NDC - 1),
                    )
                nc.vector.tensor_copy(
                    v_sb[:, tb, :, 0:DH],
                    ps[:, 0 : DH * HPC].rearrange("p (j e) -> p j e", j=HPC),
                )

            # ---- phase 2A: stats — negated causal row max (qb0 exact, rest
            # stride-8 sampled, right-aligned so one constant boundary mask
            # works for every qb; margin 40 covers the sampling gap)
            m_all = {}

            def stats_qb(j, qb):
                if qb == 0:
                    m_all[j] = sbp.tile(
                        [128, NQB], F32, tag=f"mall{j % 2}", name=f"mall{j}"
                    )
                ma = m_all[j]
                ps = psB.tile([128, 512], F32, tag="mm")
                if qb == 0:
                    nc.tensor.matmul(
                        ps[:, 0:128],
                        lhsT=(q_sb[j][0:64, 0:128]),
                        rhs=(k_sb[j][0:64, 0:128]),
                        start=True,
                        stop=True,
                    )
                    nc.vector.tensor_tensor(
                        ps[:, 0:128], ps[:, 0:128], mskS_sb[:], op=OP.add
                    )
                    nc.vector.tensor_reduce(
                        ma[:, 0:1], ps[:, 0:128], axis=AX.X, op=OP.max,
                        negate=True,
                    )
                else:
                    L = 128 * (qb + 1)
                    Ls = L // 8
                    kr = k_sb[j][0:64, 0:L].rearrange(
                        "p (n eight) -> p n eight", eight=8
                    )
                    nc.tensor.matmul(
                        ps[:, 512 - Ls : 512],
                        lhsT=(q_sb[j][0:64, qb * 128 : (qb + 1) * 128]),
                        rhs=kr[:, :, 0:1],
                        start=True,
                        stop=True,
                    )
                    nc.vector.tensor_tensor(
                        ps[:, 496:512], ps[:, 496:512], mskW_sb[:, 496:512],
                        op=OP.add,
                    )
                    nc.vector.tensor_reduce(
                        ma[:, qb : qb + 1], ps[:, 512 - Ls : 512], axis=AX.X,
                        op=OP.max, negate=True,
                    )

            def stats_fin(j, s=None):
                # transpose -m into row 64 of q' (as -max - MARGIN, fp16).
                # s=None: whole row via the idle SP queue (heads 1-3);
                # else one 512-col chunk via the ACT queue (head 0 warmup).
                lo, n = (0, NQB) if s is None else (4 * s, 4)
                pm = psB.tile([128, 512], F32, tag="mm")
                nc.tensor.transpose(
                    pm[0:n, 0:128], m_all[j][:, lo : lo + n], ident[:]
                )
                mT = sbp.tile([NQB, 128], F16, tag="mT", bufs=2)
                nc.scalar.activation(
                    mT[0:n, :], pm[0:n, 0:128], AF.Identity, bias=margin[0:n, :]
                )
                dst = q_sb[j][64:65, lo * 128 : (lo + n) * 128]
                if s is None:
                    nc.sync.dma_start(dst, mT[0:n, :])
                else:
                    nc.scalar.dma_start(dst, mT[0:n, :])

            # ---- phase 3: one [128d, 512t] out^T chunk -> o_sb; DMA per s
            o_sb = {}

            def p3_db(s, db):
                if db == 0:
                    o_sb[s] = sbp.tile(
                        [128, NDC, 512], BF16, tag="osb", bufs=2, name=f"osb{s}"
                    )
                ops = psB.tile([128, 512], F32, tag="mm")
                for p in range(2):
                    nc.tensor.matmul(
                        ops[:],
                        lhsT=(wo_sb[:, p, db * 128 : (db + 1) * 128]),
                        rhs=(zT_sb[p][:, s * 512 : (s + 1) * 512]),
                        start=(p == 0),
                        stop=(p == 1),
                    )
                if db % 2 == 0:
                    nc.scalar.activation(o_sb[s][:, db, :], ops[:], AF.Copy)
                else:
                    nc.vector.tensor_copy(o_sb[s][:, db, :], ops[:])

            def p3_out(s):
                nc.sync.dma_start(
                    outT.rearrange("(g p) t -> p g t", p=128)[
                        :, :, s * 512 : (s + 1) * 512
                    ],
                    o_sb[s][:],
                )

            # ---- phase 2B: the global S-pair pipeline ----------------------
            zps_t = {}
            pending = []

            def retire():
                u, pT = pending.pop(0)
                j, s = u["j"], u["s"]
                nkb = 4 * s + 4
                if (j, s) not in zps_t:
                    zps_t[(j, s)] = psZ.tile(
                        [65, 512], F32, tag="zz", name=f"zps{j}_{s}"
                    )
                zps = zps_t[(j, s)]
                for kb, w, off in u["z"]:
                    nc.tensor.matmul(
                        zps[:, 512 - w : 512],
                        lhsT=v_sb[:, kb, j, :],
                        rhs=pT[:, off : off + w],
                        start=(kb == 0),
                        stop=(kb == nkb - 1),
                        skip_group_check=True,
                    )
                if u["close"]:
                    r1 = sbp.tile([1, 512], F32, tag="r1", bufs=2)
                    nc.vector.reciprocal(r1[:], zps[64:65, :])
                    rb = sbp.tile([64, 512], F32, tag="rb", bufs=2)
                    nc.gpsimd.partition_broadcast(rb[:], r1[:])
                    p, po = j // 2, 64 * (j % 2)
                    nc.vector.tensor_mul(
                        zT_sb[p][po : po + 64, s * 512 : (s + 1) * 512],
                        zps[0:64, :],
                        rb[:],
                    )
                    del zps_t[(j, s)]

            def step(u, il):
                j, s, a, b = u["j"], u["s"], u["a"], u["b"]
                wa = 512 if a < 4 * s else 512 - 128 * (a - 4 * s)
                wb = 512 if b < 4 * s else 512 - 128 * (b - 4 * s)
                sps = psB.tile([128, 1024], F32, tag="mm")
                nc.tensor.matmul(
                    sps[:, 0:wa],
                    lhsT=(k_sb[j][0:65, a * 128 : (a + 1) * 128]),
                    rhs=(q_sb[j][0:65, s * 512 + 512 - wa : (s + 1) * 512]),
                    start=True,
                    stop=True,
                )
                nc.tensor.matmul(
                    sps[:, wa : wa + wb],
                    lhsT=(k_sb[j][0:65, b * 128 : (b + 1) * 128]),
                    rhs=(q_sb[j][0:65, s * 512 + 512 - wb : (s + 1) * 512]),
                    start=True,
                    stop=True,
                )
                if a >= 4 * s:
                    nc.vector.tensor_tensor(
                        sps[:, 0:128], sps[:, 0:128], mskT_sb[:], op=OP.add
                    )
                if b >= 4 * s:
                    nc.vector.tensor_tensor(
                        sps[:, wa : wa + 128],
                        sps[:, wa : wa + 128],
                        mskT_sb[:], op=OP.add,
                    )
                pT = sbp.tile([128, 1024], BF16, tag="pT", bufs=5)
                nc.scalar.activation(pT[:, 0 : wa + wb], sps[:, 0 : wa + wb], AF.Exp)
                if il is not None:
                    il()
                u["z"] = [(a, wa, 0), (b, wb, wa)]
                pending.append((u, pT))
                if len(pending) > 3:
                    retire()

            def head_units(j):
                us = []
                for s in range(NSB):
                    nkb = 4 * s + 4
                    for a in range(0, nkb, 2):
                        us.append(
                            {"j": j, "s": s, "a": a, "b": a + 1,
                             "close": a + 2 == nkb}
                        )
                return us

            # ---- emission schedule ----
            # front: head-0 projections (512-wide for latency) with head-0
            # stats + per-superblock row-max finalize woven in
            for s in range(NSB):
                qk_slab(0, s, width=1)
                for qb in range(4 * s, 4 * s + 4):
                    stats_qb(0, qb)
                stats_fin(0, s=s)

            def vp(t0, t1):
                def fn():
                    v_chunk(t0)
                    v_chunk(t1)
                return fn

            def qs2(jn, h):
                return lambda: qk_slab(jn, 2 * h, width=2)

            def st4(jn, lo):
                def fn():
                    for qb in range(lo, lo + 4):
                        stats_qb(jn, qb)
                return fn

            def sfin(jn):
                return lambda: stats_fin(jn)

            def p3p(s, db0):
                def fn():
                    p3_db(s, db0)
                    p3_db(s, db0 + 1)
                return fn

            il_map = {
                0: [vp(0, 1), vp(2, 3), qs2(1, 0), qs2(1, 1), vp(4, 5),
                    vp(6, 7), st4(1, 0), st4(1, 4), vp(8, 9), vp(10, 11),
                    st4(1, 8), st4(1, 12), sfin(1), vp(12, 13), vp(14, 15),
                    None, None, None, None, None],
                1: [qs2(2, 0), qs2(2, 1), None, None, None, None, st4(2, 0),
                    st4(2, 4), st4(2, 8), st4(2, 12), sfin(2), None, None,
                    None, None, None, None, None, None, None],
                2: [qs2(3, 0), qs2(3, 1), None, None, None, None, st4(3, 0),
                    st4(3, 4), st4(3, 8), st4(3, 12), sfin(3), None, None,
                    None, None, None, None, None, None, None],
                3: [None, None, None, None, None,
                    p3p(0, 0), p3p(0, 2), p3p(0, 4), p3p(0, 6),
                    lambda: p3_out(0),
                    p3p(1, 0), p3p(1, 2), p3p(1, 4), p3p(1, 6),
                    lambda: p3_out(1), None,
                    p3p(2, 0), p3p(2, 2), p3p(2, 4), p3p(2, 6)],
            }

            for j in range(HPC):
                il_list = il_map[j]
                for i, u in enumerate(head_units(j)):
                    step(u, il_list[i])
            while pending:
                retire()
            # tail: last superblock of the output projection
            p3_out(2)
            for db0 in (0, 2, 4, 6):
                p3_db(NSB - 1, db0)
                p3_db(NSB - 1, db0 + 1)
            p3_out(NSB - 1)
    nc.compile()
    return nc


_NC = None


def _get_nc():
    global _NC
    if _NC is None:
        _NC = build_nc()
    return _NC


def _make_in_maps(inputs):
    x = np.ascontiguousarray(np.asarray(inputs["residual_stream"], dtype=np.float32))
    W_Q = np.asarray(inputs["W_Q"], dtype=np.float32)
    W_K = np.asarray(inputs["W_K"], dtype=np.float32)
    W_V = np.asarray(inputs["W_V"], dtype=np.float32)
    W_O = np.asarray(inputs["W_output"], dtype=np.float32)

    try:
        import ml_dtypes
        bf16 = ml_dtypes.bfloat16
    except ImportError:
        bf16 = np.float32

    qi = np.arange(128)
    mskS = np.where(qi[None, :] <= qi[:, None], 0.0, -1e9).astype(np.float32)  # [q,k]
    mskT = np.ascontiguousarray(mskS.T)  # [k,q]
    sj = np.arange(16)
    mskS8 = np.where(8 * sj[None, :] <= qi[:, None], 0.0, -1e9).astype(np.float32)
    mskW = np.zeros((128, 512), np.float32)
    mskW[:, 496:512] = mskS8  # right-aligned stride-8 causal boundary

    in_maps = []
    for c in range(8):
        b, hg = c // 4, c % 4
        hs = [4 * hg + j for j in range(HPC)]
        wqk_h = np.concatenate(
            [np.concatenate([W_Q[h] / 8.0, W_K[h]], axis=1) for h in hs], axis=1
        )  # [1024, 512]
        wv_h = np.concatenate([W_V[h] for h in hs], axis=1)  # [1024, 256]
        wo_h = np.stack(
            [np.concatenate([W_O[hs[2 * p]], W_O[hs[2 * p + 1]]], axis=0)
             for p in range(2)]
        )  # [2, 128, 1024]
        in_maps.append(
            {
                "xT": np.ascontiguousarray(x[b].T).astype(np.float16),
                "wqk": np.ascontiguousarray(wqk_h).astype(np.float16),
                "wv": np.ascontiguousarray(wv_h).astype(np.float16),
                "wo": np.ascontiguousarray(wo_h).astype(bf16),
                "mskS": mskS,
                "mskW": mskW,
                "mskT": mskT,
                "ones": np.ones((1, T), np.float16),
            }
        )
    return in_maps


def _postprocess(results, inputs):
    W_O = np.asarray(inputs["W_output"], dtype=np.float32)
    b_V = np.asarray(inputs["b_V"], dtype=np.float32)
    b_out = np.asarray(inputs["b_output"], dtype=np.float32)
    out = np.zeros((2, T, D), dtype=np.float32)
    for c in range(8):
        out[c // 4] += results[c]["outT"].T.astype(np.float32)
    # z = P @ v + b_V (P rows sum to 1) -> fold b_V through W_O on the host
    const = np.einsum("he,hed->d", b_V, W_O) + b_out
    out += const[None, None, :]
    return out


def kernel(**inputs):
    from concourse.bass_utils import run_bass_kernel_spmd

    nc = _get_nc()
    res = run_bass_kernel_spmd(nc, _make_in_maps(inputs), core_ids=list(range(8)))
    return _postprocess(res.results, inputs)


def kernel_traced(**inputs):
    """Returns (output, exec_time_ns or None) using a traced run."""
    from concourse.bass_utils import run_bass_kernel_spmd

    nc = _get_nc()
    res = run_bass_kernel_spmd(
        nc, _make_in_maps(inputs), core_ids=list(range(8)), trace=True
    )
    return _postprocess(res.results, inputs), res.exec_time_ns


# revision 3
# speedup vs baseline: 1.0288x; 1.0288x over previous
"""Multi-head causal attention on 8 TRN2 NeuronCores — v5.

Sharding: data-parallel over batch (2) x tensor-parallel over heads (4 groups
of 4 heads) = 8 cores. Host sums the 4 partial output projections per batch.

Engine plan (GPSIMD cannot touch PSUM on real HW):
  PE   S^T pairs, z accumulation, projections, stats samples, transposes
  ACT  exp (paired S blocks), row-max formatting (+its DMA queue)
  DVE  PSUM drains (q/k/v), diag masks, fused stats mask+min-reduce,
       denominator copy, z normalize divide
  Pool denominator partition-broadcast (SBUF only)
  SP   batched input DMAs, phase-3 PSUM->DRAM output (f32), row-max rows

One global software pipeline over every S-pair of every head: each step
emits [S-pair matmuls, diag masks, exp], one interleave closure (v chunk /
next head's 1024-wide projection slab / stats pack / phase-3 chunk), then
retires the z matmuls of the pair three steps back.
"""

import os
import sys

import numpy as np

for _p in ("/opt/trn_rl_repo", "/root/.axon_site/_ro/trn_rl_repo"):
    if os.path.isdir(_p) and _p not in sys.path:
        sys.path.insert(0, _p)

import concourse.bass as bass
from concourse import bacc
import concourse.tile as tile
from concourse import mybir
from concourse.masks import make_identity

F32 = mybir.dt.float32
F16 = mybir.dt.float16
BF16 = mybir.dt.bfloat16
AX = mybir.AxisListType
OP = mybir.AluOpType
AF = mybir.ActivationFunctionType

T = 2048
D = 1024
HPC = 4          # heads per core
DH = 64
NQB = T // 128   # 16
NSB = T // 512   # 4
NDC = D // 128   # 8
MARGIN = 40.0


def build_nc():
    nc = bacc.Bacc("TRN2", target_bir_lowering=False)
    xT = nc.dram_tensor("xT", [D, T], F16, kind="ExternalInput")
    wqk = nc.dram_tensor("wqk", [D, 128 * HPC], F16, kind="ExternalInput")
    wv = nc.dram_tensor("wv", [D, DH * HPC], F16, kind="ExternalInput")
    wo = nc.dram_tensor("wo", [2, 128, D], BF16, kind="ExternalInput")
    ones = nc.dram_tensor("ones", [1, T], F16, kind="ExternalInput")
    outT = nc.dram_tensor("outT", [D, T], BF16, kind="ExternalOutput")

    with tile.TileContext(nc) as tc:
        with (
            tc.tile_pool(name="const", bufs=1) as constp,
            tc.tile_pool(name="big", bufs=1) as bigp,
            tc.tile_pool(name="sb", bufs=3) as sbp,
            tc.tile_pool(name="psB", bufs=3, space="PSUM") as psB,
            tc.tile_pool(name="psZ", bufs=2, space="PSUM") as psZ,
        ):
            # ---- persistent SBUF ----
            xT_sb = bigp.tile([128, NDC, T], F16, tag="xT")
            wqk_sb = bigp.tile([128, NDC, 128 * HPC], F16, tag="wqk")
            wv_sb = bigp.tile([128, NDC, DH * HPC], F16, tag="wv")
            wo_sb = bigp.tile([128, 2, D], BF16, tag="wo")
            mskG_sb = constp.tile([128, NQB, 128], F32, tag="mskG")
            mskT_sb = constp.tile([128, 128], F32, tag="mskT")
            ident = constp.tile([128, 128], F32, tag="ident")
            margin = constp.tile([NQB, 1], F32, tag="margin")
            q_sb = [bigp.tile([65, T], F16, tag=f"q{j}", name=f"q{j}") for j in range(HPC)]
            k_sb = [bigp.tile([65, T], F16, tag=f"k{j}", name=f"k{j}") for j in range(HPC)]
            v_sb = bigp.tile([128, NQB, HPC, DH + 1], BF16, tag="v")
            zT_sb = [bigp.tile([128, T], BF16, tag=f"zp{p}", name=f"zp{p}") for p in range(2)]

            xTr = xT.rearrange("(c p) t -> p c t", p=128)
            wqkr = wqk.rearrange("(c p) m -> p c m", p=128)
            nc.sync.dma_start(wqk_sb[:, :, 0:128], wqkr[:, :, 0:128])
            nc.sync.dma_start(xT_sb[:, 0:4, 0:512], xTr[:, 0:4, 0:512])
            nc.sync.dma_start(xT_sb[:, 4:8, 0:512], xTr[:, 4:8, 0:512])
            nc.sync.dma_start(wv_sb[:], wv.rearrange("(c p) m -> p c m", p=128))
            nc.sync.dma_start(k_sb[0][64:65, :], ones[:])
            nc.sync.dma_start(wqk_sb[:, :, 128:512], wqkr[:, :, 128:512])
            for s in (1, 2, 3):
                nc.sync.dma_start(
                    xT_sb[:, :, s * 512 : (s + 1) * 512],
                    xTr[:, :, s * 512 : (s + 1) * 512],
                )
            for j in range(1, HPC):
                nc.sync.dma_start(k_sb[j][64:65, :], ones[:])
            nc.sync.dma_start(wo_sb[:], wo.rearrange("p k d -> k p d"))
            make_identity(nc, ident[:])
            nc.vector.memset(margin[:], -MARGIN)
            nc.gpsimd.memset(v_sb[:, :, :, DH : DH + 1], 1.0)
            # on-device masks (Pool is otherwise idle):
            # mskT[k, q] = 0 if k <= q else -1e9
            nc.gpsimd.memset(mskT_sb[:], 0.0)
            nc.gpsimd.affine_select(
                out=mskT_sb[:], in_=mskT_sb[:], compare_op=OP.is_ge,
                fill=-1e9, base=0, pattern=[[1, 128]], channel_multiplier=-1,
            )
            # mskG block qb: 0 if i*(qb+1) <= 128*qb + p else -1e9
            nc.gpsimd.memset(mskG_sb[:], 0.0)
            for qb in range(NQB):
                nc.gpsimd.affine_select(
                    out=mskG_sb[:, qb, :], in_=mskG_sb[:, qb, :],
                    compare_op=OP.is_ge, fill=-1e9, base=128 * qb,
                    pattern=[[-(qb + 1), 128]], channel_multiplier=1,
                )

            # ---- phase 1a: qT/kT slabs. width = number of 512-superblocks
            def qk_slab(j, s0, width=1):
                W = 512 * width
                ps = psB.tile([128, 1024], F32, tag="mm")
                for h in range(width):
                    for c in range(NDC):
                        nc.tensor.matmul(
                            ps[:, h * 512 : (h + 1) * 512],
                            lhsT=(wqk_sb[:, c, j * 128 : (j + 1) * 128]),
                            rhs=(xT_sb[:, c, (s0 + h) * 512 : (s0 + h + 1) * 512]),
                            start=(c == 0),
                            stop=(c == NDC - 1),
                        )
                nc.scalar.activation(
                    q_sb[j][0:64, s0 * 512 : s0 * 512 + W], ps[0:64, 0:W],
                    AF.Copy,
                )
                nc.vector.tensor_copy(
                    k_sb[j][0:64, s0 * 512 : s0 * 512 + W], ps[64:128, 0:W]
                )

            # ---- phase 1b: v natural [t, 4*65] (ones col 64 for denominators)
            def v_chunk(tb):
                ps = psB.tile([128, 512], F32, tag="mm")
                for c in range(NDC):
                    nc.tensor.matmul(
                        ps[:, 0 : DH * HPC],
                        lhsT=(xT_sb[:, c, tb * 128 : (tb + 1) * 128]),
                        rhs=(wv_sb[:, c, :]),
                        start=(c == 0),
                        stop=(c == NDC - 1),
                    )
                nc.vector.tensor_copy(
                    v_sb[:, tb, :, 0:DH],
                    ps[:, 0 : DH * HPC].rearrange("p (j e) -> p j e", j=HPC),
                )

            # ---- phase 2A: stats — negated causal row max. Each 128-row
            # q-block qb samples its causal range at stride (qb+1), giving a
            # uniform 128 samples; 4 blocks share one PSUM tile, one boundary
            # mask add and one 3-D max-reduce. Margin 40 covers sampling.
            m_all = {}

            def stats_group(j, g):
                if g == 0:
                    m_all[j] = sbp.tile(
                        [128, NQB], F32, tag=f"mall{j % 2}", name=f"mall{j}"
                    )
                ma = m_all[j]
                ps = psB.tile([128, 512], F32, tag="mm")
                for i in range(4):
                    qb = 4 * g + i
                    L = 128 * (qb + 1)
                    if qb == 0:
                        rhs = k_sb[j][0:64, 0:128]
                    else:
                        kr = k_sb[j][0:64, 0:L].rearrange(
                            "p (n st) -> p n st", st=qb + 1
                        )
                        rhs = kr[:, :, 0:1]
                    nc.tensor.matmul(
                        ps[:, i * 128 : (i + 1) * 128],
                        lhsT=(q_sb[j][0:64, qb * 128 : (qb + 1) * 128]),
                        rhs=rhs,
                        start=True,
                        stop=True,
                    )
                pv = ps[:, 0:512].rearrange("p (n w) -> p n w", w=128)
                gv = mskG_sb[:, 4 * g : 4 * g + 4, :]
                if g == 0:
                    nc.vector.tensor_tensor(
                        ps[:, 0:512], ps[:, 0:512],
                        mskG_sb[:, 0:4, :].rearrange("p n w -> p (n w)"),
                        op=OP.add,
                    )
                else:
                    nc.vector.tensor_tensor(
                        pv[:, :, 96:128], pv[:, :, 96:128], gv[:, :, 96:128],
                        op=OP.add,
                    )
                nc.vector.tensor_reduce(
                    ma[:, 4 * g : 4 * g + 4], pv[:], axis=AX.X,
                    op=OP.max, negate=True,
                )

            def stats_fin(j, s=None):
                # transpose -m into row 64 of q' (as -max - MARGIN, fp16).
                # s=None: whole row via the idle SP queue (heads 1-3);
                # else one 512-col chunk via the ACT queue (head 0 warmup).
                lo, n = (0, NQB) if s is None else (4 * s, 4)
                pm = psB.tile([128, 512], F32, tag="mm")
                nc.tensor.transpose(
                    pm[0:n, 0:128], m_all[j][:, lo : lo + n], ident[:]
                )
                mT = sbp.tile([NQB, 128], F16, tag="mT", bufs=2)
                nc.scalar.activation(
                    mT[0:n, :], pm[0:n, 0:128], AF.Identity, bias=margin[0:n, :]
                )
                dst = q_sb[j][64:65, lo * 128 : (lo + n) * 128]
                if s is None:
                    nc.sync.dma_start(dst, mT[0:n, :])
                else:
                    nc.scalar.dma_start(dst, mT[0:n, :])

            # ---- phase 3: one [128d, 512t] out^T chunk -> o_sb; DMA per s
            o_sb = {}

            p3_alt = [0]

            def p3_pair(s, db0):
                if db0 == 0:
                    o_sb[s] = sbp.tile(
                        [128, NDC, 512], BF16, tag="osb", bufs=2, name=f"osb{s}"
                    )
                ops = psB.tile([128, 1024], F32, tag="mm")
                for i in range(2):
                    for p in range(2):
                        nc.tensor.matmul(
                            ops[:, i * 512 : (i + 1) * 512],
                            lhsT=(wo_sb[:, p, (db0 + i) * 128 : (db0 + i + 1) * 128]),
                            rhs=(zT_sb[p][:, s * 512 : (s + 1) * 512]),
                            start=(p == 0),
                            stop=(p == 1),
                        )
                dst = o_sb[s][:, db0 : db0 + 2, :].rearrange("p n w -> p (n w)")
                p3_alt[0] ^= 1
                if p3_alt[0]:
                    nc.scalar.activation(dst, ops[:], AF.Copy)
                else:
                    nc.vector.tensor_copy(dst, ops[:])

            def p3_out(s):
                nc.sync.dma_start(
                    outT.rearrange("(g p) t -> p g t", p=128)[
                        :, :, s * 512 : (s + 1) * 512
                    ],
                    o_sb[s][:],
                )

            # ---- phase 2B: the global S-pair pipeline ----------------------
            zps_t = {}
            pending = []

            def retire():
                u, pT = pending.pop(0)
                j, s = u["j"], u["s"]
                nkb = 4 * s + 4
                if (j, s) not in zps_t:
                    zps_t[(j, s)] = psZ.tile(
                        [65, 512], F32, tag="zz", name=f"zps{j}_{s}"
                    )
                zps = zps_t[(j, s)]
                for kb, w, off in u["z"]:
                    nc.tensor.matmul(
                        zps[:, 512 - w : 512],
                        lhsT=v_sb[:, kb, j, :],
                        rhs=pT[:, off : off + w],
                        start=(kb == 0),
                        stop=(kb == nkb - 1),
                        skip_group_check=True,
                    )
                if u["close"]:
                    r1 = sbp.tile([1, 512], F32, tag="r1", bufs=2)
                    nc.vector.reciprocal(r1[:], zps[64:65, :])
                    rb = sbp.tile([64, 512], F32, tag="rb", bufs=2)
                    nc.gpsimd.partition_broadcast(rb[:], r1[:])
                    p, po = j // 2, 64 * (j % 2)
                    nc.vector.tensor_mul(
                        zT_sb[p][po : po + 64, s * 512 : (s + 1) * 512],
                        zps[0:64, :],
                        rb[:],
                    )
                    del zps_t[(j, s)]

            def step(u, il):
                j, s, a, b = u["j"], u["s"], u["a"], u["b"]
                wa = 512 if a < 4 * s else 512 - 128 * (a - 4 * s)
                wb = 512 if b < 4 * s else 512 - 128 * (b - 4 * s)
                sps = psB.tile([128, 1024], F32, tag="mm")
                nc.tensor.matmul(
                    sps[:, 0:wa],
                    lhsT=(k_sb[j][0:65, a * 128 : (a + 1) * 128]),
                    rhs=(q_sb[j][0:65, s * 512 + 512 - wa : (s + 1) * 512]),
                    start=True,
                    stop=True,
                )
                nc.tensor.matmul(
                    sps[:, wa : wa + wb],
                    lhsT=(k_sb[j][0:65, b * 128 : (b + 1) * 128]),
                    rhs=(q_sb[j][0:65, s * 512 + 512 - wb : (s + 1) * 512]),
                    start=True,
                    stop=True,
                )
                if a >= 4 * s:
                    nc.vector.tensor_tensor(
                        sps[:, 0:128], sps[:, 0:128], mskT_sb[:], op=OP.add
                    )
                if b >= 4 * s:
                    nc.vector.tensor_tensor(
                        sps[:, wa : wa + 128],
                        sps[:, wa : wa + 128],
                        mskT_sb[:], op=OP.add,
                    )
                pT = sbp.tile([128, 1024], BF16, tag="pT", bufs=5)
                nc.scalar.activation(pT[:, 0 : wa + wb], sps[:, 0 : wa + wb], AF.Exp)
                if il is not None:
                    il()
                u["z"] = [(a, wa, 0), (b, wb, wa)]
                pending.append((u, pT))
                if len(pending) > 3:
                    retire()

            def head_units(j):
                us = []
                for s in range(NSB):
                    nkb = 4 * s + 4
                    for a in range(0, nkb, 2):
                        us.append(
                            {"j": j, "s": s, "a": a, "b": a + 1,
                             "close": a + 2 == nkb}
                        )
                return us

            # ---- emission schedule ----
            # front: head-0 projections (512-wide for latency) with head-0
            # stats + per-superblock row-max finalize woven in
            for s in (0, 1):
                qk_slab(0, s, width=1)
                stats_group(0, s)
                stats_fin(0, s=s)

            def vp(t0, t1):
                def fn():
                    v_chunk(t0)
                    v_chunk(t1)
                return fn

            def qs2(jn, h):
                return lambda: qk_slab(jn, 2 * h, width=2)

            def stg(jn, g):
                return lambda: stats_group(jn, g)

            def qk0s(s):
                return lambda: qk_slab(0, s, width=1)

            def stgf0(s):
                def fn():
                    stats_group(0, s)
                    stats_fin(0, s=s)
                return fn

            def sfin(jn):
                return lambda: stats_fin(jn)

            def p3p(s, db0):
                return lambda: p3_pair(s, db0)

            il_map = {
                0: [vp(0, 1), vp(2, 3), qk0s(2), stgf0(2), vp(4, 5),
                    vp(6, 7), qk0s(3), stgf0(3), vp(8, 9), qs2(1, 0),
                    vp(10, 11), qs2(1, 1), stg(1, 0), stg(1, 1), stg(1, 2),
                    stg(1, 3), sfin(1), vp(12, 13), vp(14, 15), None],
                1: [None, None, qs2(2, 0), qs2(2, 1), None, None, stg(2, 0),
                    None, stg(2, 1), None, stg(2, 2), None, stg(2, 3),
                    sfin(2), None, None, None, None, None, None],
                2: [None, None, qs2(3, 0), qs2(3, 1), None, None, stg(3, 0),
                    None, stg(3, 1), None, stg(3, 2), None, stg(3, 3),
                    sfin(3), None, None, None, None, None, None],
                3: [None, None, None, None, None,
                    p3p(0, 0), p3p(0, 2), p3p(0, 4), p3p(0, 6),
                    lambda: p3_out(0),
                    p3p(1, 0), p3p(1, 2), p3p(1, 4), p3p(1, 6),
                    lambda: p3_out(1), None,
                    p3p(2, 0), p3p(2, 2), p3p(2, 4), p3p(2, 6)],
            }

            for j in range(HPC):
                il_list = il_map[j]
                for i, u in enumerate(head_units(j)):
                    step(u, il_list[i])
            while pending:
                retire()
            # tail: last superblock of the output projection
            p3_out(2)
            p3_pair(3, 0)
            p3_pair(3, 2)
            nc.sync.dma_start(
                outT.rearrange("(g p) t -> p g t", p=128)[:, 0:4, 1536:2048],
                o_sb[3][:, 0:4, :],
            )
            p3_pair(3, 4)
            p3_pair(3, 6)
            nc.sync.dma_start(
                outT.rearrange("(g p) t -> p g t", p=128)[:, 4:8, 1536:2048],
                o_sb[3][:, 4:8, :],
            )
    nc.compile()
    return nc


_NC = None


def _get_nc():
    global _NC
    if _NC is None:
        _NC = build_nc()
    return _NC


def _make_in_maps(inputs):
    x = np.ascontiguousarray(np.asarray(inputs["residual_stream"], dtype=np.float32))
    W_Q = np.asarray(inputs["W_Q"], dtype=np.float32)
    W_K = np.asarray(inputs["W_K"], dtype=np.float32)
    W_V = np.asarray(inputs["W_V"], dtype=np.float32)
    W_O = np.asarray(inputs["W_output"], dtype=np.float32)

    try:
        import ml_dtypes
        bf16 = ml_dtypes.bfloat16
    except ImportError:
        bf16 = np.float32

    qi = np.arange(128)
    mskS = np.where(qi[None, :] <= qi[:, None], 0.0, -1e9).astype(np.float32)  # [q,k]
    mskT = np.ascontiguousarray(mskS.T)  # [k,q]
    si = np.arange(128)
    mskG = np.zeros((128, NQB, 128), np.float32)
    for qb in range(NQB):
        mskG[:, qb, :] = np.where(
            si[None, :] * (qb + 1) <= 128 * qb + qi[:, None], 0.0, -1e9
        )
    mskG = np.ascontiguousarray(mskG.reshape(128, NQB * 128))

    in_maps = []
    for c in range(8):
        b, hg = c // 4, c % 4
        hs = [4 * hg + j for j in range(HPC)]
        wqk_h = np.concatenate(
            [np.concatenate([W_Q[h] / 8.0, W_K[h]], axis=1) for h in hs], axis=1
        )  # [1024, 512]
        wv_h = np.concatenate([W_V[h] for h in hs], axis=1)  # [1024, 256]
        wo_h = np.stack(
            [np.concatenate([W_O[hs[2 * p]], W_O[hs[2 * p + 1]]], axis=0)
             for p in range(2)]
        )  # [2, 128, 1024]
        in_maps.append(
            {
                "xT": np.ascontiguousarray(x[b].T).astype(np.float16),
                "wqk": np.ascontiguousarray(wqk_h).astype(np.float16),
                "wv": np.ascontiguousarray(wv_h).astype(np.float16),
                "wo": np.ascontiguousarray(wo_h).astype(bf16),
                "ones": np.ones((1, T), np.float16),
            }
        )
    return in_maps


def _postprocess(results, inputs):
    W_O = np.asarray(inputs["W_output"], dtype=np.float32)
    b_V = np.asarray(inputs["b_V"], dtype=np.float32)
    b_out = np.asarray(inputs["b_output"], dtype=np.float32)
    out = np.zeros((2, T, D), dtype=np.float32)
    for c in range(8):
        out[c // 4] += results[c]["outT"].T.astype(np.float32)
    # z = P @ v + b_V (P rows sum to 1) -> fold b_V through W_O on the host
    const = np.einsum("he,hed->d", b_V, W_O) + b_out
    out += const[None, None, :]
    return out


def kernel(**inputs):
    from concourse.bass_utils import run_bass_kernel_spmd

    nc = _get_nc()
    res = run_bass_kernel_spmd(nc, _make_in_maps(inputs), core_ids=list(range(8)))
    return _postprocess(res.results, inputs)


def kernel_traced(**inputs):
    """Returns (output, exec_time_ns or None) using a traced run."""
    from concourse.bass_utils import run_bass_kernel_spmd

    nc = _get_nc()
    res = run_bass_kernel_spmd(
        nc, _make_in_maps(inputs), core_ids=list(range(8)), trace=True
    )
    return _postprocess(res.results, inputs), res.exec_time_ns


# revision 4
# speedup vs baseline: 1.0446x; 1.0154x over previous
"""Multi-head causal attention on 8 TRN2 NeuronCores — v5.

Sharding: data-parallel over batch (2) x tensor-parallel over heads (4 groups
of 4 heads) = 8 cores. Host sums the 4 partial output projections per batch.

Engine plan (GPSIMD cannot touch PSUM on real HW):
  PE   S^T pairs, z accumulation, projections, stats samples, transposes
  ACT  exp (paired S blocks), row-max formatting (+its DMA queue)
  DVE  PSUM drains (q/k/v), diag masks, fused stats mask+min-reduce,
       denominator copy, z normalize divide
  Pool denominator partition-broadcast (SBUF only)
  SP   batched input DMAs, phase-3 PSUM->DRAM output (f32), row-max rows

One global software pipeline over every S-pair of every head: each step
emits [S-pair matmuls, diag masks, exp], one interleave closure (v chunk /
next head's 1024-wide projection slab / stats pack / phase-3 chunk), then
retires the z matmuls of the pair three steps back.
"""

import os
import sys

import numpy as np

for _p in ("/opt/trn_rl_repo", "/root/.axon_site/_ro/trn_rl_repo"):
    if os.path.isdir(_p) and _p not in sys.path:
        sys.path.insert(0, _p)

import concourse.bass as bass
from concourse import bacc
import concourse.tile as tile
from concourse import mybir
from concourse.masks import make_identity

F32 = mybir.dt.float32
F16 = mybir.dt.float16
BF16 = mybir.dt.bfloat16
AX = mybir.AxisListType
OP = mybir.AluOpType
AF = mybir.ActivationFunctionType

T = 2048
D = 1024
HPC = 4          # heads per core
DH = 64
NQB = T // 128   # 16
NSB = T // 512   # 4
NDC = D // 128   # 8
MARGIN = 40.0


def build_nc():
    nc = bacc.Bacc("TRN2", target_bir_lowering=False)
    xT = nc.dram_tensor("xT", [D, T], F16, kind="ExternalInput")
    wqk = nc.dram_tensor("wqk", [D, 128 * HPC], F16, kind="ExternalInput")
    wv = nc.dram_tensor("wv", [D, DH * HPC], F16, kind="ExternalInput")
    wo = nc.dram_tensor("wo", [2, 128, D], BF16, kind="ExternalInput")
    ones = nc.dram_tensor("ones", [1, T], F16, kind="ExternalInput")
    outT = nc.dram_tensor("outT", [D, T], BF16, kind="ExternalOutput")

    with tile.TileContext(nc) as tc:
        with (
            tc.tile_pool(name="const", bufs=1) as constp,
            tc.tile_pool(name="big", bufs=1) as bigp,
            tc.tile_pool(name="sb", bufs=3) as sbp,
            tc.tile_pool(name="psB", bufs=3, space="PSUM") as psB,
            tc.tile_pool(name="psZ", bufs=2, space="PSUM") as psZ,
        ):
            # ---- persistent SBUF ----
            xT_sb = bigp.tile([128, NDC, T], F16, tag="xT")
            wqk_sb = bigp.tile([128, NDC, 128 * HPC], F16, tag="wqk")
            wv_sb = bigp.tile([128, NDC, DH * HPC], F16, tag="wv")
            wo_sb = bigp.tile([128, 2, D], BF16, tag="wo")
            mskG_sb = constp.tile([128, NQB, 128], F32, tag="mskG")
            mskT_sb = constp.tile([128, 128], F32, tag="mskT")
            ident = constp.tile([128, 128], F32, tag="ident")
            margin = constp.tile([NQB, 1], F32, tag="margin")
            q_sb = [bigp.tile([65, T], F16, tag=f"q{j}", name=f"q{j}") for j in range(HPC)]
            k_sb = [bigp.tile([65, T], F16, tag=f"k{j}", name=f"k{j}") for j in range(HPC)]
            v_sb = bigp.tile([128, NQB, HPC, DH + 1], BF16, tag="v")
            zT_sb = [bigp.tile([128, T], BF16, tag=f"zp{p}", name=f"zp{p}") for p in range(2)]

            xTr = xT.rearrange("(c p) t -> p c t", p=128)
            wqkr = wqk.rearrange("(c p) m -> p c m", p=128)
            nc.sync.dma_start(wqk_sb[:, :, 0:128], wqkr[:, :, 0:128])
            nc.sync.dma_start(xT_sb[:, 0:4, 0:512], xTr[:, 0:4, 0:512])
            nc.sync.dma_start(xT_sb[:, 4:8, 0:512], xTr[:, 4:8, 0:512])
            nc.sync.dma_start(wv_sb[:], wv.rearrange("(c p) m -> p c m", p=128))
            nc.sync.dma_start(k_sb[0][64:65, :], ones[:])
            nc.sync.dma_start(xT_sb[:, :, 512:1024], xTr[:, :, 512:1024])
            nc.sync.dma_start(wqk_sb[:, :, 128:512], wqkr[:, :, 128:512])
            for s in (2, 3):
                nc.sync.dma_start(
                    xT_sb[:, :, s * 512 : (s + 1) * 512],
                    xTr[:, :, s * 512 : (s + 1) * 512],
                )
            for j in range(1, HPC):
                nc.sync.dma_start(k_sb[j][64:65, :], ones[:])
            nc.sync.dma_start(wo_sb[:], wo.rearrange("p k d -> k p d"))
            make_identity(nc, ident[:])
            nc.vector.memset(margin[:], -MARGIN)
            nc.gpsimd.memset(v_sb[:, :, :, DH : DH + 1], 1.0)
            # on-device masks (Pool is otherwise idle):
            # mskT[k, q] = 0 if k <= q else -1e9
            nc.gpsimd.memset(mskT_sb[:], 0.0)
            nc.gpsimd.affine_select(
                out=mskT_sb[:], in_=mskT_sb[:], compare_op=OP.is_ge,
                fill=-1e9, base=0, pattern=[[1, 128]], channel_multiplier=-1,
            )
            # mskG block qb: 0 if i*(qb+1) <= 128*qb + p else -1e9
            nc.gpsimd.memset(mskG_sb[:], 0.0)
            for qb in range(NQB):
                nc.gpsimd.affine_select(
                    out=mskG_sb[:, qb, :], in_=mskG_sb[:, qb, :],
                    compare_op=OP.is_ge, fill=-1e9, base=128 * qb,
                    pattern=[[-(qb + 1), 128]], channel_multiplier=1,
                )

            # ---- phase 1a: qT/kT slabs. width = number of 512-superblocks.
            # Copies drain in 512-col chunks so critical DVE ops (diag masks,
            # normalize) never queue behind a >1us copy; q goes via ACT only
            # in the front where ACT is idle.
            def qk_slab(j, s0, width=1, q_act=True):
                ps = psB.tile([128, 1024], F32, tag="mm")
                for h in range(width):
                    for c in range(NDC):
                        nc.tensor.matmul(
                            ps[:, h * 512 : (h + 1) * 512],
                            lhsT=(wqk_sb[:, c, j * 128 : (j + 1) * 128]),
                            rhs=(xT_sb[:, c, (s0 + h) * 512 : (s0 + h + 1) * 512]),
                            start=(c == 0),
                            stop=(c == NDC - 1),
                        )
                for h in range(width):
                    lo, hi = (s0 + h) * 512, (s0 + h + 1) * 512
                    if q_act:
                        nc.scalar.activation(
                            q_sb[j][0:64, lo:hi], ps[0:64, h * 512 : (h + 1) * 512],
                            AF.Copy,
                        )
                    else:
                        nc.vector.tensor_copy(
                            q_sb[j][0:64, lo:hi], ps[0:64, h * 512 : (h + 1) * 512]
                        )
                    if q_act and width == 2:
                        nc.scalar.activation(
                            k_sb[j][0:64, lo:hi], ps[64:128, h * 512 : (h + 1) * 512],
                            AF.Copy,
                        )
                    else:
                        nc.vector.tensor_copy(
                            k_sb[j][0:64, lo:hi], ps[64:128, h * 512 : (h + 1) * 512]
                        )

            # ---- phase 1b: v natural [t, 4*65] (ones col 64 for denominators)
            def v_chunk(tb):
                ps = psB.tile([128, 512], F32, tag="mm")
                for c in range(NDC):
                    nc.tensor.matmul(
                        ps[:, 0 : DH * HPC],
                        lhsT=(xT_sb[:, c, tb * 128 : (tb + 1) * 128]),
                        rhs=(wv_sb[:, c, :]),
                        start=(c == 0),
                        stop=(c == NDC - 1),
                    )
                nc.vector.tensor_copy(
                    v_sb[:, tb, :, 0:DH],
                    ps[:, 0 : DH * HPC].rearrange("p (j e) -> p j e", j=HPC),
                )

            # ---- phase 2A: stats — negated causal row max. Each 128-row
            # q-block qb samples its causal range at stride (qb+1), giving a
            # uniform 128 samples; 4 blocks share one PSUM tile, one boundary
            # mask add and one 3-D max-reduce. Margin 40 covers sampling.
            m_all = {}

            def stats_group(j, g):
                if g == 0:
                    m_all[j] = sbp.tile(
                        [128, NQB], F32, tag=f"mall{j % 2}", name=f"mall{j}"
                    )
                ma = m_all[j]
                ps = psB.tile([128, 512], F32, tag="mm")
                for i in range(4):
                    qb = 4 * g + i
                    L = 128 * (qb + 1)
                    if qb == 0:
                        rhs = k_sb[j][0:64, 0:128]
                    else:
                        kr = k_sb[j][0:64, 0:L].rearrange(
                            "p (n st) -> p n st", st=qb + 1
                        )
                        rhs = kr[:, :, 0:1]
                    nc.tensor.matmul(
                        ps[:, i * 128 : (i + 1) * 128],
                        lhsT=(q_sb[j][0:64, qb * 128 : (qb + 1) * 128]),
                        rhs=rhs,
                        start=True,
                        stop=True,
                    )
                pv = ps[:, 0:512].rearrange("p (n w) -> p n w", w=128)
                gv = mskG_sb[:, 4 * g : 4 * g + 4, :]
                if g == 0:
                    nc.vector.tensor_tensor(
                        ps[:, 0:512], ps[:, 0:512],
                        mskG_sb[:, 0:4, :].rearrange("p n w -> p (n w)"),
                        op=OP.add,
                    )
                else:
                    nc.vector.tensor_tensor(
                        pv[:, :, 96:128], pv[:, :, 96:128], gv[:, :, 96:128],
                        op=OP.add,
                    )
                nc.vector.tensor_reduce(
                    ma[:, 4 * g : 4 * g + 4], pv[:], axis=AX.X,
                    op=OP.max, negate=True,
                )

            def stats_fin(j, s=None):
                # transpose -m into row 64 of q' (as -max - MARGIN, fp16).
                # s=None: whole row via the idle SP queue (heads 1-3);
                # else one 512-col chunk via the ACT queue (head 0 warmup).
                lo, n = (0, NQB) if s is None else (4 * s, 4)
                pm = psB.tile([128, 512], F32, tag="mm")
                nc.tensor.transpose(
                    pm[0:n, 0:128], m_all[j][:, lo : lo + n], ident[:]
                )
                mT = sbp.tile([NQB, 128], F16, tag="mT", bufs=2)
                nc.scalar.activation(
                    mT[0:n, :], pm[0:n, 0:128], AF.Identity, bias=margin[0:n, :]
                )
                dst = q_sb[j][64:65, lo * 128 : (lo + n) * 128]
                if s is None:
                    nc.sync.dma_start(dst, mT[0:n, :])
                else:
                    nc.scalar.dma_start(dst, mT[0:n, :])

            # ---- phase 3: one [128d, 512t] out^T chunk -> o_sb; DMA per s
            o_sb = {}

            p3_alt = [0]

            def p3_pair(s, db0):
                if db0 == 0:
                    o_sb[s] = sbp.tile(
                        [128, NDC, 512], BF16, tag="osb", bufs=2, name=f"osb{s}"
                    )
                ops = psB.tile([128, 1024], F32, tag="mm")
                for i in range(2):
                    for p in range(2):
                        nc.tensor.matmul(
                            ops[:, i * 512 : (i + 1) * 512],
                            lhsT=(wo_sb[:, p, (db0 + i) * 128 : (db0 + i + 1) * 128]),
                            rhs=(zT_sb[p][:, s * 512 : (s + 1) * 512]),
                            start=(p == 0),
                            stop=(p == 1),
                        )
                dst = o_sb[s][:, db0 : db0 + 2, :].rearrange("p n w -> p (n w)")
                p3_alt[0] ^= 1
                if s == 2 or p3_alt[0]:
                    nc.scalar.activation(dst, ops[:], AF.Copy)
                else:
                    nc.vector.tensor_copy(dst, ops[:])

            def p3_out(s):
                nc.sync.dma_start(
                    outT.rearrange("(g p) t -> p g t", p=128)[
                        :, :, s * 512 : (s + 1) * 512
                    ],
                    o_sb[s][:],
                )

            # ---- phase 2B: the global S-pair pipeline ----------------------
            zps_t = {}
            pending = []

            def retire():
                u, pT = pending.pop(0)
                j, s = u["j"], u["s"]
                nkb = 4 * s + 4
                if (j, s) not in zps_t:
                    zps_t[(j, s)] = psZ.tile(
                        [65, 512], F32, tag="zz", name=f"zps{j}_{s}"
                    )
                zps = zps_t[(j, s)]
                for kb, w, off in u["z"]:
                    nc.tensor.matmul(
                        zps[:, 512 - w : 512],
                        lhsT=v_sb[:, kb, j, :],
                        rhs=pT[:, off : off + w],
                        start=(kb == 0),
                        stop=(kb == nkb - 1),
                        skip_group_check=True,
                    )
                if u["close"]:
                    r1 = sbp.tile([1, 512], F32, tag="r1", bufs=2)
                    nc.vector.reciprocal(r1[:], zps[64:65, :])
                    rb = sbp.tile([64, 512], F32, tag="rb", bufs=2)
                    nc.gpsimd.partition_broadcast(rb[:], r1[:])
                    p, po = j // 2, 64 * (j % 2)
                    nc.vector.tensor_mul(
                        zT_sb[p][po : po + 64, s * 512 : (s + 1) * 512],
                        zps[0:64, :],
                        rb[:],
                    )
                    del zps_t[(j, s)]

            def step(u, il):
                j, s, a, b = u["j"], u["s"], u["a"], u["b"]
                wa = 512 if a < 4 * s else 512 - 128 * (a - 4 * s)
                wb = 512 if b < 4 * s else 512 - 128 * (b - 4 * s)
                sps = psB.tile([128, 1024], F32, tag="mm")
                nc.tensor.matmul(
                    sps[:, 0:wa],
                    lhsT=(k_sb[j][0:65, a * 128 : (a + 1) * 128]),
                    rhs=(q_sb[j][0:65, s * 512 + 512 - wa : (s + 1) * 512]),
                    start=True,
                    stop=True,
                )
                nc.tensor.matmul(
                    sps[:, wa : wa + wb],
                    lhsT=(k_sb[j][0:65, b * 128 : (b + 1) * 128]),
                    rhs=(q_sb[j][0:65, s * 512 + 512 - wb : (s + 1) * 512]),
                    start=True,
                    stop=True,
                )
                if a >= 4 * s:
                    nc.vector.tensor_tensor(
                        sps[:, 0:128], sps[:, 0:128], mskT_sb[:], op=OP.add
                    )
                if b >= 4 * s:
                    nc.vector.tensor_tensor(
                        sps[:, wa : wa + 128],
                        sps[:, wa : wa + 128],
                        mskT_sb[:], op=OP.add,
                    )
                pT = sbp.tile([128, 1024], BF16, tag="pT", bufs=6)
                nc.scalar.activation(pT[:, 0 : wa + wb], sps[:, 0 : wa + wb], AF.Exp)
                if il is not None:
                    il()
                u["z"] = [(a, wa, 0), (b, wb, wa)]
                pending.append((u, pT))
                if len(pending) > 4:
                    retire()

            def head_units(j):
                us = []
                for s in range(NSB):
                    nkb = 4 * s + 4
                    for a in range(0, nkb, 2):
                        us.append(
                            {"j": j, "s": s, "a": a, "b": a + 1,
                             "close": a + 2 == nkb}
                        )
                return us

            # ---- emission schedule ----
            # front: head-0 projections (512-wide for latency) with head-0
            # stats + per-superblock row-max finalize woven in
            for s in (0, 1):
                qk_slab(0, s, width=1, q_act=True)
                stats_group(0, s)
                stats_fin(0, s=s)

            def vp(t0, t1):
                def fn():
                    v_chunk(t0)
                    v_chunk(t1)
                return fn

            def vq(t0):
                def fn():
                    for t in range(t0, t0 + 4):
                        v_chunk(t)
                return fn

            def qk1s(s):
                return lambda: qk_slab(1, s, width=1, q_act=False)

            def qs2(jn, h):
                return lambda: qk_slab(jn, 2 * h, width=2)

            def stg(jn, g):
                return lambda: stats_group(jn, g)

            def qk0s(s):
                return lambda: qk_slab(0, s, width=1, q_act=True)

            def stgf0(s):
                def fn():
                    stats_group(0, s)
                    stats_fin(0, s=s)
                return fn

            def sfin(jn):
                return lambda: stats_fin(jn)

            def p3p(s, db0):
                return lambda: p3_pair(s, db0)

            il_map = {
                0: [qk1s(0), qk0s(2), stgf0(2), vp(0, 1), vp(2, 3),
                    qk1s(1), stg(1, 0), vq(4), qk0s(3), stgf0(3),
                    stg(1, 1), qk1s(2), qk1s(3), vp(8, 9), vp(10, 11),
                    stg(1, 2), stg(1, 3), sfin(1), vp(12, 13), vp(14, 15)],
                1: [None, None, qs2(2, 0), qs2(2, 1), None, None, stg(2, 0),
                    None, stg(2, 1), None, stg(2, 2), None, stg(2, 3),
                    sfin(2), None, None, None, None, None, None],
                2: [None, None, qs2(3, 0), qs2(3, 1), None, None, stg(3, 0),
                    None, stg(3, 1), None, stg(3, 2), None, stg(3, 3),
                    sfin(3), None, None, None, None, None, None],
                3: [None, None, None, None, None, None,
                    p3p(0, 0), p3p(0, 2), p3p(0, 4), p3p(0, 6),
                    lambda: p3_out(0),
                    p3p(1, 0), p3p(1, 2), p3p(1, 4), p3p(1, 6),
                    lambda: p3_out(1),
                    p3p(2, 0), p3p(2, 2), p3p(2, 4), p3p(2, 6)],
            }

            for j in range(HPC):
                il_list = il_map[j]
                for i, u in enumerate(head_units(j)):
                    step(u, il_list[i])
            # ---- tail: retire the last units with a split normalize so
            # phase 3's last superblock overlaps the final softmax columns.
            outv = outT.rearrange("(g p) t -> p g t", p=128)
            for g0 in (0, 4):
                nc.sync.dma_start(
                    outv[:, g0 : g0 + 4, 1024:1536],
                    o_sb[2][:, g0 : g0 + 4, :],
                )
            while len(pending) > 2:
                retire()
            pending[1][0]["close"] = False
            retire()  # z for kbs 12,13 -> cols [0:256] of (3,3) complete
            zps33 = zps_t[(3, 3)]
            r1a = sbp.tile([1, 512], F32, tag="r1", bufs=2, name="r1a")
            nc.vector.reciprocal(r1a[0:1, 0:256], zps33[64:65, 0:256])
            rba = sbp.tile([64, 512], F32, tag="rb", bufs=2, name="rba")
            nc.gpsimd.partition_broadcast(rba[0:64, 0:256], r1a[0:1, 0:256])
            nc.vector.tensor_mul(
                zT_sb[1][64:128, 1536:1792], zps33[0:64, 0:256], rba[0:64, 0:256]
            )
            retire()  # z for kbs 14,15 -> cols [256:512] complete
            r1b = sbp.tile([1, 512], F32, tag="r1", bufs=2, name="r1b")
            nc.vector.reciprocal(r1b[0:1, 0:256], zps33[64:65, 256:512])
            rbb = sbp.tile([64, 512], F32, tag="rb", bufs=2, name="rbb")
            nc.gpsimd.partition_broadcast(rbb[0:64, 0:256], r1b[0:1, 0:256])
            nc.vector.tensor_mul(
                zT_sb[1][64:128, 1792:2048], zps33[0:64, 256:512], rbb[0:64, 0:256]
            )
            del zps_t[(3, 3)]

            o_sb[3] = sbp.tile(
                [128, NDC, 512], BF16, tag="osb", bufs=2, name="osb3"
            )

            def p3_pair_h(db0, h):
                ops = psB.tile([128, 1024], F32, tag="mm")
                for i in range(2):
                    for p in range(2):
                        nc.tensor.matmul(
                            ops[:, i * 512 + h * 256 : i * 512 + (h + 1) * 256],
                            lhsT=(wo_sb[:, p, (db0 + i) * 128 : (db0 + i + 1) * 128]),
                            rhs=(zT_sb[p][:, 1536 + h * 256 : 1792 + h * 256]),
                            start=(p == 0),
                            stop=(p == 1),
                        )
                dst = o_sb[3][:, db0 : db0 + 2, h * 256 : (h + 1) * 256]
                srcv = ops[:, 0:1024].rearrange("p (i w) -> p i w", w=512)[
                    :, :, h * 256 : (h + 1) * 256
                ]
                if db0 % 4 == 0:
                    nc.scalar.activation(dst, srcv, AF.Copy)
                else:
                    nc.vector.tensor_copy(dst, srcv)

            for h in range(2):
                for db0 in (0, 2, 4, 6):
                    p3_pair_h(db0, h)
                for g0 in (0, 4):
                    nc.sync.dma_start(
                        outv[:, g0 : g0 + 4,
                             1536 + h * 256 : 1792 + h * 256],
                        o_sb[3][:, g0 : g0 + 4, h * 256 : (h + 1) * 256],
                    )
    nc.compile()
    return nc


_NC = None


def _get_nc():
    global _NC
    if _NC is None:
        _NC = build_nc()
    return _NC


def _make_in_maps(inputs):
    x = np.ascontiguousarray(np.asarray(inputs["residual_stream"], dtype=np.float32))
    W_Q = np.asarray(inputs["W_Q"], dtype=np.float32)
    W_K = np.asarray(inputs["W_K"], dtype=np.float32)
    W_V = np.asarray(inputs["W_V"], dtype=np.float32)
    W_O = np.asarray(inputs["W_output"], dtype=np.float32)

    try:
        import ml_dtypes
        bf16 = ml_dtypes.bfloat16
    except ImportError:
        bf16 = np.float32

    qi = np.arange(128)
    mskS = np.where(qi[None, :] <= qi[:, None], 0.0, -1e9).astype(np.float32)  # [q,k]
    mskT = np.ascontiguousarray(mskS.T)  # [k,q]
    si = np.arange(128)
    mskG = np.zeros((128, NQB, 128), np.float32)
    for qb in range(NQB):
        mskG[:, qb, :] = np.where(
            si[None, :] * (qb + 1) <= 128 * qb + qi[:, None], 0.0, -1e9
        )
    mskG = np.ascontiguousarray(mskG.reshape(128, NQB * 128))

    in_maps = []
    for c in range(8):
        b, hg = c // 4, c % 4
        hs = [4 * hg + j for j in range(HPC)]
        wqk_h = np.concatenate(
            [np.concatenate([W_Q[h] / 8.0, W_K[h]], axis=1) for h in hs], axis=1
        )  # [1024, 512]
        wv_h = np.concatenate([W_V[h] for h in hs], axis=1)  # [1024, 256]
        wo_h = np.stack(
            [np.concatenate([W_O[hs[2 * p]], W_O[hs[2 * p + 1]]], axis=0)
             for p in range(2)]
        )  # [2, 128, 1024]
        in_maps.append(
            {
                "xT": np.ascontiguousarray(x[b].T).astype(np.float16),
                "wqk": np.ascontiguousarray(wqk_h).astype(np.float16),
                "wv": np.ascontiguousarray(wv_h).astype(np.float16),
                "wo": np.ascontiguousarray(wo_h).astype(bf16),
                "ones": np.ones((1, T), np.float16),
            }
        )
    return in_maps


def _postprocess(results, inputs):
    W_O = np.asarray(inputs["W_output"], dtype=np.float32)
    b_V = np.asarray(inputs["b_V"], dtype=np.float32)
    b_out = np.asarray(inputs["b_output"], dtype=np.float32)
    out = np.zeros((2, T, D), dtype=np.float32)
    for c in range(8):
        out[c // 4] += results[c]["outT"].T.astype(np.float32)
    # z = P @ v + b_V (P rows sum to 1) -> fold b_V through W_O on the host
    const = np.einsum("he,hed->d", b_V, W_O) + b_out
    out += const[None, None, :]
    return out


def kernel(**inputs):
    from concourse.bass_utils import run_bass_kernel_spmd

    nc = _get_nc()
    res = run_bass_kernel_spmd(nc, _make_in_maps(inputs), core_ids=list(range(8)))
    return _postprocess(res.results, inputs)


def kernel_traced(**inputs):
    """Returns (output, exec_time_ns or None) using a traced run."""
    from concourse.bass_utils import run_bass_kernel_spmd

    nc = _get_nc()
    res = run_bass_kernel_spmd(
        nc, _make_in_maps(inputs), core_ids=list(range(8)), trace=True
    )
    return _postprocess(res.results, inputs), res.exec_time_ns


# revision 5
# speedup vs baseline: 1.0460x; 1.0013x over previous
"""Multi-head causal attention on 8 TRN2 NeuronCores — v5.

Sharding: data-parallel over batch (2) x tensor-parallel over heads (4 groups
of 4 heads) = 8 cores. Host sums the 4 partial output projections per batch.

Engine plan (GPSIMD cannot touch PSUM on real HW):
  PE   S^T pairs, z accumulation, projections, stats samples, transposes
  ACT  exp (paired S blocks), row-max formatting (+its DMA queue)
  DVE  PSUM drains (q/k/v), diag masks, fused stats mask+min-reduce,
       denominator copy, z normalize divide
  Pool denominator partition-broadcast (SBUF only)
  SP   batched input DMAs, phase-3 PSUM->DRAM output (f32), row-max rows

One global software pipeline over every S-pair of every head: each step
emits [S-pair matmuls, diag masks, exp], one interleave closure (v chunk /
next head's 1024-wide projection slab / stats pack / phase-3 chunk), then
retires the z matmuls of the pair three steps back.
"""

import os
import sys

import numpy as np

for _p in ("/opt/trn_rl_repo", "/root/.axon_site/_ro/trn_rl_repo"):
    if os.path.isdir(_p) and _p not in sys.path:
        sys.path.insert(0, _p)

import concourse.bass as bass
from concourse import bacc
import concourse.tile as tile
from concourse import mybir
from concourse.masks import make_identity

F32 = mybir.dt.float32
F16 = mybir.dt.float16
BF16 = mybir.dt.bfloat16
AX = mybir.AxisListType
OP = mybir.AluOpType
AF = mybir.ActivationFunctionType

T = 2048
D = 1024
HPC = 4          # heads per core
DH = 64
NQB = T // 128   # 16
NSB = T // 512   # 4
NDC = D // 128   # 8
MARGIN = 40.0


def build_nc():
    nc = bacc.Bacc("TRN2", target_bir_lowering=False)
    xT = nc.dram_tensor("xT", [D, T], F16, kind="ExternalInput")
    wqk = nc.dram_tensor("wqk", [D, 128 * HPC], F16, kind="ExternalInput")
    wv = nc.dram_tensor("wv", [D, DH * HPC], F16, kind="ExternalInput")
    wo = nc.dram_tensor("wo", [2, 128, D], BF16, kind="ExternalInput")
    ones = nc.dram_tensor("ones", [1, T], F16, kind="ExternalInput")
    outT = nc.dram_tensor("outT", [D, T], BF16, kind="ExternalOutput")

    with tile.TileContext(nc) as tc:
        with (
            tc.tile_pool(name="const", bufs=1) as constp,
            tc.tile_pool(name="big", bufs=1) as bigp,
            tc.tile_pool(name="sb", bufs=3) as sbp,
            tc.tile_pool(name="psB", bufs=3, space="PSUM") as psB,
            tc.tile_pool(name="psZ", bufs=2, space="PSUM") as psZ,
        ):
            # ---- persistent SBUF ----
            xT_sb = bigp.tile([128, NDC, T], F16, tag="xT")
            wqk_sb = bigp.tile([128, NDC, 128 * HPC], F16, tag="wqk")
            wv_sb = bigp.tile([128, NDC, DH * HPC], F16, tag="wv")
            wo_sb = bigp.tile([128, 2, D], BF16, tag="wo")
            mskG_sb = constp.tile([128, NQB, 128], F32, tag="mskG")
            mskT_sb = constp.tile([128, 128], F32, tag="mskT")
            mskT2_sb = constp.tile([128, 2, 128], F32, tag="mskT2")
            ident = constp.tile([128, 128], F32, tag="ident")
            margin = constp.tile([NQB, 1], F32, tag="margin")
            q_sb = [bigp.tile([65, T], F16, tag=f"q{j}", name=f"q{j}") for j in range(HPC)]
            k_sb = [bigp.tile([65, T], F16, tag=f"k{j}", name=f"k{j}") for j in range(HPC)]
            v_sb = bigp.tile([128, NQB, HPC, DH + 1], BF16, tag="v")
            zT_sb = [bigp.tile([128, T], BF16, tag=f"zp{p}", name=f"zp{p}") for p in range(2)]

            xTr = xT.rearrange("(c p) t -> p c t", p=128)
            wqkr = wqk.rearrange("(c p) m -> p c m", p=128)
            nc.sync.dma_start(wqk_sb[:, :, 0:128], wqkr[:, :, 0:128])
            nc.sync.dma_start(xT_sb[:, 0:4, 0:512], xTr[:, 0:4, 0:512])
            nc.sync.dma_start(xT_sb[:, 4:8, 0:512], xTr[:, 4:8, 0:512])
            nc.sync.dma_start(wv_sb[:], wv.rearrange("(c p) m -> p c m", p=128))
            nc.sync.dma_start(k_sb[0][64:65, :], ones[:])
            nc.sync.dma_start(xT_sb[:, :, 512:1024], xTr[:, :, 512:1024])
            nc.sync.dma_start(wqk_sb[:, :, 128:512], wqkr[:, :, 128:512])
            for s in (2, 3):
                nc.sync.dma_start(
                    xT_sb[:, :, s * 512 : (s + 1) * 512],
                    xTr[:, :, s * 512 : (s + 1) * 512],
                )
            for j in range(1, HPC):
                nc.sync.dma_start(k_sb[j][64:65, :], ones[:])
            nc.sync.dma_start(wo_sb[:], wo.rearrange("p k d -> k p d"))
            make_identity(nc, ident[:])
            nc.vector.memset(margin[:], -MARGIN)
            nc.gpsimd.memset(v_sb[:, :, :, DH : DH + 1], 1.0)
            # on-device masks (Pool is otherwise idle):
            # mskT[k, q] = 0 if k <= q else -1e9
            nc.gpsimd.memset(mskT_sb[:], 0.0)
            nc.gpsimd.affine_select(
                out=mskT_sb[:], in_=mskT_sb[:], compare_op=OP.is_ge,
                fill=-1e9, base=0, pattern=[[1, 128]], channel_multiplier=-1,
            )
            nc.gpsimd.tensor_copy(mskT2_sb[:, 0, :], mskT_sb[:])
            nc.gpsimd.tensor_copy(mskT2_sb[:, 1, :], mskT_sb[:])
            # mskG block qb: 0 if i*(qb+1) <= 128*qb + p else -1e9
            nc.gpsimd.memset(mskG_sb[:], 0.0)
            for qb in range(NQB):
                nc.gpsimd.affine_select(
                    out=mskG_sb[:, qb, :], in_=mskG_sb[:, qb, :],
                    compare_op=OP.is_ge, fill=-1e9, base=128 * qb,
                    pattern=[[-(qb + 1), 128]], channel_multiplier=1,
                )

            # ---- phase 1a: qT/kT slabs. width = number of 512-superblocks.
            # Copies drain in 512-col chunks so critical DVE ops (diag masks,
            # normalize) never queue behind a >1us copy; q goes via ACT only
            # in the front where ACT is idle.
            def qk_slab(j, s0, width=1, q_act=True):
                ps = psB.tile([128, 1024], F32, tag="mm")
                for h in range(width):
                    for c in range(NDC):
                        nc.tensor.matmul(
                            ps[:, h * 512 : (h + 1) * 512],
                            lhsT=(wqk_sb[:, c, j * 128 : (j + 1) * 128]),
                            rhs=(xT_sb[:, c, (s0 + h) * 512 : (s0 + h + 1) * 512]),
                            start=(c == 0),
                            stop=(c == NDC - 1),
                        )
                for h in range(width):
                    lo, hi = (s0 + h) * 512, (s0 + h + 1) * 512
                    if q_act:
                        nc.scalar.activation(
                            q_sb[j][0:64, lo:hi], ps[0:64, h * 512 : (h + 1) * 512],
                            AF.Copy,
                        )
                    else:
                        nc.vector.tensor_copy(
                            q_sb[j][0:64, lo:hi], ps[0:64, h * 512 : (h + 1) * 512]
                        )
                    if q_act and width == 2:
                        nc.scalar.activation(
                            k_sb[j][0:64, lo:hi], ps[64:128, h * 512 : (h + 1) * 512],
                            AF.Copy,
                        )
                    else:
                        nc.vector.tensor_copy(
                            k_sb[j][0:64, lo:hi], ps[64:128, h * 512 : (h + 1) * 512]
                        )

            # ---- phase 1b: v natural [t, 4*65] (ones col 64 for denominators)
            def v_chunk(tb):
                ps = psB.tile([128, 512], F32, tag="mm")
                for c in range(NDC):
                    nc.tensor.matmul(
                        ps[:, 0 : DH * HPC],
                        lhsT=(xT_sb[:, c, tb * 128 : (tb + 1) * 128]),
                        rhs=(wv_sb[:, c, :]),
                        start=(c == 0),
                        stop=(c == NDC - 1),
                    )
                nc.vector.tensor_copy(
                    v_sb[:, tb, :, 0:DH],
                    ps[:, 0 : DH * HPC].rearrange("p (j e) -> p j e", j=HPC),
                )

            # ---- phase 2A: stats — negated causal row max. Each 128-row
            # q-block qb samples its causal range at stride (qb+1), giving a
            # uniform 128 samples; 4 blocks share one PSUM tile, one boundary
            # mask add and one 3-D max-reduce. Margin 40 covers sampling.
            m_all = {}

            def stats_group(j, g):
                if g == 0:
                    m_all[j] = sbp.tile(
                        [128, NQB], F32, tag=f"mall{j % 2}", name=f"mall{j}"
                    )
                ma = m_all[j]
                ps = psB.tile([128, 512], F32, tag="mm")
                for i in range(4):
                    qb = 4 * g + i
                    L = 128 * (qb + 1)
                    if qb == 0:
                        rhs = k_sb[j][0:64, 0:128]
                    else:
                        kr = k_sb[j][0:64, 0:L].rearrange(
                            "p (n st) -> p n st", st=qb + 1
                        )
                        rhs = kr[:, :, 0:1]
                    nc.tensor.matmul(
                        ps[:, i * 128 : (i + 1) * 128],
                        lhsT=(q_sb[j][0:64, qb * 128 : (qb + 1) * 128]),
                        rhs=rhs,
                        start=True,
                        stop=True,
                    )
                pv = ps[:, 0:512].rearrange("p (n w) -> p n w", w=128)
                gv = mskG_sb[:, 4 * g : 4 * g + 4, :]
                if g == 0:
                    nc.vector.tensor_tensor(
                        ps[:, 0:512], ps[:, 0:512],
                        mskG_sb[:, 0:4, :].rearrange("p n w -> p (n w)"),
                        op=OP.add,
                    )
                else:
                    nc.vector.tensor_tensor(
                        pv[:, :, 96:128], pv[:, :, 96:128], gv[:, :, 96:128],
                        op=OP.add,
                    )
                nc.vector.tensor_reduce(
                    ma[:, 4 * g : 4 * g + 4], pv[:], axis=AX.X,
                    op=OP.max, negate=True,
                )

            def stats_fin(j, s=None):
                # transpose -m into row 64 of q' (as -max - MARGIN, fp16).
                # s=None: whole row via the idle SP queue (heads 1-3);
                # else one 512-col chunk via the ACT queue (head 0 warmup).
                lo, n = (0, NQB) if s is None else (4 * s, 4)
                pm = psB.tile([128, 512], F32, tag="mm")
                nc.tensor.transpose(
                    pm[0:n, 0:128], m_all[j][:, lo : lo + n], ident[:]
                )
                mT = sbp.tile([NQB, 128], F16, tag="mT", bufs=2)
                nc.scalar.activation(
                    mT[0:n, :], pm[0:n, 0:128], AF.Identity, bias=margin[0:n, :]
                )
                dst = q_sb[j][64:65, lo * 128 : (lo + n) * 128]
                if s is None:
                    nc.sync.dma_start(dst, mT[0:n, :])
                else:
                    nc.scalar.dma_start(dst, mT[0:n, :])

            # ---- phase 3: one [128d, 512t] out^T chunk -> o_sb; DMA per s
            o_sb = {}

            p3_alt = [0]

            def p3_pair(s, db0):
                if db0 == 0:
                    o_sb[s] = sbp.tile(
                        [128, NDC, 512], BF16, tag="osb", bufs=2, name=f"osb{s}"
                    )
                ops = psB.tile([128, 1024], F32, tag="mm")
                for i in range(2):
                    for p in range(2):
                        nc.tensor.matmul(
                            ops[:, i * 512 : (i + 1) * 512],
                            lhsT=(wo_sb[:, p, (db0 + i) * 128 : (db0 + i + 1) * 128]),
                            rhs=(zT_sb[p][:, s * 512 : (s + 1) * 512]),
                            start=(p == 0),
                            stop=(p == 1),
                        )
                dst = o_sb[s][:, db0 : db0 + 2, :].rearrange("p n w -> p (n w)")
                p3_alt[0] ^= 1
                if s == 2 or p3_alt[0]:
                    nc.scalar.activation(dst, ops[:], AF.Copy)
                else:
                    nc.vector.tensor_copy(dst, ops[:])

            def p3_out(s):
                nc.sync.dma_start(
                    outT.rearrange("(g p) t -> p g t", p=128)[
                        :, :, s * 512 : (s + 1) * 512
                    ],
                    o_sb[s][:],
                )

            # ---- phase 2B: the global S-pair pipeline ----------------------
            zps_t = {}
            pending = []

            def retire():
                u, pT = pending.pop(0)
                j, s = u["j"], u["s"]
                nkb = 4 * s + 4
                if (j, s) not in zps_t:
                    zps_t[(j, s)] = psZ.tile(
                        [65, 512], F32, tag="zz", name=f"zps{j}_{s}"
                    )
                zps = zps_t[(j, s)]
                for kb, w, off in u["z"]:
                    nc.tensor.matmul(
                        zps[:, 512 - w : 512],
                        lhsT=v_sb[:, kb, j, :],
                        rhs=pT[:, off : off + w],
                        start=(kb == 0),
                        stop=(kb == nkb - 1),
                        skip_group_check=True,
                    )
                if u["close"]:
                    r1 = sbp.tile([1, 512], F32, tag="r1", bufs=2)
                    nc.vector.reciprocal(r1[:], zps[64:65, :])
                    rb = sbp.tile([64, 512], F32, tag="rb", bufs=2)
                    nc.gpsimd.partition_broadcast(rb[:], r1[:])
                    p, po = j // 2, 64 * (j % 2)
                    nc.vector.tensor_mul(
                        zT_sb[p][po : po + 64, s * 512 : (s + 1) * 512],
                        zps[0:64, :],
                        rb[:],
                    )
                    del zps_t[(j, s)]

            def step(u, il):
                j, s, a, b = u["j"], u["s"], u["a"], u["b"]
                wa = 512 if a < 4 * s else 512 - 128 * (a - 4 * s)
                wb = 512 if b < 4 * s else 512 - 128 * (b - 4 * s)
                sps = psB.tile([128, 1024], F32, tag="mm")
                nc.tensor.matmul(
                    sps[:, 0:wa],
                    lhsT=(k_sb[j][0:65, a * 128 : (a + 1) * 128]),
                    rhs=(q_sb[j][0:65, s * 512 + 512 - wa : (s + 1) * 512]),
                    start=True,
                    stop=True,
                )
                nc.tensor.matmul(
                    sps[:, wa : wa + wb],
                    lhsT=(k_sb[j][0:65, b * 128 : (b + 1) * 128]),
                    rhs=(q_sb[j][0:65, s * 512 + 512 - wb : (s + 1) * 512]),
                    start=True,
                    stop=True,
                )
                if a >= 4 * s:
                    # both kbs of a pair are diagonal together: one strided
                    # add covers both [*, 0:128] and [*, wa:wa+128]
                    dv = sps[:, 0 : 2 * wa].rearrange(
                        "p (n w) -> p n w", w=wa
                    )[:, :, 0:128]
                    nc.vector.tensor_tensor(dv, dv, mskT2_sb[:], op=OP.add)
                pT = sbp.tile([128, 1024], BF16, tag="pT", bufs=6)
                nc.scalar.activation(pT[:, 0 : wa + wb], sps[:, 0 : wa + wb], AF.Exp)
                if il is not None:
                    il()
                u["z"] = [(a, wa, 0), (b, wb, wa)]
                pending.append((u, pT))
                if len(pending) > 4:
                    retire()

            def head_units(j):
                us = []
                for s in range(NSB):
                    nkb = 4 * s + 4
                    for a in range(0, nkb, 2):
                        us.append(
                            {"j": j, "s": s, "a": a, "b": a + 1,
                             "close": a + 2 == nkb}
                        )
                return us

            # ---- emission schedule ----
            # front: head-0 projections (512-wide for latency) with head-0
            # stats + per-superblock row-max finalize woven in
            for s in (0, 1):
                qk_slab(0, s, width=1, q_act=True)
                stats_group(0, s)
                stats_fin(0, s=s)

            def vp(t0, t1):
                def fn():
                    v_chunk(t0)
                    v_chunk(t1)
                return fn

            def vq(t0):
                def fn():
                    for t in range(t0, t0 + 4):
                        v_chunk(t)
                return fn

            def qk1s(s):
                return lambda: qk_slab(1, s, width=1, q_act=False)

            def qs2(jn, h):
                return lambda: qk_slab(jn, 2 * h, width=2)

            def stg(jn, g):
                return lambda: stats_group(jn, g)

            def qk0s(s):
                return lambda: qk_slab(0, s, width=1, q_act=True)

            def stgf0(s):
                def fn():
                    stats_group(0, s)
                    stats_fin(0, s=s)
                return fn

            def sfin(jn):
                return lambda: stats_fin(jn)

            def p3p(s, db0):
                return lambda: p3_pair(s, db0)

            il_map = {
                0: [qk1s(0), qk0s(2), stgf0(2), vp(0, 1), vp(2, 3),
                    qk1s(1), stg(1, 0), vq(4), qk0s(3), stgf0(3),
                    stg(1, 1), qk1s(2), qk1s(3), vp(8, 9), vp(10, 11),
                    stg(1, 2), stg(1, 3), sfin(1), vp(12, 13), vp(14, 15)],
                1: [None, None, qs2(2, 0), qs2(2, 1), None, None, stg(2, 0),
                    None, stg(2, 1), None, stg(2, 2), None, stg(2, 3),
                    sfin(2), None, None, None, None, None, None],
                2: [None, None, qs2(3, 0), qs2(3, 1), None, None, stg(3, 0),
                    None, stg(3, 1), None, stg(3, 2), None, stg(3, 3),
                    sfin(3), None, None, None, None, None, None],
                3: [None, None, None, None, None, None,
                    p3p(0, 0), p3p(0, 2), p3p(0, 4), p3p(0, 6),
                    lambda: p3_out(0),
                    p3p(1, 0), p3p(1, 2), p3p(1, 4), p3p(1, 6),
                    lambda: p3_out(1),
                    p3p(2, 0), p3p(2, 2), p3p(2, 4), p3p(2, 6)],
            }

            for j in range(HPC):
                il_list = il_map[j]
                for i, u in enumerate(head_units(j)):
                    step(u, il_list[i])
            # ---- tail: retire the last units with a split normalize so
            # phase 3's last superblock overlaps the final softmax columns.
            outv = outT.rearrange("(g p) t -> p g t", p=128)
            for g0 in (0, 4):
                nc.sync.dma_start(
                    outv[:, g0 : g0 + 4, 1024:1536],
                    o_sb[2][:, g0 : g0 + 4, :],
                )
            while len(pending) > 2:
                retire()
            pending[1][0]["close"] = False
            retire()  # z for kbs 12,13 -> cols [0:256] of (3,3) complete
            zps33 = zps_t[(3, 3)]
            r1a = sbp.tile([1, 512], F32, tag="r1", bufs=2, name="r1a")
            nc.vector.reciprocal(r1a[0:1, 0:256], zps33[64:65, 0:256])
            rba = sbp.tile([64, 512], F32, tag="rb", bufs=2, name="rba")
            nc.gpsimd.partition_broadcast(rba[0:64, 0:256], r1a[0:1, 0:256])
            nc.vector.tensor_mul(
                zT_sb[1][64:128, 1536:1792], zps33[0:64, 0:256], rba[0:64, 0:256]
            )
            retire()  # z for kbs 14,15 -> cols [256:512] complete
            r1b = sbp.tile([1, 512], F32, tag="r1", bufs=2, name="r1b")
            nc.vector.reciprocal(r1b[0:1, 0:256], zps33[64:65, 256:512])
            rbb = sbp.tile([64, 512], F32, tag="rb", bufs=2, name="rbb")
            nc.gpsimd.partition_broadcast(rbb[0:64, 0:256], r1b[0:1, 0:256])
            nc.vector.tensor_mul(
                zT_sb[1][64:128, 1792:2048], zps33[0:64, 256:512], rbb[0:64, 0:256]
            )
            del zps_t[(3, 3)]

            o_sb[3] = sbp.tile(
                [128, NDC, 512], BF16, tag="osb", bufs=2, name="osb3"
            )

            def p3_pair_h(db0, h):
                ops = psB.tile([128, 1024], F32, tag="mm")
                for i in range(2):
                    for p in range(2):
                        nc.tensor.matmul(
                            ops[:, i * 512 + h * 256 : i * 512 + (h + 1) * 256],
                            lhsT=(wo_sb[:, p, (db0 + i) * 128 : (db0 + i + 1) * 128]),
                            rhs=(zT_sb[p][:, 1536 + h * 256 : 1792 + h * 256]),
                            start=(p == 0),
                            stop=(p == 1),
                        )
                dst = o_sb[3][:, db0 : db0 + 2, h * 256 : (h + 1) * 256]
                srcv = ops[:, 0:1024].rearrange("p (i w) -> p i w", w=512)[
                    :, :, h * 256 : (h + 1) * 256
                ]
                if db0 % 4 == 0:
                    nc.scalar.activation(dst, srcv, AF.Copy)
                else:
                    nc.vector.tensor_copy(dst, srcv)

            for h in range(2):
                for db0 in (0, 2, 4, 6):
                    p3_pair_h(db0, h)
                for g0 in (0, 4):
                    nc.sync.dma_start(
                        outv[:, g0 : g0 + 4,
                             1536 + h * 256 : 1792 + h * 256],
                        o_sb[3][:, g0 : g0 + 4, h * 256 : (h + 1) * 256],
                    )
    nc.compile()
    return nc


_NC = None


def _get_nc():
    global _NC
    if _NC is None:
        _NC = build_nc()
    return _NC


def _make_in_maps(inputs):
    x = np.ascontiguousarray(np.asarray(inputs["residual_stream"], dtype=np.float32))
    W_Q = np.asarray(inputs["W_Q"], dtype=np.float32)
    W_K = np.asarray(inputs["W_K"], dtype=np.float32)
    W_V = np.asarray(inputs["W_V"], dtype=np.float32)
    W_O = np.asarray(inputs["W_output"], dtype=np.float32)

    try:
        import ml_dtypes
        bf16 = ml_dtypes.bfloat16
    except ImportError:
        bf16 = np.float32

    qi = np.arange(128)
    mskS = np.where(qi[None, :] <= qi[:, None], 0.0, -1e9).astype(np.float32)  # [q,k]
    mskT = np.ascontiguousarray(mskS.T)  # [k,q]
    si = np.arange(128)
    mskG = np.zeros((128, NQB, 128), np.float32)
    for qb in range(NQB):
        mskG[:, qb, :] = np.where(
            si[None, :] * (qb + 1) <= 128 * qb + qi[:, None], 0.0, -1e9
        )
    mskG = np.ascontiguousarray(mskG.reshape(128, NQB * 128))

    in_maps = []
    for c in range(8):
        b, hg = c // 4, c % 4
        hs = [4 * hg + j for j in range(HPC)]
        wqk_h = np.concatenate(
            [np.concatenate([W_Q[h] / 8.0, W_K[h]], axis=1) for h in hs], axis=1
        )  # [1024, 512]
        wv_h = np.concatenate([W_V[h] for h in hs], axis=1)  # [1024, 256]
        wo_h = np.stack(
            [np.concatenate([W_O[hs[2 * p]], W_O[hs[2 * p + 1]]], axis=0)
             for p in range(2)]
        )  # [2, 128, 1024]
        in_maps.append(
            {
                "xT": np.ascontiguousarray(x[b].T).astype(np.float16),
                "wqk": np.ascontiguousarray(wqk_h).astype(np.float16),
                "wv": np.ascontiguousarray(wv_h).astype(np.float16),
                "wo": np.ascontiguousarray(wo_h).astype(bf16),
                "ones": np.ones((1, T), np.float16),
            }
        )
    return in_maps


def _postprocess(results, inputs):
    W_O = np.asarray(inputs["W_output"], dtype=np.float32)
    b_V = np.asarray(inputs["b_V"], dtype=np.float32)
    b_out = np.asarray(inputs["b_output"], dtype=np.float32)
    out = np.zeros((2, T, D), dtype=np.float32)
    for c in range(8):
        out[c // 4] += results[c]["outT"].T.astype(np.float32)
    # z = P @ v + b_V (P rows sum to 1) -> fold b_V through W_O on the host
    const = np.einsum("he,hed->d", b_V, W_O) + b_out
    out += const[None, None, :]
    return out


def kernel(**inputs):
    from concourse.bass_utils import run_bass_kernel_spmd

    nc = _get_nc()
    res = run_bass_kernel_spmd(nc, _make_in_maps(inputs), core_ids=list(range(8)))
    return _postprocess(res.results, inputs)


def kernel_traced(**inputs):
    """Returns (output, exec_time_ns or None) using a traced run."""
    from concourse.bass_utils import run_bass_kernel_spmd

    nc = _get_nc()
    res = run_bass_kernel_spmd(
        nc, _make_in_maps(inputs), core_ids=list(range(8)), trace=True
    )
    return _postprocess(res.results, inputs), res.exec_time_ns


# revision 6
# speedup vs baseline: 1.0527x; 1.0064x over previous
"""Multi-head causal attention on 8 TRN2 NeuronCores — v5.

Sharding: data-parallel over batch (2) x tensor-parallel over heads (4 groups
of 4 heads) = 8 cores. Host sums the 4 partial output projections per batch.

Engine plan (GPSIMD cannot touch PSUM on real HW):
  PE   S^T pairs, z accumulation, projections, stats samples, transposes
  ACT  exp (paired S blocks), row-max formatting (+its DMA queue)
  DVE  PSUM drains (q/k/v), diag masks, fused stats mask+min-reduce,
       denominator copy, z normalize divide
  Pool denominator partition-broadcast (SBUF only)
  SP   batched input DMAs, phase-3 PSUM->DRAM output (f32), row-max rows

One global software pipeline over every S-pair of every head: each step
emits [S-pair matmuls, diag masks, exp], one interleave closure (v chunk /
next head's 1024-wide projection slab / stats pack / phase-3 chunk), then
retires the z matmuls of the pair three steps back.
"""

import os
import sys

import numpy as np

for _p in ("/opt/trn_rl_repo", "/root/.axon_site/_ro/trn_rl_repo"):
    if os.path.isdir(_p) and _p not in sys.path:
        sys.path.insert(0, _p)

import concourse.bass as bass
from concourse import bacc
import concourse.tile as tile
from concourse import mybir
from concourse.masks import make_identity

F32 = mybir.dt.float32
F16 = mybir.dt.float16
BF16 = mybir.dt.bfloat16
AX = mybir.AxisListType
OP = mybir.AluOpType
AF = mybir.ActivationFunctionType

T = 2048
D = 1024
HPC = 4          # heads per core
DH = 64
NQB = T // 128   # 16
NSB = T // 512   # 4
NDC = D // 128   # 8
MARGIN = 40.0


def build_nc():
    nc = bacc.Bacc("TRN2", target_bir_lowering=False)
    xT = nc.dram_tensor("xT", [D, T], F16, kind="ExternalInput")
    wqk = nc.dram_tensor("wqk", [D, 128 * HPC], F16, kind="ExternalInput")
    wv = nc.dram_tensor("wv", [D, DH * HPC], F16, kind="ExternalInput")
    wo = nc.dram_tensor("wo", [2, 128, D], BF16, kind="ExternalInput")
    ones = nc.dram_tensor("ones", [1, T], F16, kind="ExternalInput")
    outT = nc.dram_tensor("outT", [D, T], BF16, kind="ExternalOutput")

    with tile.TileContext(nc) as tc:
        with (
            tc.tile_pool(name="const", bufs=1) as constp,
            tc.tile_pool(name="big", bufs=1) as bigp,
            tc.tile_pool(name="sb", bufs=3) as sbp,
            tc.tile_pool(name="psB", bufs=3, space="PSUM") as psB,
            tc.tile_pool(name="psZ", bufs=2, space="PSUM") as psZ,
        ):
            # ---- persistent SBUF ----
            xT_sb = bigp.tile([128, NDC, T], F16, tag="xT")
            wqk_sb = bigp.tile([128, NDC, 128 * HPC], F16, tag="wqk")
            wv_sb = bigp.tile([128, NDC, DH * HPC], F16, tag="wv")
            wo_sb = bigp.tile([128, 2, D], BF16, tag="wo")
            mskG_sb = constp.tile([128, NQB, 128], F32, tag="mskG")
            mskT_sb = constp.tile([128, 128], F32, tag="mskT")
            mskT2_sb = constp.tile([128, 2, 128], F32, tag="mskT2")
            ident = constp.tile([128, 128], F32, tag="ident")
            margin = constp.tile([NQB, 1], F32, tag="margin")
            q_sb = [bigp.tile([65, T], F16, tag=f"q{j}", name=f"q{j}") for j in range(HPC)]
            k_sb = [bigp.tile([65, T], F16, tag=f"k{j}", name=f"k{j}") for j in range(HPC)]
            v_sb = bigp.tile([128, NQB, HPC, DH + 1], BF16, tag="v")
            zT_sb = [bigp.tile([128, T], BF16, tag=f"zp{p}", name=f"zp{p}") for p in range(2)]

            xTr = xT.rearrange("(c p) t -> p c t", p=128)
            wqkr = wqk.rearrange("(c p) m -> p c m", p=128)
            nc.sync.dma_start(wqk_sb[:, :, 0:128], wqkr[:, :, 0:128])
            nc.sync.dma_start(xT_sb[:, 0:4, 0:512], xTr[:, 0:4, 0:512])
            nc.sync.dma_start(xT_sb[:, 4:8, 0:512], xTr[:, 4:8, 0:512])
            nc.sync.dma_start(wv_sb[:], wv.rearrange("(c p) m -> p c m", p=128))
            nc.sync.dma_start(k_sb[0][64:65, :], ones[:])
            nc.sync.dma_start(xT_sb[:, :, 512:1024], xTr[:, :, 512:1024])
            nc.sync.dma_start(wqk_sb[:, :, 128:512], wqkr[:, :, 128:512])
            for s in (2, 3):
                nc.sync.dma_start(
                    xT_sb[:, :, s * 512 : (s + 1) * 512],
                    xTr[:, :, s * 512 : (s + 1) * 512],
                )
            for j in range(1, HPC):
                nc.sync.dma_start(k_sb[j][64:65, :], ones[:])
            nc.sync.dma_start(wo_sb[:], wo.rearrange("p k d -> k p d"))
            make_identity(nc, ident[:])
            nc.vector.memset(margin[:], -MARGIN)
            nc.gpsimd.memset(v_sb[:, :, :, DH : DH + 1], 1.0)
            # on-device masks (Pool is otherwise idle):
            # mskT[k, q] = 0 if k <= q else -1e9
            nc.gpsimd.memset(mskT_sb[:], 0.0)
            nc.gpsimd.affine_select(
                out=mskT_sb[:], in_=mskT_sb[:], compare_op=OP.is_ge,
                fill=-1e9, base=0, pattern=[[1, 128]], channel_multiplier=-1,
            )
            nc.gpsimd.tensor_copy(mskT2_sb[:, 0, :], mskT_sb[:])
            nc.gpsimd.tensor_copy(mskT2_sb[:, 1, :], mskT_sb[:])
            # mskG block qb: 0 if i*(qb+1) <= 128*qb + p else -1e9
            nc.gpsimd.memset(mskG_sb[:], 0.0)
            for qb in range(NQB):
                nc.gpsimd.affine_select(
                    out=mskG_sb[:, qb, :], in_=mskG_sb[:, qb, :],
                    compare_op=OP.is_ge, fill=-1e9, base=128 * qb,
                    pattern=[[-(qb + 1), 128]], channel_multiplier=1,
                )

            # ---- phase 1a: qT/kT slabs. width = number of 512-superblocks.
            # Copies drain in 512-col chunks so critical DVE ops (diag masks,
            # normalize) never queue behind a >1us copy; q goes via ACT only
            # in the front where ACT is idle.
            def qk_slab(j, s0, width=1, q_act=True):
                ps = psB.tile([128, 1024], F32, tag="mm")
                for h in range(width):
                    for c in range(NDC):
                        nc.tensor.matmul(
                            ps[:, h * 512 : (h + 1) * 512],
                            lhsT=(wqk_sb[:, c, j * 128 : (j + 1) * 128]),
                            rhs=(xT_sb[:, c, (s0 + h) * 512 : (s0 + h + 1) * 512]),
                            start=(c == 0),
                            stop=(c == NDC - 1),
                        )
                for h in range(width):
                    lo, hi = (s0 + h) * 512, (s0 + h + 1) * 512
                    if q_act:
                        nc.scalar.activation(
                            q_sb[j][0:64, lo:hi], ps[0:64, h * 512 : (h + 1) * 512],
                            AF.Copy,
                        )
                    else:
                        nc.vector.tensor_copy(
                            q_sb[j][0:64, lo:hi], ps[0:64, h * 512 : (h + 1) * 512]
                        )
                    if q_act and width == 2:
                        nc.scalar.activation(
                            k_sb[j][0:64, lo:hi], ps[64:128, h * 512 : (h + 1) * 512],
                            AF.Copy,
                        )
                    else:
                        nc.vector.tensor_copy(
                            k_sb[j][0:64, lo:hi], ps[64:128, h * 512 : (h + 1) * 512]
                        )

            # ---- phase 1b: v natural [t, 4*65] (ones col 64 for denominators)
            def v_chunk(tb):
                ps = psB.tile([128, 512], F32, tag="mm")
                for c in range(NDC):
                    nc.tensor.matmul(
                        ps[:, 0 : DH * HPC],
                        lhsT=(xT_sb[:, c, tb * 128 : (tb + 1) * 128]),
                        rhs=(wv_sb[:, c, :]),
                        start=(c == 0),
                        stop=(c == NDC - 1),
                    )
                nc.vector.tensor_copy(
                    v_sb[:, tb, :, 0:DH],
                    ps[:, 0 : DH * HPC].rearrange("p (j e) -> p j e", j=HPC),
                )

            # ---- phase 2A: stats — negated causal row max. Each 128-row
            # q-block qb samples its causal range at stride (qb+1), giving a
            # uniform 128 samples; 4 blocks share one PSUM tile, one boundary
            # mask add and one 3-D max-reduce. Margin 40 covers sampling.
            m_all = {}

            def stats_group(j, g):
                if g == 0:
                    m_all[j] = sbp.tile(
                        [128, NQB], F32, tag=f"mall{j % 2}", name=f"mall{j}"
                    )
                ma = m_all[j]
                ps = psB.tile([128, 512], F32, tag="mm")
                for i in range(4):
                    qb = 4 * g + i
                    L = 128 * (qb + 1)
                    if qb == 0:
                        rhs = k_sb[j][0:64, 0:128]
                    else:
                        kr = k_sb[j][0:64, 0:L].rearrange(
                            "p (n st) -> p n st", st=qb + 1
                        )
                        rhs = kr[:, :, 0:1]
                    nc.tensor.matmul(
                        ps[:, i * 128 : (i + 1) * 128],
                        lhsT=(q_sb[j][0:64, qb * 128 : (qb + 1) * 128]),
                        rhs=rhs,
                        start=True,
                        stop=True,
                    )
                pv = ps[:, 0:512].rearrange("p (n w) -> p n w", w=128)
                gv = mskG_sb[:, 4 * g : 4 * g + 4, :]
                if g == 0:
                    nc.vector.tensor_tensor(
                        ps[:, 0:512], ps[:, 0:512],
                        mskG_sb[:, 0:4, :].rearrange("p n w -> p (n w)"),
                        op=OP.add,
                    )
                else:
                    nc.vector.tensor_tensor(
                        pv[:, :, 96:128], pv[:, :, 96:128], gv[:, :, 96:128],
                        op=OP.add,
                    )
                nc.vector.tensor_reduce(
                    ma[:, 4 * g : 4 * g + 4], pv[:], axis=AX.X,
                    op=OP.max, negate=True,
                )

            def stats_fin(j, s=None):
                # transpose -m into row 64 of q' (as -max - MARGIN, fp16).
                # s=None: whole row via the idle SP queue (heads 1-3);
                # else one 512-col chunk via the ACT queue (head 0 warmup).
                lo, n = (0, NQB) if s is None else (4 * s, 4)
                pm = psB.tile([128, 512], F32, tag="mm")
                nc.tensor.transpose(
                    pm[0:n, 0:128], m_all[j][:, lo : lo + n], ident[:]
                )
                mT = sbp.tile([NQB, 128], F16, tag="mT", bufs=2)
                nc.scalar.activation(
                    mT[0:n, :], pm[0:n, 0:128], AF.Identity, bias=margin[0:n, :]
                )
                dst = q_sb[j][64:65, lo * 128 : (lo + n) * 128]
                if s is None:
                    nc.sync.dma_start(dst, mT[0:n, :])
                else:
                    nc.scalar.dma_start(dst, mT[0:n, :])

            # ---- phase 3: one [128d, 512t] out^T chunk -> o_sb; DMA per s
            o_sb = {}

            p3_alt = [0]

            def p3_pair(s, db0):
                if db0 == 0:
                    o_sb[s] = sbp.tile(
                        [128, NDC, 512], BF16, tag="osb", bufs=2, name=f"osb{s}"
                    )
                ops = psB.tile([128, 1024], F32, tag="mm")
                for i in range(2):
                    for p in range(2):
                        nc.tensor.matmul(
                            ops[:, i * 512 : (i + 1) * 512],
                            lhsT=(wo_sb[:, p, (db0 + i) * 128 : (db0 + i + 1) * 128]),
                            rhs=(zT_sb[p][:, s * 512 : (s + 1) * 512]),
                            start=(p == 0),
                            stop=(p == 1),
                        )
                dst = o_sb[s][:, db0 : db0 + 2, :].rearrange("p n w -> p (n w)")
                p3_alt[0] ^= 1
                if s == 2 or p3_alt[0]:
                    nc.scalar.activation(dst, ops[:], AF.Copy)
                else:
                    nc.vector.tensor_copy(dst, ops[:])

            def p3_out(s):
                nc.sync.dma_start(
                    outT.rearrange("(g p) t -> p g t", p=128)[
                        :, :, s * 512 : (s + 1) * 512
                    ],
                    o_sb[s][:],
                )

            # ---- phase 2B: the global S-pair pipeline ----------------------
            zps_t = {}
            pending = []
            norm_q = []

            def retire():
                u, pT = pending.pop(0)
                if norm_q:
                    norm_q.pop(0)()
                j, s = u["j"], u["s"]
                nkb = 4 * s + 4
                if (j, s) not in zps_t:
                    zps_t[(j, s)] = psZ.tile(
                        [65, 512], F32, tag="zz", name=f"zps{j}_{s}"
                    )
                zps = zps_t[(j, s)]
                for kb, w, off in u["z"]:
                    nc.tensor.matmul(
                        zps[:, 512 - w : 512],
                        lhsT=v_sb[:, kb, j, :],
                        rhs=pT[:, off : off + w],
                        start=(kb == 0),
                        stop=(kb == nkb - 1),
                        skip_group_check=True,
                    )
                if u["close"]:
                    def mknorm(j=j, s=s, zps=zps):
                        def donorm():
                            r1 = sbp.tile([1, 512], F32, tag="r1", bufs=2)
                            nc.vector.reciprocal(r1[:], zps[64:65, :])
                            rb = sbp.tile([64, 512], F32, tag="rb", bufs=2)
                            nc.gpsimd.partition_broadcast(rb[:], r1[:])
                            p, po = j // 2, 64 * (j % 2)
                            nc.vector.tensor_mul(
                                zT_sb[p][po : po + 64, s * 512 : (s + 1) * 512],
                                zps[0:64, :],
                                rb[:],
                            )
                        return donorm
                    norm_q.append(mknorm())
                    del zps_t[(j, s)]

            def step(u, il):
                j, s, a, b = u["j"], u["s"], u["a"], u["b"]
                wa = 512 if a < 4 * s else 512 - 128 * (a - 4 * s)
                wb = 512 if b < 4 * s else 512 - 128 * (b - 4 * s)
                sps = psB.tile([128, 1024], F32, tag="mm")
                nc.tensor.matmul(
                    sps[:, 0:wa],
                    lhsT=(k_sb[j][0:65, a * 128 : (a + 1) * 128]),
                    rhs=(q_sb[j][0:65, s * 512 + 512 - wa : (s + 1) * 512]),
                    start=True,
                    stop=True,
                )
                nc.tensor.matmul(
                    sps[:, wa : wa + wb],
                    lhsT=(k_sb[j][0:65, b * 128 : (b + 1) * 128]),
                    rhs=(q_sb[j][0:65, s * 512 + 512 - wb : (s + 1) * 512]),
                    start=True,
                    stop=True,
                )
                if a >= 4 * s:
                    # both kbs of a pair are diagonal together: one strided
                    # add covers both [*, 0:128] and [*, wa:wa+128]
                    dv = sps[:, 0 : 2 * wa].rearrange(
                        "p (n w) -> p n w", w=wa
                    )[:, :, 0:128]
                    nc.vector.tensor_tensor(dv, dv, mskT2_sb[:], op=OP.add)
                pT = sbp.tile([128, 1024], BF16, tag="pT", bufs=6)
                nc.scalar.activation(pT[:, 0 : wa + wb], sps[:, 0 : wa + wb], AF.Exp)
                if il is not None:
                    il()
                u["z"] = [(a, wa, 0), (b, wb, wa)]
                pending.append((u, pT))
                if len(pending) > 4:
                    retire()

            def head_units(j):
                us = []
                for s in range(NSB):
                    nkb = 4 * s + 4
                    for a in range(0, nkb, 2):
                        us.append(
                            {"j": j, "s": s, "a": a, "b": a + 1,
                             "close": a + 2 == nkb}
                        )
                return us

            # ---- emission schedule ----
            # front: head-0 projections (512-wide for latency) with head-0
            # stats + per-superblock row-max finalize woven in
            for s in (0, 1):
                qk_slab(0, s, width=1, q_act=True)
                stats_group(0, s)
                stats_fin(0, s=s)

            def vp(t0, t1):
                def fn():
                    v_chunk(t0)
                    v_chunk(t1)
                return fn

            def vq(t0):
                def fn():
                    for t in range(t0, t0 + 4):
                        v_chunk(t)
                return fn

            def qk1s(s):
                return lambda: qk_slab(1, s, width=1, q_act=False)

            def qs2(jn, h):
                return lambda: qk_slab(jn, 2 * h, width=2)

            def stg(jn, g):
                return lambda: stats_group(jn, g)

            def qk0s(s):
                return lambda: qk_slab(0, s, width=1, q_act=True)

            def stgf0(s):
                def fn():
                    stats_group(0, s)
                    stats_fin(0, s=s)
                return fn

            def sfin(jn):
                return lambda: stats_fin(jn)

            def p3p(s, db0):
                return lambda: p3_pair(s, db0)

            il_map = {
                0: [qk1s(0), qk0s(2), stgf0(2), vp(0, 1), vp(2, 3),
                    qk1s(1), stg(1, 0), vq(4), qk0s(3), stgf0(3),
                    stg(1, 1), qk1s(2), qk1s(3), vp(8, 9), vp(10, 11),
                    stg(1, 2), stg(1, 3), sfin(1), vp(12, 13), vp(14, 15)],
                1: [None, None, qs2(2, 0), qs2(2, 1), None, None, stg(2, 0),
                    None, stg(2, 1), None, stg(2, 2), None, stg(2, 3),
                    sfin(2), None, None, None, None, None, None],
                2: [None, None, qs2(3, 0), qs2(3, 1), None, None, stg(3, 0),
                    None, stg(3, 1), None, stg(3, 2), None, stg(3, 3),
                    sfin(3), None, None, None, None, None, None],
                3: [None, None, None, None, None, None, None,
                    p3p(0, 0), p3p(0, 2), p3p(0, 4), p3p(0, 6),
                    lambda: p3_out(0),
                    p3p(1, 0), p3p(1, 2), p3p(1, 4), p3p(1, 6),
                    lambda: p3_out(1),
                    p3p(2, 0), p3p(2, 2), p3p(2, 4)],
            }

            for j in range(HPC):
                il_list = il_map[j]
                for i, u in enumerate(head_units(j)):
                    step(u, il_list[i])
            # ---- tail: retire the last units with a split normalize so
            # phase 3's last superblock overlaps the final softmax columns.
            outv = outT.rearrange("(g p) t -> p g t", p=128)
            nc.sync.dma_start(
                outv[:, 0:4, 1024:1536], o_sb[2][:, 0:4, :]
            )
            while len(pending) > 2:
                retire()
            while norm_q:
                norm_q.pop(0)()
            p3_pair(2, 6)
            nc.sync.dma_start(
                outv[:, 4:8, 1024:1536], o_sb[2][:, 4:8, :]
            )
            pending[1][0]["close"] = False
            retire()  # z for kbs 12,13 -> cols [0:256] of (3,3) complete
            zps33 = zps_t[(3, 3)]
            r1a = sbp.tile([1, 512], F32, tag="r1", bufs=2, name="r1a")
            nc.vector.reciprocal(r1a[0:1, 0:256], zps33[64:65, 0:256])
            rba = sbp.tile([64, 512], F32, tag="rb", bufs=2, name="rba")
            nc.gpsimd.partition_broadcast(rba[0:64, 0:256], r1a[0:1, 0:256])
            nc.vector.tensor_mul(
                zT_sb[1][64:128, 1536:1792], zps33[0:64, 0:256], rba[0:64, 0:256]
            )
            retire()  # z for kbs 14,15 -> cols [256:512] complete
            r1b = sbp.tile([1, 512], F32, tag="r1", bufs=2, name="r1b")
            nc.vector.reciprocal(r1b[0:1, 0:256], zps33[64:65, 256:512])
            rbb = sbp.tile([64, 512], F32, tag="rb", bufs=2, name="rbb")
            nc.gpsimd.partition_broadcast(rbb[0:64, 0:256], r1b[0:1, 0:256])
            nc.vector.tensor_mul(
                zT_sb[1][64:128, 1792:2048], zps33[0:64, 256:512], rbb[0:64, 0:256]
            )
            del zps_t[(3, 3)]

            o_sb[3] = sbp.tile(
                [128, NDC, 512], BF16, tag="osb", bufs=2, name="osb3"
            )

            def p3_pair_h(db0, h):
                ops = psB.tile([128, 1024], F32, tag="mm")
                for i in range(2):
                    for p in range(2):
                        nc.tensor.matmul(
                            ops[:, i * 512 + h * 256 : i * 512 + (h + 1) * 256],
                            lhsT=(wo_sb[:, p, (db0 + i) * 128 : (db0 + i + 1) * 128]),
                            rhs=(zT_sb[p][:, 1536 + h * 256 : 1792 + h * 256]),
                            start=(p == 0),
                            stop=(p == 1),
                        )
                dst = o_sb[3][:, db0 : db0 + 2, h * 256 : (h + 1) * 256]
                srcv = ops[:, 0:1024].rearrange("p (i w) -> p i w", w=512)[
                    :, :, h * 256 : (h + 1) * 256
                ]
                if db0 % 4 == 0:
                    nc.scalar.activation(dst, srcv, AF.Copy)
                else:
                    nc.vector.tensor_copy(dst, srcv)

            for h in range(2):
                for db0 in (0, 2, 4, 6):
                    p3_pair_h(db0, h)
                for g0 in (0, 4):
                    nc.sync.dma_start(
                        outv[:, g0 : g0 + 4,
                             1536 + h * 256 : 1792 + h * 256],
                        o_sb[3][:, g0 : g0 + 4, h * 256 : (h + 1) * 256],
                    )
    nc.compile()
    return nc


_NC = None


def _get_nc():
    global _NC
    if _NC is None:
        _NC = build_nc()
    return _NC


def _make_in_maps(inputs):
    x = np.ascontiguousarray(np.asarray(inputs["residual_stream"], dtype=np.float32))
    W_Q = np.asarray(inputs["W_Q"], dtype=np.float32)
    W_K = np.asarray(inputs["W_K"], dtype=np.float32)
    W_V = np.asarray(inputs["W_V"], dtype=np.float32)
    W_O = np.asarray(inputs["W_output"], dtype=np.float32)

    try:
        import ml_dtypes
        bf16 = ml_dtypes.bfloat16
    except ImportError:
        bf16 = np.float32

    qi = np.arange(128)
    mskS = np.where(qi[None, :] <= qi[:, None], 0.0, -1e9).astype(np.float32)  # [q,k]
    mskT = np.ascontiguousarray(mskS.T)  # [k,q]
    si = np.arange(128)
    mskG = np.zeros((128, NQB, 128), np.float32)
    for qb in range(NQB):
        mskG[:, qb, :] = np.where(
            si[None, :] * (qb + 1) <= 128 * qb + qi[:, None], 0.0, -1e9
        )
    mskG = np.ascontiguousarray(mskG.reshape(128, NQB * 128))

    in_maps = []
    for c in range(8):
        b, hg = c // 4, c % 4
        hs = [4 * hg + j for j in range(HPC)]
        wqk_h = np.concatenate(
            [np.concatenate([W_Q[h] / 8.0, W_K[h]], axis=1) for h in hs], axis=1
        )  # [1024, 512]
        wv_h = np.concatenate([W_V[h] for h in hs], axis=1)  # [1024, 256]
        wo_h = np.stack(
            [np.concatenate([W_O[hs[2 * p]], W_O[hs[2 * p + 1]]], axis=0)
             for p in range(2)]
        )  # [2, 128, 1024]
        in_maps.append(
            {
                "xT": np.ascontiguousarray(x[b].T).astype(np.float16),
                "wqk": np.ascontiguousarray(wqk_h).astype(np.float16),
                "wv": np.ascontiguousarray(wv_h).astype(np.float16),
                "wo": np.ascontiguousarray(wo_h).astype(bf16),
                "ones": np.ones((1, T), np.float16),
            }
        )
    return in_maps


def _postprocess(results, inputs):
    W_O = np.asarray(inputs["W_output"], dtype=np.float32)
    b_V = np.asarray(inputs["b_V"], dtype=np.float32)
    b_out = np.asarray(inputs["b_output"], dtype=np.float32)
    out = np.zeros((2, T, D), dtype=np.float32)
    for c in range(8):
        out[c // 4] += results[c]["outT"].T.astype(np.float32)
    # z = P @ v + b_V (P rows sum to 1) -> fold b_V through W_O on the host
    const = np.einsum("he,hed->d", b_V, W_O) + b_out
    out += const[None, None, :]
    return out


def kernel(**inputs):
    from concourse.bass_utils import run_bass_kernel_spmd

    nc = _get_nc()
    res = run_bass_kernel_spmd(nc, _make_in_maps(inputs), core_ids=list(range(8)))
    return _postprocess(res.results, inputs)


def kernel_traced(**inputs):
    """Returns (output, exec_time_ns or None) using a traced run."""
    from concourse.bass_utils import run_bass_kernel_spmd

    nc = _get_nc()
    res = run_bass_kernel_spmd(
        nc, _make_in_maps(inputs), core_ids=list(range(8)), trace=True
    )
    return _postprocess(res.results, inputs), res.exec_time_ns


# revision 7
# speedup vs baseline: 1.0817x; 1.0276x over previous
"""Multi-head causal attention on 8 TRN2 NeuronCores.

Sharding: data-parallel over batch (2) x tensor-parallel over heads (4 groups
of 4 heads) = 8 cores. Each core emits a partial output projection in
transposed layout; the host sums the 4 partials per batch and folds the
(zero) biases.

Precision: x/W_QK/q/k fp16 (tf32-class mantissa; halves input DMA, full-rate
matmuls at any width), V/P/z/W_O/out bf16, all accumulation f32 in PSUM.
Softmax max-subtraction uses a sampled causal row max (each 128-row q-block
samples its range at stride (qb+1) = 128 samples) with a 40-unit safety
margin; the un-normalized exp weights stay finite in bf16/f32 by
construction (measured emulation rel err 4.5e-3 vs the noisy reference).

Engine plan (GPSIMD cannot touch PSUM on this HW):
  PE   S^T pairs, z accumulation, projections, stats samples, transposes
  ACT  exp over paired S^T blocks (2-bank PSUM tiles), q/k PSUM drains,
       row-max formatting + its DMA queue, half of phase-3 drains
  DVE  v drain, fused diagonal-pair causal masks, stats boundary masks +
       grouped 3-D max-reduces, denominator reciprocal, z normalize
  Pool on-device mask generation (affine_select), denominator broadcast
  SP   batched fp16/bf16 input DMAs, per-superblock output DMAs

Schedule: one global software pipeline over every S-pair unit of every
head. A step emits [2 S^T matmuls, fused diag mask, exp], then one
interleave closure (v chunk / next head's projection slab / next head's
stats group / phase-3 chunk), then retires the z matmuls of the unit four
steps back; normalize (reciprocal+broadcast+multiply) is deferred one
further step so it lands in the mask-free region of the next superblock.
The front consumes x slabs as they stream; the tail splits the last
superblock's normalize and phase 3 in halves to overlap the final DMAs.

CoreSim cost model: 153.8us vs 222.3us baseline (1.45x); rel err 4.5e-3.
"""

import os
import sys

import numpy as np

for _p in ("/opt/trn_rl_repo", "/root/.axon_site/_ro/trn_rl_repo"):
    if os.path.isdir(_p) and _p not in sys.path:
        sys.path.insert(0, _p)

import concourse.bass as bass
from concourse import bacc
import concourse.tile as tile
from concourse import mybir
from concourse.masks import make_identity

F32 = mybir.dt.float32
F16 = mybir.dt.float16
BF16 = mybir.dt.bfloat16
AX = mybir.AxisListType
OP = mybir.AluOpType
AF = mybir.ActivationFunctionType

T = 2048
D = 1024
HPC = 4          # heads per core
DH = 64
NQB = T // 128   # 16
NSB = T // 512   # 4
NDC = D // 128   # 8
MARGIN = 40.0


def build_nc():
    nc = bacc.Bacc("TRN2", target_bir_lowering=False)
    xT = nc.dram_tensor("xT", [D, T], F16, kind="ExternalInput")
    wqk = nc.dram_tensor("wqk", [D, 128 * HPC], F16, kind="ExternalInput")
    wv = nc.dram_tensor("wv", [D, DH * HPC], F16, kind="ExternalInput")
    wo = nc.dram_tensor("wo", [2, 128, D], BF16, kind="ExternalInput")
    ones = nc.dram_tensor("ones", [1, T], F16, kind="ExternalInput")
    outT = nc.dram_tensor("outT", [D, T], BF16, kind="ExternalOutput")

    with tile.TileContext(nc) as tc:
        with (
            tc.tile_pool(name="const", bufs=1) as constp,
            tc.tile_pool(name="big", bufs=1) as bigp,
            tc.tile_pool(name="sb", bufs=3) as sbp,
            tc.tile_pool(name="psB", bufs=3, space="PSUM") as psB,
            tc.tile_pool(name="psZ", bufs=2, space="PSUM") as psZ,
        ):
            # ---- persistent SBUF ----
            xT_sb = bigp.tile([128, NDC, T], F16, tag="xT")
            wqk_sb = bigp.tile([128, NDC, 128 * HPC], F16, tag="wqk")
            wv_sb = bigp.tile([128, NDC, DH * HPC], F16, tag="wv")
            wo_sb = bigp.tile([128, 2, D], BF16, tag="wo")
            mskG_sb = constp.tile([128, NQB, 128], F32, tag="mskG")
            mskT_sb = constp.tile([128, 128], F32, tag="mskT")
            mskT2_sb = constp.tile([128, 2, 128], F32, tag="mskT2")
            ident = constp.tile([128, 128], F32, tag="ident")
            margin = constp.tile([NQB, 1], F32, tag="margin")
            q_sb = [bigp.tile([65, T], F16, tag=f"q{j}", name=f"q{j}") for j in range(HPC)]
            k_sb = [bigp.tile([65, T], F16, tag=f"k{j}", name=f"k{j}") for j in range(HPC)]
            v_sb = bigp.tile([128, NQB, HPC, DH + 1], BF16, tag="v")
            zT_sb = [bigp.tile([128, T], BF16, tag=f"zp{p}", name=f"zp{p}") for p in range(2)]

            xTr = xT.rearrange("(c p) t -> p c t", p=128)
            wqkr = wqk.rearrange("(c p) m -> p c m", p=128)
            nc.sync.dma_start(wqk_sb[:, :, 0:128], wqkr[:, :, 0:128])
            nc.sync.dma_start(xT_sb[:, 0:4, 0:512], xTr[:, 0:4, 0:512])
            nc.sync.dma_start(xT_sb[:, 4:8, 0:512], xTr[:, 4:8, 0:512])
            nc.sync.dma_start(wv_sb[:], wv.rearrange("(c p) m -> p c m", p=128))
            nc.sync.dma_start(k_sb[0][64:65, :], ones[:])
            nc.sync.dma_start(xT_sb[:, :, 512:1024], xTr[:, :, 512:1024])
            nc.sync.dma_start(wqk_sb[:, :, 128:512], wqkr[:, :, 128:512])
            for s in (2, 3):
                nc.sync.dma_start(
                    xT_sb[:, :, s * 512 : (s + 1) * 512],
                    xTr[:, :, s * 512 : (s + 1) * 512],
                )
            for j in range(1, HPC):
                nc.sync.dma_start(k_sb[j][64:65, :], ones[:])
            nc.sync.dma_start(wo_sb[:], wo.rearrange("p k d -> k p d"))
            make_identity(nc, ident[:])
            nc.vector.memset(margin[:], -MARGIN)
            nc.gpsimd.memset(v_sb[:, :, :, DH : DH + 1], 1.0)
            # on-device masks (Pool is otherwise idle):
            # mskT[k, q] = 0 if k <= q else -1e9
            nc.gpsimd.memset(mskT_sb[:], 0.0)
            nc.gpsimd.affine_select(
                out=mskT_sb[:], in_=mskT_sb[:], compare_op=OP.is_ge,
                fill=-1e9, base=0, pattern=[[1, 128]], channel_multiplier=-1,
            )
            nc.gpsimd.tensor_copy(mskT2_sb[:, 0, :], mskT_sb[:])
            nc.gpsimd.tensor_copy(mskT2_sb[:, 1, :], mskT_sb[:])
            # mskG block qb: 0 if i*(qb+1) <= 128*qb + p else -1e9
            nc.gpsimd.memset(mskG_sb[:], 0.0)
            for qb in range(NQB):
                nc.gpsimd.affine_select(
                    out=mskG_sb[:, qb, :], in_=mskG_sb[:, qb, :],
                    compare_op=OP.is_ge, fill=-1e9, base=128 * qb,
                    pattern=[[-(qb + 1), 128]], channel_multiplier=1,
                )

            # ---- phase 1a: qT/kT slabs. width = number of 512-superblocks.
            # Copies drain in 512-col chunks so critical DVE ops (diag masks,
            # normalize) never queue behind a >1us copy; q goes via ACT only
            # in the front where ACT is idle.
            def qk_slab(j, s0, width=1, q_act=True):
                ps = psB.tile([128, 1024], F32, tag="mm")
                for h in range(width):
                    for c in range(NDC):
                        nc.tensor.matmul(
                            ps[:, h * 512 : (h + 1) * 512],
                            lhsT=(wqk_sb[:, c, j * 128 : (j + 1) * 128]),
                            rhs=(xT_sb[:, c, (s0 + h) * 512 : (s0 + h + 1) * 512]),
                            start=(c == 0),
                            stop=(c == NDC - 1),
                        )
                for h in range(width):
                    lo, hi = (s0 + h) * 512, (s0 + h + 1) * 512
                    if q_act:
                        nc.scalar.activation(
                            q_sb[j][0:64, lo:hi], ps[0:64, h * 512 : (h + 1) * 512],
                            AF.Copy,
                        )
                    else:
                        nc.vector.tensor_copy(
                            q_sb[j][0:64, lo:hi], ps[0:64, h * 512 : (h + 1) * 512]
                        )
                    if False and q_act and width == 2:
                        nc.scalar.activation(
                            k_sb[j][0:64, lo:hi], ps[64:128, h * 512 : (h + 1) * 512],
                            AF.Copy,
                        )
                    else:
                        nc.vector.tensor_copy(
                            k_sb[j][0:64, lo:hi], ps[64:128, h * 512 : (h + 1) * 512]
                        )

            # ---- phase 1b: v natural [t, 4*65] (ones col 64 for denominators)
            def v_chunk(tb):
                ps = psB.tile([128, 512], F32, tag="mm")
                for c in range(NDC):
                    nc.tensor.matmul(
                        ps[:, 0 : DH * HPC],
                        lhsT=(xT_sb[:, c, tb * 128 : (tb + 1) * 128]),
                        rhs=(wv_sb[:, c, :]),
                        start=(c == 0),
                        stop=(c == NDC - 1),
                    )
                nc.vector.tensor_copy(
                    v_sb[:, tb, :, 0:DH],
                    ps[:, 0 : DH * HPC].rearrange("p (j e) -> p j e", j=HPC),
                )

            # ---- phase 2A: stats — negated causal row max. Each 128-row
            # q-block qb samples its causal range at stride (qb+1), giving a
            # uniform 128 samples; 4 blocks share one PSUM tile, one boundary
            # mask add and one 3-D max-reduce. Margin 40 covers sampling.
            m_all = {}

            def stats_group(j, g):
                if g == 0:
                    m_all[j] = sbp.tile(
                        [128, NQB], F32, tag=f"mall{j % 2}", name=f"mall{j}"
                    )
                ma = m_all[j]
                ps = psB.tile([128, 512], F32, tag="mm")
                for i in range(4):
                    qb = 4 * g + i
                    L = 128 * (qb + 1)
                    if qb == 0:
                        rhs = k_sb[j][0:64, 0:128]
                    else:
                        kr = k_sb[j][0:64, 0:L].rearrange(
                            "p (n st) -> p n st", st=qb + 1
                        )
                        rhs = kr[:, :, 0:1]
                    nc.tensor.matmul(
                        ps[:, i * 128 : (i + 1) * 128],
                        lhsT=(q_sb[j][0:64, qb * 128 : (qb + 1) * 128]),
                        rhs=rhs,
                        start=True,
                        stop=True,
                    )
                pv = ps[:, 0:512].rearrange("p (n w) -> p n w", w=128)
                gv = mskG_sb[:, 4 * g : 4 * g + 4, :]
                if g == 0:
                    nc.vector.tensor_tensor(
                        ps[:, 0:512], ps[:, 0:512],
                        mskG_sb[:, 0:4, :].rearrange("p n w -> p (n w)"),
                        op=OP.add,
                    )
                else:
                    nc.vector.tensor_tensor(
                        pv[:, :, 96:128], pv[:, :, 96:128], gv[:, :, 96:128],
                        op=OP.add,
                    )
                nc.vector.tensor_reduce(
                    ma[:, 4 * g : 4 * g + 4], pv[:], axis=AX.X,
                    op=OP.max, negate=True,
                )

            def stats_fin(j, s=None):
                # transpose -m into row 64 of q' (as -max - MARGIN, fp16).
                # s=None: whole row via the idle SP queue (heads 1-3);
                # else one 512-col chunk via the ACT queue (head 0 warmup).
                lo, n = (0, NQB) if s is None else (4 * s, 4)
                pm = psB.tile([128, 512], F32, tag="mm")
                nc.tensor.transpose(
                    pm[0:n, 0:128], m_all[j][:, lo : lo + n], ident[:]
                )
                mT = sbp.tile([NQB, 128], F16, tag="mT", bufs=2)
                nc.scalar.activation(
                    mT[0:n, :], pm[0:n, 0:128], AF.Identity, bias=margin[0:n, :]
                )
                dst = q_sb[j][64:65, lo * 128 : (lo + n) * 128]
                if s is None:
                    nc.sync.dma_start(dst, mT[0:n, :])
                else:
                    nc.scalar.dma_start(dst, mT[0:n, :])

            # ---- phase 3: one [128d, 512t] out^T chunk -> o_sb; DMA per s
            o_sb = {}

            p3_alt = [0]

            def p3_pair(s, db0):
                if db0 == 0:
                    o_sb[s] = sbp.tile(
                        [128, NDC, 512], BF16, tag="osb", bufs=2, name=f"osb{s}"
                    )
                ops = psB.tile([128, 1024], F32, tag="mm")
                for i in range(2):
                    for p in range(2):
                        nc.tensor.matmul(
                            ops[:, i * 512 : (i + 1) * 512],
                            lhsT=(wo_sb[:, p, (db0 + i) * 128 : (db0 + i + 1) * 128]),
                            rhs=(zT_sb[p][:, s * 512 : (s + 1) * 512]),
                            start=(p == 0),
                            stop=(p == 1),
                        )
                dst = o_sb[s][:, db0 : db0 + 2, :].rearrange("p n w -> p (n w)")
                p3_alt[0] ^= 1
                if s == 2 or p3_alt[0]:
                    nc.scalar.activation(dst, ops[:], AF.Copy)
                else:
                    nc.vector.tensor_copy(dst, ops[:])

            def p3_out(s):
                nc.sync.dma_start(
                    outT.rearrange("(g p) t -> p g t", p=128)[
                        :, :, s * 512 : (s + 1) * 512
                    ],
                    o_sb[s][:],
                )

            # ---- phase 2B: the global S-pair pipeline ----------------------
            zps_t = {}
            pending = []
            norm_q = []

            def retire():
                u, pT = pending.pop(0)
                if norm_q:
                    norm_q.pop(0)()
                j, s = u["j"], u["s"]
                nkb = 4 * s + 4
                if (j, s) not in zps_t:
                    zps_t[(j, s)] = psZ.tile(
                        [65, 512], F32, tag="zz", name=f"zps{j}_{s}"
                    )
                zps = zps_t[(j, s)]
                for kb, w, off in u["z"]:
                    nc.tensor.matmul(
                        zps[:, 512 - w : 512],
                        lhsT=v_sb[:, kb, j, :],
                        rhs=pT[:, off : off + w],
                        start=(kb == 0),
                        stop=(kb == nkb - 1),
                        skip_group_check=True,
                    )
                if u["close"]:
                    def mknorm(j=j, s=s, zps=zps):
                        def donorm():
                            r1 = sbp.tile([1, 512], F32, tag="r1", bufs=2)
                            nc.vector.reciprocal(r1[:], zps[64:65, :])
                            rb = sbp.tile([64, 512], F32, tag="rb", bufs=2)
                            nc.gpsimd.partition_broadcast(rb[:], r1[:])
                            p, po = j // 2, 64 * (j % 2)
                            nc.vector.tensor_mul(
                                zT_sb[p][po : po + 64, s * 512 : (s + 1) * 512],
                                zps[0:64, :],
                                rb[:],
                            )
                        return donorm
                    norm_q.append(mknorm())
                    del zps_t[(j, s)]

            def step(u, il):
                j, s, a, b = u["j"], u["s"], u["a"], u["b"]
                wa = 512 if a < 4 * s else 512 - 128 * (a - 4 * s)
                wb = 512 if b < 4 * s else 512 - 128 * (b - 4 * s)
                sps = psB.tile([128, 1024], F32, tag="mm")
                nc.tensor.matmul(
                    sps[:, 0:wa],
                    lhsT=(k_sb[j][0:65, a * 128 : (a + 1) * 128]),
                    rhs=(q_sb[j][0:65, s * 512 + 512 - wa : (s + 1) * 512]),
                    start=True,
                    stop=True,
                )
                nc.tensor.matmul(
                    sps[:, wa : wa + wb],
                    lhsT=(k_sb[j][0:65, b * 128 : (b + 1) * 128]),
                    rhs=(q_sb[j][0:65, s * 512 + 512 - wb : (s + 1) * 512]),
                    start=True,
                    stop=True,
                )
                if a >= 4 * s:
                    # both kbs of a pair are diagonal together: one strided
                    # add covers both [*, 0:128] and [*, wa:wa+128]
                    dv = sps[:, 0 : 2 * wa].rearrange(
                        "p (n w) -> p n w", w=wa
                    )[:, :, 0:128]
                    nc.vector.tensor_tensor(dv, dv, mskT2_sb[:], op=OP.add)
                pT = sbp.tile([128, 1024], BF16, tag="pT", bufs=6)
                nc.scalar.activation(pT[:, 0 : wa + wb], sps[:, 0 : wa + wb], AF.Exp)
                if il is not None:
                    il()
                u["z"] = [(a, wa, 0), (b, wb, wa)]
                pending.append((u, pT))
                if len(pending) > 4:
                    retire()

            def head_units(j):
                us = []
                for s in range(NSB):
                    nkb = 4 * s + 4
                    for a in range(0, nkb, 2):
                        us.append(
                            {"j": j, "s": s, "a": a, "b": a + 1,
                             "close": a + 2 == nkb}
                        )
                return us

            # ---- emission schedule ----
            # front: head-0 projections (512-wide for latency) with head-0
            # stats + per-superblock row-max finalize woven in
            for s in (0, 1):
                qk_slab(0, s, width=1, q_act=True)
                stats_group(0, s)
                stats_fin(0, s=s)

            def vp(t0, t1):
                def fn():
                    v_chunk(t0)
                    v_chunk(t1)
                return fn

            def vq(t0):
                def fn():
                    for t in range(t0, t0 + 4):
                        v_chunk(t)
                return fn

            def qk1s(s):
                return lambda: qk_slab(1, s, width=1, q_act=False)

            def qs2(jn, h):
                return lambda: qk_slab(jn, 2 * h, width=2)

            def stg(jn, g):
                return lambda: stats_group(jn, g)

            def qk0s(s):
                return lambda: qk_slab(0, s, width=1, q_act=True)

            def stgf0(s):
                def fn():
                    stats_group(0, s)
                    stats_fin(0, s=s)
                return fn

            def sfin(jn):
                return lambda: stats_fin(jn)

            def p3p(s, db0):
                return lambda: p3_pair(s, db0)

            il_map = {
                0: [qk1s(0), qk0s(2), stgf0(2), vp(0, 1), vp(2, 3),
                    qk1s(1), stg(1, 0), vq(4), qk0s(3), stgf0(3),
                    stg(1, 1), qk1s(2), qk1s(3), vp(8, 9), vp(10, 11),
                    stg(1, 2), stg(1, 3), sfin(1), vp(12, 13), vp(14, 15)],
                1: [None, None, qs2(2, 0), qs2(2, 1), None, None, stg(2, 0),
                    None, stg(2, 1), None, stg(2, 2), None, stg(2, 3),
                    sfin(2), None, None, None, None, None, None],
                2: [None, None, qs2(3, 0), qs2(3, 1), None, None, stg(3, 0),
                    None, stg(3, 1), None, stg(3, 2), None, stg(3, 3),
                    sfin(3), None, None, None, None, None, None],
                3: [None, None, None, None, None, None, None,
                    p3p(0, 0), p3p(0, 2), p3p(0, 4), p3p(0, 6),
                    lambda: p3_out(0),
                    p3p(1, 0), p3p(1, 2), p3p(1, 4), p3p(1, 6),
                    lambda: p3_out(1),
                    p3p(2, 0), p3p(2, 2), p3p(2, 4)],
            }

            for j in range(HPC):
                il_list = il_map[j]
                for i, u in enumerate(head_units(j)):
                    step(u, il_list[i])
            # ---- tail: retire the last units with a split normalize so
            # phase 3's last superblock overlaps the final softmax columns.
            outv = outT.rearrange("(g p) t -> p g t", p=128)
            nc.sync.dma_start(
                outv[:, 0:4, 1024:1536], o_sb[2][:, 0:4, :]
            )
            while len(pending) > 2:
                retire()
            while norm_q:
                norm_q.pop(0)()
            p3_pair(2, 6)
            nc.sync.dma_start(
                outv[:, 4:8, 1024:1536], o_sb[2][:, 4:8, :]
            )
            pending[1][0]["close"] = False
            retire()  # z for kbs 12,13 -> cols [0:256] of (3,3) complete
            zps33 = zps_t[(3, 3)]
            r1a = sbp.tile([1, 512], F32, tag="r1", bufs=2, name="r1a")
            nc.vector.reciprocal(r1a[0:1, 0:256], zps33[64:65, 0:256])
            rba = sbp.tile([64, 512], F32, tag="rb", bufs=2, name="rba")
            nc.gpsimd.partition_broadcast(rba[0:64, 0:256], r1a[0:1, 0:256])
            nc.vector.tensor_mul(
                zT_sb[1][64:128, 1536:1792], zps33[0:64, 0:256], rba[0:64, 0:256]
            )
            retire()  # z for kbs 14,15 -> cols [256:512] complete
            r1b = sbp.tile([1, 512], F32, tag="r1", bufs=2, name="r1b")
            nc.vector.reciprocal(r1b[0:1, 0:256], zps33[64:65, 256:512])
            rbb = sbp.tile([64, 512], F32, tag="rb", bufs=2, name="rbb")
            nc.gpsimd.partition_broadcast(rbb[0:64, 0:256], r1b[0:1, 0:256])
            nc.vector.tensor_mul(
                zT_sb[1][64:128, 1792:2048], zps33[0:64, 256:512], rbb[0:64, 0:256]
            )
            del zps_t[(3, 3)]

            o_sb[3] = sbp.tile(
                [128, NDC, 512], BF16, tag="osb", bufs=2, name="osb3"
            )

            def p3_pair_h(db0, h):
                ops = psB.tile([128, 1024], F32, tag="mm")
                for i in range(2):
                    for p in range(2):
                        nc.tensor.matmul(
                            ops[:, i * 512 + h * 256 : i * 512 + (h + 1) * 256],
                            lhsT=(wo_sb[:, p, (db0 + i) * 128 : (db0 + i + 1) * 128]),
                            rhs=(zT_sb[p][:, 1536 + h * 256 : 1792 + h * 256]),
                            start=(p == 0),
                            stop=(p == 1),
                        )
                dst = o_sb[3][:, db0 : db0 + 2, h * 256 : (h + 1) * 256]
                srcv = ops[:, 0:1024].rearrange("p (i w) -> p i w", w=512)[
                    :, :, h * 256 : (h + 1) * 256
                ]
                if db0 % 4 == 0:
                    nc.scalar.activation(dst, srcv, AF.Copy)
                else:
                    nc.vector.tensor_copy(dst, srcv)

            for h in range(2):
                for db0 in (0, 2, 4, 6):
                    p3_pair_h(db0, h)
                for g0 in (0, 4):
                    nc.sync.dma_start(
                        outv[:, g0 : g0 + 4,
                             1536 + h * 256 : 1792 + h * 256],
                        o_sb[3][:, g0 : g0 + 4, h * 256 : (h + 1) * 256],
                    )
    nc.compile()
    return nc


_NC = None


def _get_nc():
    global _NC
    if _NC is None:
        _NC = build_nc()
    return _NC


def _make_in_maps(inputs):
    x = np.ascontiguousarray(np.asarray(inputs["residual_stream"], dtype=np.float32))
    W_Q = np.asarray(inputs["W_Q"], dtype=np.float32)
    W_K = np.asarray(inputs["W_K"], dtype=np.float32)
    W_V = np.asarray(inputs["W_V"], dtype=np.float32)
    W_O = np.asarray(inputs["W_output"], dtype=np.float32)

    try:
        import ml_dtypes
        bf16 = ml_dtypes.bfloat16
    except ImportError:
        bf16 = np.float32

    qi = np.arange(128)
    mskS = np.where(qi[None, :] <= qi[:, None], 0.0, -1e9).astype(np.float32)  # [q,k]
    mskT = np.ascontiguousarray(mskS.T)  # [k,q]
    si = np.arange(128)
    mskG = np.zeros((128, NQB, 128), np.float32)
    for qb in range(NQB):
        mskG[:, qb, :] = np.where(
            si[None, :] * (qb + 1) <= 128 * qb + qi[:, None], 0.0, -1e9
        )
    mskG = np.ascontiguousarray(mskG.reshape(128, NQB * 128))

    in_maps = []
    for c in range(8):
        b, hg = c // 4, c % 4
        hs = [4 * hg + j for j in range(HPC)]
        wqk_h = np.concatenate(
            [np.concatenate([W_Q[h] / 8.0, W_K[h]], axis=1) for h in hs], axis=1
        )  # [1024, 512]
        wv_h = np.concatenate([W_V[h] for h in hs], axis=1)  # [1024, 256]
        wo_h = np.stack(
            [np.concatenate([W_O[hs[2 * p]], W_O[hs[2 * p + 1]]], axis=0)
             for p in range(2)]
        )  # [2, 128, 1024]
        in_maps.append(
            {
                "xT": np.ascontiguousarray(x[b].T).astype(np.float16),
                "wqk": np.ascontiguousarray(wqk_h).astype(np.float16),
                "wv": np.ascontiguousarray(wv_h).astype(np.float16),
                "wo": np.ascontiguousarray(wo_h).astype(bf16),
                "ones": np.ones((1, T), np.float16),
            }
        )
    return in_maps


def _postprocess(results, inputs):
    W_O = np.asarray(inputs["W_output"], dtype=np.float32)
    b_V = np.asarray(inputs["b_V"], dtype=np.float32)
    b_out = np.asarray(inputs["b_output"], dtype=np.float32)
    out = np.zeros((2, T, D), dtype=np.float32)
    for c in range(8):
        out[c // 4] += results[c]["outT"].T.astype(np.float32)
    # z = P @ v + b_V (P rows sum to 1) -> fold b_V through W_O on the host
    const = np.einsum("he,hed->d", b_V, W_O) + b_out
    out += const[None, None, :]
    return out


def kernel(**inputs):
    from concourse.bass_utils import run_bass_kernel_spmd

    nc = _get_nc()
    res = run_bass_kernel_spmd(nc, _make_in_maps(inputs), core_ids=list(range(8)))
    return _postprocess(res.results, inputs)


def kernel_traced(**inputs):
    """Returns (output, exec_time_ns or None) using a traced run."""
    from concourse.bass_utils import run_bass_kernel_spmd

    nc = _get_nc()
    res = run_bass_kernel_spmd(
        nc, _make_in_maps(inputs), core_ids=list(range(8)), trace=True
    )
    return _postprocess(res.results, inputs), res.exec_time_ns


# revision 8
# speedup vs baseline: 1.0885x; 1.0063x over previous
"""Multi-head causal attention on 8 TRN2 NeuronCores.

Sharding: data-parallel over batch (2) x tensor-parallel over heads (4 groups
of 4 heads) = 8 cores. Each core emits a partial output projection in
transposed layout; the host sums the 4 partials per batch and folds the
(zero) biases.

Precision: x/W_QK/q/k fp16 (tf32-class mantissa; halves input DMA, full-rate
matmuls at any width), V/P/z/W_O/out bf16, all accumulation f32 in PSUM.
Softmax max-subtraction uses a sampled causal row max (each 128-row q-block
samples its range at stride (qb+1) = 128 samples) with a 40-unit safety
margin; the un-normalized exp weights stay finite in bf16/f32 by
construction (measured emulation rel err 4.5e-3 vs the noisy reference).

Engine plan (GPSIMD cannot touch PSUM on this HW):
  PE   S^T pairs, z accumulation, projections, stats samples, transposes
  ACT  exp over paired S^T blocks (2-bank PSUM tiles), q/k PSUM drains,
       row-max formatting + its DMA queue, half of phase-3 drains
  DVE  v drain, fused diagonal-pair causal masks, stats boundary masks +
       grouped 3-D max-reduces, denominator reciprocal, z normalize
  Pool on-device mask generation (affine_select), denominator broadcast
  SP   batched fp16/bf16 input DMAs, per-superblock output DMAs

Schedule: one global software pipeline over every S-pair unit of every
head. A step emits [2 S^T matmuls, fused diag mask, exp], then one
interleave closure (v chunk / next head's projection slab / next head's
stats group / phase-3 chunk), then retires the z matmuls of the unit four
steps back; normalize (reciprocal+broadcast+multiply) is deferred one
further step so it lands in the mask-free region of the next superblock.
The front consumes x slabs as they stream; the tail splits the last
superblock's normalize and phase 3 in halves to overlap the final DMAs.

CoreSim cost model: 149.7us vs 222.3us baseline (1.49x); rel err 4.5e-3.
"""

import os
import sys

import numpy as np

for _p in ("/opt/trn_rl_repo", "/root/.axon_site/_ro/trn_rl_repo"):
    if os.path.isdir(_p) and _p not in sys.path:
        sys.path.insert(0, _p)

import concourse.bass as bass
from concourse import bacc
import concourse.tile as tile
from concourse import mybir
from concourse.masks import make_identity

F32 = mybir.dt.float32
F16 = mybir.dt.float16
BF16 = mybir.dt.bfloat16
AX = mybir.AxisListType
OP = mybir.AluOpType
AF = mybir.ActivationFunctionType

T = 2048
D = 1024
HPC = 4          # heads per core
DH = 64
NQB = T // 128   # 16
NSB = T // 512   # 4
NDC = D // 128   # 8
MARGIN = 40.0


def build_nc():
    nc = bacc.Bacc("TRN2", target_bir_lowering=False)
    xT = nc.dram_tensor("xT", [D, T], F16, kind="ExternalInput")
    wqk = nc.dram_tensor("wqk", [D, 128 * HPC], F16, kind="ExternalInput")
    wv = nc.dram_tensor("wv", [D, DH * HPC], F16, kind="ExternalInput")
    wo = nc.dram_tensor("wo", [2, 128, D], BF16, kind="ExternalInput")
    ones = nc.dram_tensor("ones", [1, T], F16, kind="ExternalInput")
    outT = nc.dram_tensor("outT", [D, T], BF16, kind="ExternalOutput")

    with tile.TileContext(nc) as tc:
        with (
            tc.tile_pool(name="const", bufs=1) as constp,
            tc.tile_pool(name="big", bufs=1) as bigp,
            tc.tile_pool(name="sb", bufs=3) as sbp,
            tc.tile_pool(name="psB", bufs=3, space="PSUM") as psB,
            tc.tile_pool(name="psZ", bufs=2, space="PSUM") as psZ,
        ):
            # ---- persistent SBUF ----
            xT_sb = bigp.tile([128, NDC, T], F16, tag="xT")
            wqk_sb = bigp.tile([128, NDC, 128 * HPC], F16, tag="wqk")
            wv_sb = bigp.tile([128, NDC, DH * HPC], F16, tag="wv")
            wo_sb = bigp.tile([128, 2, D], BF16, tag="wo")
            mskG_sb = constp.tile([128, NQB, 128], F32, tag="mskG")
            mskT_sb = constp.tile([128, 128], F32, tag="mskT")
            mskT2_sb = constp.tile([128, 2, 128], F32, tag="mskT2")
            ident = constp.tile([128, 128], F32, tag="ident")
            margin = constp.tile([NQB, 1], F32, tag="margin")
            q_sb = [bigp.tile([65, T], F16, tag=f"q{j}", name=f"q{j}") for j in range(HPC)]
            k_sb = [bigp.tile([65, T], F16, tag=f"k{j}", name=f"k{j}") for j in range(HPC)]
            v_sb = bigp.tile([128, NQB, HPC, DH + 1], BF16, tag="v")
            zT_sb = [bigp.tile([128, T], BF16, tag=f"zp{p}", name=f"zp{p}") for p in range(2)]

            xTr = xT.rearrange("(c p) t -> p c t", p=128)
            wqkr = wqk.rearrange("(c p) m -> p c m", p=128)
            nc.sync.dma_start(wqk_sb[:, :, 0:128], wqkr[:, :, 0:128])
            nc.sync.dma_start(xT_sb[:, 0:4, 0:512], xTr[:, 0:4, 0:512])
            nc.sync.dma_start(xT_sb[:, 4:8, 0:512], xTr[:, 4:8, 0:512])
            nc.sync.dma_start(wv_sb[:], wv.rearrange("(c p) m -> p c m", p=128))
            nc.sync.dma_start(k_sb[0][64:65, :], ones[:])
            nc.sync.dma_start(xT_sb[:, 0:4, 512:1024], xTr[:, 0:4, 512:1024])
            nc.sync.dma_start(xT_sb[:, 4:8, 512:1024], xTr[:, 4:8, 512:1024])
            nc.sync.dma_start(wqk_sb[:, :, 128:512], wqkr[:, :, 128:512])
            for s in (2, 3):
                for c0 in (0, 4):
                    nc.sync.dma_start(
                        xT_sb[:, c0 : c0 + 4, s * 512 : (s + 1) * 512],
                        xTr[:, c0 : c0 + 4, s * 512 : (s + 1) * 512],
                    )
            for j in range(1, HPC):
                nc.sync.dma_start(k_sb[j][64:65, :], ones[:])
            nc.sync.dma_start(wo_sb[:], wo.rearrange("p k d -> k p d"))
            make_identity(nc, ident[:])
            nc.vector.memset(margin[:], -MARGIN)
            nc.gpsimd.memset(v_sb[:, :, :, DH : DH + 1], 1.0)
            # on-device masks (Pool is otherwise idle):
            # mskT[k, q] = 0 if k <= q else -1e9
            nc.gpsimd.memset(mskT_sb[:], 0.0)
            nc.gpsimd.affine_select(
                out=mskT_sb[:], in_=mskT_sb[:], compare_op=OP.is_ge,
                fill=-1e9, base=0, pattern=[[1, 128]], channel_multiplier=-1,
            )
            nc.gpsimd.tensor_copy(mskT2_sb[:, 0, :], mskT_sb[:])
            nc.gpsimd.tensor_copy(mskT2_sb[:, 1, :], mskT_sb[:])
            # mskG block qb: 0 if i*(qb+1) <= 128*qb + p else -1e9
            nc.gpsimd.memset(mskG_sb[:], 0.0)
            for qb in range(NQB):
                nc.gpsimd.affine_select(
                    out=mskG_sb[:, qb, :], in_=mskG_sb[:, qb, :],
                    compare_op=OP.is_ge, fill=-1e9, base=128 * qb,
                    pattern=[[-(qb + 1), 128]], channel_multiplier=1,
                )

            # ---- phase 1a: qT/kT slabs. width = number of 512-superblocks.
            # Copies drain in 512-col chunks so critical DVE ops (diag masks,
            # normalize) never queue behind a >1us copy; q goes via ACT only
            # in the front where ACT is idle.
            def qk_slab(j, s0, width=1, q_act=True):
                ps = psB.tile([128, 1024], F32, tag="mm")
                for h in range(width):
                    for c in range(NDC):
                        nc.tensor.matmul(
                            ps[:, h * 512 : (h + 1) * 512],
                            lhsT=(wqk_sb[:, c, j * 128 : (j + 1) * 128]),
                            rhs=(xT_sb[:, c, (s0 + h) * 512 : (s0 + h + 1) * 512]),
                            start=(c == 0),
                            stop=(c == NDC - 1),
                        )
                for h in range(width):
                    lo, hi = (s0 + h) * 512, (s0 + h + 1) * 512
                    if q_act:
                        nc.scalar.activation(
                            q_sb[j][0:64, lo:hi], ps[0:64, h * 512 : (h + 1) * 512],
                            AF.Copy,
                        )
                    else:
                        nc.vector.tensor_copy(
                            q_sb[j][0:64, lo:hi], ps[0:64, h * 512 : (h + 1) * 512]
                        )
                    if False and q_act and width == 2:
                        nc.scalar.activation(
                            k_sb[j][0:64, lo:hi], ps[64:128, h * 512 : (h + 1) * 512],
                            AF.Copy,
                        )
                    else:
                        nc.vector.tensor_copy(
                            k_sb[j][0:64, lo:hi], ps[64:128, h * 512 : (h + 1) * 512]
                        )

            # ---- phase 1b: v natural [t, 4*65] (ones col 64 for denominators)
            def v_chunk(tb):
                ps = psB.tile([128, 512], F32, tag="mm")
                for c in range(NDC):
                    nc.tensor.matmul(
                        ps[:, 0 : DH * HPC],
                        lhsT=(xT_sb[:, c, tb * 128 : (tb + 1) * 128]),
                        rhs=(wv_sb[:, c, :]),
                        start=(c == 0),
                        stop=(c == NDC - 1),
                    )
                nc.vector.tensor_copy(
                    v_sb[:, tb, :, 0:DH],
                    ps[:, 0 : DH * HPC].rearrange("p (j e) -> p j e", j=HPC),
                )

            # ---- phase 2A: stats — negated causal row max. Each 128-row
            # q-block qb samples its causal range at stride (qb+1), giving a
            # uniform 128 samples; 4 blocks share one PSUM tile, one boundary
            # mask add and one 3-D max-reduce. Margin 40 covers sampling.
            m_all = {}

            def stats_group(j, g, nq=4):
                if g == 0:
                    m_all[j] = sbp.tile(
                        [128, NQB], F32, tag=f"mall{j % 2}", name=f"mall{j}"
                    )
                ma = m_all[j]
                ps = psB.tile([128, 128 * nq], F32, tag="mm")
                for i in range(nq):
                    qb = nq * g + i
                    L = 128 * (qb + 1)
                    if qb == 0:
                        rhs = k_sb[j][0:64, 0:128]
                    else:
                        kr = k_sb[j][0:64, 0:L].rearrange(
                            "p (n st) -> p n st", st=qb + 1
                        )
                        rhs = kr[:, :, 0:1]
                    nc.tensor.matmul(
                        ps[:, i * 128 : (i + 1) * 128],
                        lhsT=(q_sb[j][0:64, qb * 128 : (qb + 1) * 128]),
                        rhs=rhs,
                        start=True,
                        stop=True,
                    )
                W = 128 * nq
                pv = ps[:, 0:W].rearrange("p (n w) -> p n w", w=128)
                gv = mskG_sb[:, nq * g : nq * g + nq, :]
                if g == 0:
                    # first group holds qb0 (full causal mask) + wide edges
                    nc.vector.tensor_tensor(
                        ps[:, 0:W], ps[:, 0:W],
                        gv.rearrange("p n w -> p (n w)"),
                        op=OP.add,
                    )
                else:
                    nc.vector.tensor_tensor(
                        pv[:, :, 96:128], pv[:, :, 96:128], gv[:, :, 96:128],
                        op=OP.add,
                    )
                nc.vector.tensor_reduce(
                    ma[:, nq * g : nq * g + nq], pv[:], axis=AX.X,
                    op=OP.max, negate=True,
                )

            def stats_fin(j, s=None):
                # transpose -m into row 64 of q' (as -max - MARGIN, fp16).
                # s=None: whole row via the idle SP queue (heads 1-3);
                # else one 512-col chunk via the ACT queue (head 0 warmup).
                lo, n = (0, NQB) if s is None else (4 * s, 4)
                pm = psB.tile([128, 512], F32, tag="mm")
                nc.tensor.transpose(
                    pm[0:n, 0:128], m_all[j][:, lo : lo + n], ident[:]
                )
                mT = sbp.tile([NQB, 128], F16, tag="mT", bufs=2)
                nc.scalar.activation(
                    mT[0:n, :], pm[0:n, 0:128], AF.Identity, bias=margin[0:n, :]
                )
                dst = q_sb[j][64:65, lo * 128 : (lo + n) * 128]
                if s is None:
                    nc.sync.dma_start(dst, mT[0:n, :])
                else:
                    nc.scalar.dma_start(dst, mT[0:n, :])

            # ---- phase 3: one [128d, 512t] out^T chunk -> o_sb; DMA per s
            o_sb = {}

            p3_alt = [0]

            def p3_pair(s, db0):
                if db0 == 0:
                    o_sb[s] = sbp.tile(
                        [128, NDC, 512], BF16, tag="osb", bufs=2, name=f"osb{s}"
                    )
                ops = psB.tile([128, 1024], F32, tag="mm")
                for i in range(2):
                    for p in range(2):
                        nc.tensor.matmul(
                            ops[:, i * 512 : (i + 1) * 512],
                            lhsT=(wo_sb[:, p, (db0 + i) * 128 : (db0 + i + 1) * 128]),
                            rhs=(zT_sb[p][:, s * 512 : (s + 1) * 512]),
                            start=(p == 0),
                            stop=(p == 1),
                        )
                dst = o_sb[s][:, db0 : db0 + 2, :].rearrange("p n w -> p (n w)")
                p3_alt[0] ^= 1
                if s == 2 or p3_alt[0]:
                    nc.scalar.activation(dst, ops[:], AF.Copy)
                else:
                    nc.vector.tensor_copy(dst, ops[:])

            def p3_out(s):
                nc.sync.dma_start(
                    outT.rearrange("(g p) t -> p g t", p=128)[
                        :, :, s * 512 : (s + 1) * 512
                    ],
                    o_sb[s][:],
                )

            # ---- phase 2B: the global S-pair pipeline ----------------------
            zps_t = {}
            pending = []
            norm_q = []

            def retire():
                u, pT = pending.pop(0)
                if norm_q:
                    norm_q.pop(0)()
                j, s = u["j"], u["s"]
                nkb = 4 * s + 4
                if (j, s) not in zps_t:
                    zps_t[(j, s)] = psZ.tile(
                        [65, 512], F32, tag="zz", name=f"zps{j}_{s}"
                    )
                zps = zps_t[(j, s)]
                for kb, w, off in u["z"]:
                    nc.tensor.matmul(
                        zps[:, 512 - w : 512],
                        lhsT=v_sb[:, kb, j, :],
                        rhs=pT[:, off : off + w],
                        start=(kb == 0),
                        stop=(kb == nkb - 1),
                        skip_group_check=True,
                    )
                if u["close"]:
                    def mknorm(j=j, s=s, zps=zps):
                        def donorm():
                            r1 = sbp.tile([1, 512], F32, tag="r1", bufs=2)
                            nc.vector.reciprocal(r1[:], zps[64:65, :])
                            rb = sbp.tile([64, 512], F32, tag="rb", bufs=2)
                            nc.gpsimd.partition_broadcast(rb[:], r1[:])
                            p, po = j // 2, 64 * (j % 2)
                            nc.vector.tensor_mul(
                                zT_sb[p][po : po + 64, s * 512 : (s + 1) * 512],
                                zps[0:64, :],
                                rb[:],
                            )
                        return donorm
                    norm_q.append(mknorm())
                    del zps_t[(j, s)]

            def step(u, il):
                j, s, a, b = u["j"], u["s"], u["a"], u["b"]
                wa = 512 if a < 4 * s else 512 - 128 * (a - 4 * s)
                wb = 512 if b < 4 * s else 512 - 128 * (b - 4 * s)
                sps = psB.tile([128, 1024], F32, tag="mm")
                nc.tensor.matmul(
                    sps[:, 0:wa],
                    lhsT=(k_sb[j][0:65, a * 128 : (a + 1) * 128]),
                    rhs=(q_sb[j][0:65, s * 512 + 512 - wa : (s + 1) * 512]),
                    start=True,
                    stop=True,
                )
                nc.tensor.matmul(
                    sps[:, wa : wa + wb],
                    lhsT=(k_sb[j][0:65, b * 128 : (b + 1) * 128]),
                    rhs=(q_sb[j][0:65, s * 512 + 512 - wb : (s + 1) * 512]),
                    start=True,
                    stop=True,
                )
                if a >= 4 * s:
                    # both kbs of a pair are diagonal together: one strided
                    # add covers both [*, 0:128] and [*, wa:wa+128]
                    dv = sps[:, 0 : 2 * wa].rearrange(
                        "p (n w) -> p n w", w=wa
                    )[:, :, 0:128]
                    nc.vector.tensor_tensor(dv, dv, mskT2_sb[:], op=OP.add)
                pT = sbp.tile([128, 1024], BF16, tag="pT", bufs=6)
                nc.scalar.activation(pT[:, 0 : wa + wb], sps[:, 0 : wa + wb], AF.Exp)
                if il is not None:
                    il()
                u["z"] = [(a, wa, 0), (b, wb, wa)]
                pending.append((u, pT))
                if len(pending) > 4:
                    retire()

            def head_units(j):
                us = []
                for s in range(NSB):
                    nkb = 4 * s + 4
                    for a in range(0, nkb, 2):
                        us.append(
                            {"j": j, "s": s, "a": a, "b": a + 1,
                             "close": a + 2 == nkb}
                        )
                return us

            # ---- emission schedule ----
            # front: head-0 projections (512-wide for latency) with head-0
            # stats + per-superblock row-max finalize woven in
            for s in (0, 1):
                qk_slab(0, s, width=1, q_act=True)
                stats_group(0, s)
                stats_fin(0, s=s)

            def vp(t0, t1):
                def fn():
                    v_chunk(t0)
                    v_chunk(t1)
                return fn

            def vq(t0):
                def fn():
                    for t in range(t0, t0 + 4):
                        v_chunk(t)
                return fn

            def qk1s(s):
                return lambda: qk_slab(1, s, width=1, q_act=False)

            def qs2(jn, h):
                return lambda: qk_slab(jn, 2 * h, width=2)

            def stg(jn, g):
                return lambda: stats_group(jn, g)

            def qk0s(s):
                return lambda: qk_slab(0, s, width=1, q_act=True)

            def stgf0(s):
                def fn():
                    stats_group(0, s)
                    stats_fin(0, s=s)
                return fn

            def sfin(jn):
                return lambda: stats_fin(jn)

            def p3p(s, db0):
                return lambda: p3_pair(s, db0)

            il_map = {
                0: [qk1s(0), qk0s(2), stgf0(2), vp(0, 1), vp(2, 3),
                    qk1s(1), qk1s(2), vq(4), qk0s(3), stgf0(3),
                    stg(1, 0), stg(1, 1), stg(1, 2), qk1s(3), vq(8),
                    stg(1, 3), sfin(1), vp(12, 13), vp(14, 15), None],
                1: [None, None, qs2(2, 0), qs2(2, 1), None, None, stg(2, 0),
                    None, stg(2, 1), None, stg(2, 2), None, stg(2, 3),
                    sfin(2), None, None, None, None, None, None],
                2: [None, None, qs2(3, 0), qs2(3, 1), None, None, stg(3, 0),
                    None, stg(3, 1), None, stg(3, 2), None, stg(3, 3),
                    sfin(3), None, None, None, None, None, None],
                3: [None, None, None, None, None, None, None,
                    p3p(0, 0), p3p(0, 2), p3p(0, 4), p3p(0, 6),
                    lambda: p3_out(0),
                    p3p(1, 0), p3p(1, 2), p3p(1, 4), p3p(1, 6),
                    lambda: p3_out(1),
                    p3p(2, 0), p3p(2, 2), p3p(2, 4)],
            }

            for j in range(HPC):
                il_list = il_map[j]
                for i, u in enumerate(head_units(j)):
                    step(u, il_list[i])
            # ---- tail: retire the last units with a split normalize so
            # phase 3's last superblock overlaps the final softmax columns.
            outv = outT.rearrange("(g p) t -> p g t", p=128)
            nc.sync.dma_start(
                outv[:, 0:4, 1024:1536], o_sb[2][:, 0:4, :]
            )
            while len(pending) > 2:
                retire()
            while norm_q:
                norm_q.pop(0)()
            p3_pair(2, 6)
            nc.sync.dma_start(
                outv[:, 4:8, 1024:1536], o_sb[2][:, 4:8, :]
            )
            pending[1][0]["close"] = False
            retire()  # z for kbs 12,13 -> cols [0:256] of (3,3) complete
            zps33 = zps_t[(3, 3)]
            r1a = sbp.tile([1, 512], F32, tag="r1", bufs=2, name="r1a")
            nc.vector.reciprocal(r1a[0:1, 0:256], zps33[64:65, 0:256])
            rba = sbp.tile([64, 512], F32, tag="rb", bufs=2, name="rba")
            nc.gpsimd.partition_broadcast(rba[0:64, 0:256], r1a[0:1, 0:256])
            nc.vector.tensor_mul(
                zT_sb[1][64:128, 1536:1792], zps33[0:64, 0:256], rba[0:64, 0:256]
            )
            retire()  # z for kbs 14,15 -> cols [256:512] complete
            r1b = sbp.tile([1, 512], F32, tag="r1", bufs=2, name="r1b")
            nc.vector.reciprocal(r1b[0:1, 0:256], zps33[64:65, 256:512])
            rbb = sbp.tile([64, 512], F32, tag="rb", bufs=2, name="rbb")
            nc.gpsimd.partition_broadcast(rbb[0:64, 0:256], r1b[0:1, 0:256])
            nc.vector.tensor_mul(
                zT_sb[1][64:128, 1792:2048], zps33[0:64, 256:512], rbb[0:64, 0:256]
            )
            del zps_t[(3, 3)]

            o_sb[3] = sbp.tile(
                [128, NDC, 512], BF16, tag="osb", bufs=2, name="osb3"
            )

            def p3_pair_h(db0, h):
                ops = psB.tile([128, 1024], F32, tag="mm")
                for i in range(2):
                    for p in range(2):
                        nc.tensor.matmul(
                            ops[:, i * 512 + h * 256 : i * 512 + (h + 1) * 256],
                            lhsT=(wo_sb[:, p, (db0 + i) * 128 : (db0 + i + 1) * 128]),
                            rhs=(zT_sb[p][:, 1536 + h * 256 : 1792 + h * 256]),
                            start=(p == 0),
                            stop=(p == 1),
                        )
                dst = o_sb[3][:, db0 : db0 + 2, h * 256 : (h + 1) * 256]
                srcv = ops[:, 0:1024].rearrange("p (i w) -> p i w", w=512)[
                    :, :, h * 256 : (h + 1) * 256
                ]
                if db0 % 4 == 0:
                    nc.scalar.activation(dst, srcv, AF.Copy)
                else:
                    nc.vector.tensor_copy(dst, srcv)

            for h in range(2):
                for db0 in (0, 2):
                    p3_pair_h(db0, h)
                nc.sync.dma_start(
                    outv[:, 0:4, 1536 + h * 256 : 1792 + h * 256],
                    o_sb[3][:, 0:4, h * 256 : (h + 1) * 256],
                )
                for db0 in (4, 6):
                    p3_pair_h(db0, h)
                nc.sync.dma_start(
                    outv[:, 4:8, 1536 + h * 256 : 1792 + h * 256],
                    o_sb[3][:, 4:8, h * 256 : (h + 1) * 256],
                )
    nc.compile()
    return nc


_NC = None


def _get_nc():
    global _NC
    if _NC is None:
        _NC = build_nc()
    return _NC


def _make_in_maps(inputs):
    x = np.ascontiguousarray(np.asarray(inputs["residual_stream"], dtype=np.float32))
    W_Q = np.asarray(inputs["W_Q"], dtype=np.float32)
    W_K = np.asarray(inputs["W_K"], dtype=np.float32)
    W_V = np.asarray(inputs["W_V"], dtype=np.float32)
    W_O = np.asarray(inputs["W_output"], dtype=np.float32)

    try:
        import ml_dtypes
        bf16 = ml_dtypes.bfloat16
    except ImportError:
        bf16 = np.float32

    qi = np.arange(128)
    mskS = np.where(qi[None, :] <= qi[:, None], 0.0, -1e9).astype(np.float32)  # [q,k]
    mskT = np.ascontiguousarray(mskS.T)  # [k,q]
    si = np.arange(128)
    mskG = np.zeros((128, NQB, 128), np.float32)
    for qb in range(NQB):
        mskG[:, qb, :] = np.where(
            si[None, :] * (qb + 1) <= 128 * qb + qi[:, None], 0.0, -1e9
        )
    mskG = np.ascontiguousarray(mskG.reshape(128, NQB * 128))

    in_maps = []
    for c in range(8):
        b, hg = c // 4, c % 4
        hs = [4 * hg + j for j in range(HPC)]
        wqk_h = np.concatenate(
            [np.concatenate([W_Q[h] / 8.0, W_K[h]], axis=1) for h in hs], axis=1
        )  # [1024, 512]
        wv_h = np.concatenate([W_V[h] for h in hs], axis=1)  # [1024, 256]
        wo_h = np.stack(
            [np.concatenate([W_O[hs[2 * p]], W_O[hs[2 * p + 1]]], axis=0)
             for p in range(2)]
        )  # [2, 128, 1024]
        in_maps.append(
            {
                "xT": np.ascontiguousarray(x[b].T).astype(np.float16),
                "wqk": np.ascontiguousarray(wqk_h).astype(np.float16),
                "wv": np.ascontiguousarray(wv_h).astype(np.float16),
                "wo": np.ascontiguousarray(wo_h).astype(bf16),
                "ones": np.ones((1, T), np.float16),
            }
        )
    return in_maps


def _postprocess(results, inputs):
    W_O = np.asarray(inputs["W_output"], dtype=np.float32)
    b_V = np.asarray(inputs["b_V"], dtype=np.float32)
    b_out = np.asarray(inputs["b_output"], dtype=np.float32)
    out = np.zeros((2, T, D), dtype=np.float32)
    for c in range(8):
        out[c // 4] += results[c]["outT"].T.astype(np.float32)
    # z = P @ v + b_V (P rows sum to 1) -> fold b_V through W_O on the host
    const = np.einsum("he,hed->d", b_V, W_O) + b_out
    out += const[None, None, :]
    return out


def kernel(**inputs):
    from concourse.bass_utils import run_bass_kernel_spmd

    nc = _get_nc()
    res = run_bass_kernel_spmd(nc, _make_in_maps(inputs), core_ids=list(range(8)))
    return _postprocess(res.results, inputs)


def kernel_traced(**inputs):
    """Returns (output, exec_time_ns or None) using a traced run."""
    from concourse.bass_utils import run_bass_kernel_spmd

    nc = _get_nc()
    res = run_bass_kernel_spmd(
        nc, _make_in_maps(inputs), core_ids=list(range(8)), trace=True
    )
    return _postprocess(res.results, inputs), res.exec_time_ns


# revision 9
# speedup vs baseline: 1.0926x; 1.0038x over previous
"""Multi-head causal attention on 8 TRN2 NeuronCores.

Sharding: data-parallel over batch (2) x tensor-parallel over heads (4 groups
of 4 heads) = 8 cores. Each core emits a partial output projection in
transposed layout; the host sums the 4 partials per batch and folds the
(zero) biases.

Precision: x/W_QK/q/k fp16 (tf32-class mantissa; halves input DMA, full-rate
matmuls at any width), V/P/z/W_O/out bf16, all accumulation f32 in PSUM.
Softmax max-subtraction uses a sampled causal row max (each 128-row q-block
samples its range at stride (qb+1) = 128 samples) with a 40-unit safety
margin; the un-normalized exp weights stay finite in bf16/f32 by
construction (measured emulation rel err 4.5e-3 vs the noisy reference).

Engine plan (GPSIMD cannot touch PSUM on this HW):
  PE   S^T pairs, z accumulation, projections, stats samples, transposes
  ACT  exp over paired S^T blocks (2-bank PSUM tiles), q/k PSUM drains,
       row-max formatting + its DMA queue, half of phase-3 drains
  DVE  v drain, fused diagonal-pair causal masks, stats boundary masks +
       grouped 3-D max-reduces, denominator reciprocal, z normalize
  Pool on-device mask generation (affine_select), denominator broadcast
  SP   batched fp16/bf16 input DMAs, per-superblock output DMAs

Schedule: one global software pipeline over every S-pair unit of every
head. A step emits [2 S^T matmuls, fused diag mask, exp], then one
interleave closure (v chunk / next head's projection slab / next head's
stats group / phase-3 chunk), then retires the z matmuls of the unit four
steps back; normalize (reciprocal+broadcast+multiply) is deferred one
further step so it lands in the mask-free region of the next superblock.
The front consumes x slabs as they stream; the tail splits the last
superblock's normalize and phase 3 in halves to overlap the final DMAs.

CoreSim cost model: 148.7us vs 222.3us baseline (1.50x); rel err 4.5e-3.
"""

import os
import sys

import numpy as np

for _p in ("/opt/trn_rl_repo", "/root/.axon_site/_ro/trn_rl_repo"):
    if os.path.isdir(_p) and _p not in sys.path:
        sys.path.insert(0, _p)

import concourse.bass as bass
from concourse import bacc
import concourse.tile as tile
from concourse import mybir
from concourse.masks import make_identity

F32 = mybir.dt.float32
F16 = mybir.dt.float16
BF16 = mybir.dt.bfloat16
AX = mybir.AxisListType
OP = mybir.AluOpType
AF = mybir.ActivationFunctionType

T = 2048
D = 1024
HPC = 4          # heads per core
DH = 64
NQB = T // 128   # 16
NSB = T // 512   # 4
NDC = D // 128   # 8
MARGIN = 40.0


def build_nc():
    nc = bacc.Bacc("TRN2", target_bir_lowering=False)
    xT = nc.dram_tensor("xT", [D, T], F16, kind="ExternalInput")
    wqk = nc.dram_tensor("wqk", [D, 128 * HPC], F16, kind="ExternalInput")
    wv = nc.dram_tensor("wv", [D, DH * HPC], F16, kind="ExternalInput")
    wo = nc.dram_tensor("wo", [2, 128, D], BF16, kind="ExternalInput")
    ones = nc.dram_tensor("ones", [1, T], F16, kind="ExternalInput")
    outT = nc.dram_tensor("outT", [D, T], BF16, kind="ExternalOutput")

    with tile.TileContext(nc) as tc:
        with (
            tc.tile_pool(name="const", bufs=1) as constp,
            tc.tile_pool(name="big", bufs=1) as bigp,
            tc.tile_pool(name="sb", bufs=3) as sbp,
            tc.tile_pool(name="psB", bufs=3, space="PSUM") as psB,
            tc.tile_pool(name="psZ", bufs=2, space="PSUM") as psZ,
        ):
            # ---- persistent SBUF ----
            xT_sb = bigp.tile([128, NDC, T], F16, tag="xT")
            wqk_sb = bigp.tile([128, NDC, 128 * HPC], F16, tag="wqk")
            wv_sb = bigp.tile([128, NDC, DH * HPC], F16, tag="wv")
            wo_sb = bigp.tile([128, 2, D], BF16, tag="wo")
            mskG_sb = constp.tile([128, NQB, 128], F32, tag="mskG")
            mskT_sb = constp.tile([128, 128], F32, tag="mskT")
            mskT2_sb = constp.tile([128, 2, 128], F32, tag="mskT2")
            ident = constp.tile([128, 128], F32, tag="ident")
            margin = constp.tile([NQB, 1], F32, tag="margin")
            q_sb = [bigp.tile([65, T], F16, tag=f"q{j}", name=f"q{j}") for j in range(HPC)]
            k_sb = [bigp.tile([65, T], F16, tag=f"k{j}", name=f"k{j}") for j in range(HPC)]
            v_sb = bigp.tile([128, NQB, HPC, DH + 1], BF16, tag="v")
            zT_sb = [bigp.tile([128, T], BF16, tag=f"zp{p}", name=f"zp{p}") for p in range(2)]

            xTr = xT.rearrange("(c p) t -> p c t", p=128)
            wqkr = wqk.rearrange("(c p) m -> p c m", p=128)
            nc.sync.dma_start(wqk_sb[:, :, 0:128], wqkr[:, :, 0:128])
            nc.sync.dma_start(xT_sb[:, 0:4, 0:512], xTr[:, 0:4, 0:512])
            nc.sync.dma_start(xT_sb[:, 4:8, 0:512], xTr[:, 4:8, 0:512])
            nc.sync.dma_start(wv_sb[:], wv.rearrange("(c p) m -> p c m", p=128))
            nc.sync.dma_start(k_sb[0][64:65, :], ones[:])
            nc.sync.dma_start(wqk_sb[:, :, 128:256], wqkr[:, :, 128:256])
            nc.sync.dma_start(xT_sb[:, 0:4, 512:1024], xTr[:, 0:4, 512:1024])
            nc.sync.dma_start(xT_sb[:, 4:8, 512:1024], xTr[:, 4:8, 512:1024])
            nc.sync.dma_start(wqk_sb[:, :, 256:512], wqkr[:, :, 256:512])
            for s in (2, 3):
                for c0 in (0, 4):
                    nc.sync.dma_start(
                        xT_sb[:, c0 : c0 + 4, s * 512 : (s + 1) * 512],
                        xTr[:, c0 : c0 + 4, s * 512 : (s + 1) * 512],
                    )
            for j in range(1, HPC):
                nc.sync.dma_start(k_sb[j][64:65, :], ones[:])
            nc.sync.dma_start(wo_sb[:], wo.rearrange("p k d -> k p d"))
            make_identity(nc, ident[:])
            nc.vector.memset(margin[:], -MARGIN)
            nc.gpsimd.memset(v_sb[:, :, :, DH : DH + 1], 1.0)
            # on-device masks (Pool is otherwise idle):
            # mskT[k, q] = 0 if k <= q else -1e9
            nc.gpsimd.memset(mskT_sb[:], 0.0)
            nc.gpsimd.affine_select(
                out=mskT_sb[:], in_=mskT_sb[:], compare_op=OP.is_ge,
                fill=-1e9, base=0, pattern=[[1, 128]], channel_multiplier=-1,
            )
            nc.gpsimd.tensor_copy(mskT2_sb[:, 0, :], mskT_sb[:])
            nc.gpsimd.tensor_copy(mskT2_sb[:, 1, :], mskT_sb[:])
            # mskG block qb: 0 if i*(qb+1) <= 128*qb + p else -1e9
            nc.gpsimd.memset(mskG_sb[:], 0.0)
            for qb in range(NQB):
                nc.gpsimd.affine_select(
                    out=mskG_sb[:, qb, :], in_=mskG_sb[:, qb, :],
                    compare_op=OP.is_ge, fill=-1e9, base=128 * qb,
                    pattern=[[-(qb + 1), 128]], channel_multiplier=1,
                )

            # ---- phase 1a: qT/kT slabs. width = number of 512-superblocks.
            # Copies drain in 512-col chunks so critical DVE ops (diag masks,
            # normalize) never queue behind a >1us copy; q goes via ACT only
            # in the front where ACT is idle.
            def qk_slab(j, s0, width=1, q_act=True):
                ps = psB.tile([128, 1024], F32, tag="mm")
                for h in range(width):
                    for c in range(NDC):
                        nc.tensor.matmul(
                            ps[:, h * 512 : (h + 1) * 512],
                            lhsT=(wqk_sb[:, c, j * 128 : (j + 1) * 128]),
                            rhs=(xT_sb[:, c, (s0 + h) * 512 : (s0 + h + 1) * 512]),
                            start=(c == 0),
                            stop=(c == NDC - 1),
                        )
                for h in range(width):
                    lo, hi = (s0 + h) * 512, (s0 + h + 1) * 512
                    if q_act:
                        nc.scalar.activation(
                            q_sb[j][0:64, lo:hi], ps[0:64, h * 512 : (h + 1) * 512],
                            AF.Copy,
                        )
                    else:
                        nc.vector.tensor_copy(
                            q_sb[j][0:64, lo:hi], ps[0:64, h * 512 : (h + 1) * 512]
                        )
                    if False and q_act and width == 2:
                        nc.scalar.activation(
                            k_sb[j][0:64, lo:hi], ps[64:128, h * 512 : (h + 1) * 512],
                            AF.Copy,
                        )
                    else:
                        nc.vector.tensor_copy(
                            k_sb[j][0:64, lo:hi], ps[64:128, h * 512 : (h + 1) * 512]
                        )

            # ---- phase 1b: v natural [t, 4*65] (ones col 64 for denominators)
            def v_chunk(tb):
                ps = psB.tile([128, 512], F32, tag="mm")
                for c in range(NDC):
                    nc.tensor.matmul(
                        ps[:, 0 : DH * HPC],
                        lhsT=(xT_sb[:, c, tb * 128 : (tb + 1) * 128]),
                        rhs=(wv_sb[:, c, :]),
                        start=(c == 0),
                        stop=(c == NDC - 1),
                    )
                nc.vector.tensor_copy(
                    v_sb[:, tb, :, 0:DH],
                    ps[:, 0 : DH * HPC].rearrange("p (j e) -> p j e", j=HPC),
                )

            # ---- phase 2A: stats — negated causal row max. Each 128-row
            # q-block qb samples its causal range at stride (qb+1), giving a
            # uniform 128 samples; 4 blocks share one PSUM tile, one boundary
            # mask add and one 3-D max-reduce. Margin 40 covers sampling.
            m_all = {}

            def stats_group(j, g, nq=4):
                if g == 0:
                    m_all[j] = sbp.tile(
                        [128, NQB], F32, tag=f"mall{j % 2}", name=f"mall{j}"
                    )
                ma = m_all[j]
                ps = psB.tile([128, 128 * nq], F32, tag="mm")
                for i in range(nq):
                    qb = nq * g + i
                    L = 128 * (qb + 1)
                    if qb == 0:
                        rhs = k_sb[j][0:64, 0:128]
                    else:
                        kr = k_sb[j][0:64, 0:L].rearrange(
                            "p (n st) -> p n st", st=qb + 1
                        )
                        rhs = kr[:, :, 0:1]
                    nc.tensor.matmul(
                        ps[:, i * 128 : (i + 1) * 128],
                        lhsT=(q_sb[j][0:64, qb * 128 : (qb + 1) * 128]),
                        rhs=rhs,
                        start=True,
                        stop=True,
                    )
                W = 128 * nq
                pv = ps[:, 0:W].rearrange("p (n w) -> p n w", w=128)
                gv = mskG_sb[:, nq * g : nq * g + nq, :]
                if g == 0:
                    # first group holds qb0 (full causal mask) + wide edges
                    nc.vector.tensor_tensor(
                        ps[:, 0:W], ps[:, 0:W],
                        gv.rearrange("p n w -> p (n w)"),
                        op=OP.add,
                    )
                else:
                    nc.vector.tensor_tensor(
                        pv[:, :, 96:128], pv[:, :, 96:128], gv[:, :, 96:128],
                        op=OP.add,
                    )
                nc.vector.tensor_reduce(
                    ma[:, nq * g : nq * g + nq], pv[:], axis=AX.X,
                    op=OP.max, negate=True,
                )

            def stats_fin(j, s=None):
                # transpose -m into row 64 of q' (as -max - MARGIN, fp16).
                # s=None: whole row via the idle SP queue (heads 1-3);
                # else one 512-col chunk via the ACT queue (head 0 warmup).
                lo, n = (0, NQB) if s is None else (4 * s, 4)
                pm = psB.tile([128, 512], F32, tag="mm")
                nc.tensor.transpose(
                    pm[0:n, 0:128], m_all[j][:, lo : lo + n], ident[:]
                )
                mT = sbp.tile([NQB, 128], F16, tag="mT", bufs=3)
                nc.scalar.activation(
                    mT[0:n, :], pm[0:n, 0:128], AF.Identity, bias=margin[0:n, :]
                )
                dst = q_sb[j][64:65, lo * 128 : (lo + n) * 128]
                if s is None:
                    nc.sync.dma_start(dst, mT[0:n, :])
                else:
                    nc.scalar.dma_start(dst, mT[0:n, :])

            # ---- phase 3: one [128d, 512t] out^T chunk -> o_sb; DMA per s
            o_sb = {}

            p3_alt = [0]

            def p3_pair(s, db0):
                if db0 == 0:
                    o_sb[s] = sbp.tile(
                        [128, NDC, 512], BF16, tag="osb", bufs=2, name=f"osb{s}"
                    )
                ops = psB.tile([128, 1024], F32, tag="mm")
                for i in range(2):
                    for p in range(2):
                        nc.tensor.matmul(
                            ops[:, i * 512 : (i + 1) * 512],
                            lhsT=(wo_sb[:, p, (db0 + i) * 128 : (db0 + i + 1) * 128]),
                            rhs=(zT_sb[p][:, s * 512 : (s + 1) * 512]),
                            start=(p == 0),
                            stop=(p == 1),
                        )
                dst = o_sb[s][:, db0 : db0 + 2, :].rearrange("p n w -> p (n w)")
                p3_alt[0] ^= 1
                if s == 2 or p3_alt[0]:
                    nc.scalar.activation(dst, ops[:], AF.Copy)
                else:
                    nc.vector.tensor_copy(dst, ops[:])

            def p3_out(s):
                nc.sync.dma_start(
                    outT.rearrange("(g p) t -> p g t", p=128)[
                        :, :, s * 512 : (s + 1) * 512
                    ],
                    o_sb[s][:],
                )

            # ---- phase 2B: the global S-pair pipeline ----------------------
            zps_t = {}
            pending = []
            norm_q = []

            def retire():
                u, pT = pending.pop(0)
                if norm_q:
                    norm_q.pop(0)()
                j, s = u["j"], u["s"]
                nkb = 4 * s + 4
                if (j, s) not in zps_t:
                    zps_t[(j, s)] = psZ.tile(
                        [65, 512], F32, tag="zz", name=f"zps{j}_{s}"
                    )
                zps = zps_t[(j, s)]
                for kb, w, off in u["z"]:
                    nc.tensor.matmul(
                        zps[:, 512 - w : 512],
                        lhsT=v_sb[:, kb, j, :],
                        rhs=pT[:, off : off + w],
                        start=(kb == 0),
                        stop=(kb == nkb - 1),
                        skip_group_check=True,
                    )
                if u["close"]:
                    def mknorm(j=j, s=s, zps=zps):
                        def donorm():
                            r1 = sbp.tile([1, 512], F32, tag="r1", bufs=3)
                            nc.vector.reciprocal(r1[:], zps[64:65, :])
                            rb = sbp.tile([64, 512], F32, tag="rb", bufs=3)
                            nc.gpsimd.partition_broadcast(rb[:], r1[:])
                            p, po = j // 2, 64 * (j % 2)
                            nc.vector.tensor_mul(
                                zT_sb[p][po : po + 64, s * 512 : (s + 1) * 512],
                                zps[0:64, :],
                                rb[:],
                            )
                        return donorm
                    norm_q.append(mknorm())
                    del zps_t[(j, s)]

            def step(u, il):
                j, s, a, b = u["j"], u["s"], u["a"], u["b"]
                wa = 512 if a < 4 * s else 512 - 128 * (a - 4 * s)
                wb = 512 if b < 4 * s else 512 - 128 * (b - 4 * s)
                sps = psB.tile([128, 1024], F32, tag="mm")
                nc.tensor.matmul(
                    sps[:, 0:wa],
                    lhsT=(k_sb[j][0:65, a * 128 : (a + 1) * 128]),
                    rhs=(q_sb[j][0:65, s * 512 + 512 - wa : (s + 1) * 512]),
                    start=True,
                    stop=True,
                )
                nc.tensor.matmul(
                    sps[:, wa : wa + wb],
                    lhsT=(k_sb[j][0:65, b * 128 : (b + 1) * 128]),
                    rhs=(q_sb[j][0:65, s * 512 + 512 - wb : (s + 1) * 512]),
                    start=True,
                    stop=True,
                )
                if a >= 4 * s:
                    # both kbs of a pair are diagonal together: one strided
                    # add covers both [*, 0:128] and [*, wa:wa+128]
                    dv = sps[:, 0 : 2 * wa].rearrange(
                        "p (n w) -> p n w", w=wa
                    )[:, :, 0:128]
                    nc.vector.tensor_tensor(dv, dv, mskT2_sb[:], op=OP.add)
                pT = sbp.tile([128, 1024], BF16, tag="pT", bufs=6)
                nc.scalar.activation(pT[:, 0 : wa + wb], sps[:, 0 : wa + wb], AF.Exp)
                if il is not None:
                    il()
                u["z"] = [(a, wa, 0), (b, wb, wa)]
                pending.append((u, pT))
                if len(pending) > 4:
                    retire()

            def head_units(j):
                us = []
                for s in range(NSB):
                    nkb = 4 * s + 4
                    for a in range(0, nkb, 2):
                        us.append(
                            {"j": j, "s": s, "a": a, "b": a + 1,
                             "close": a + 2 == nkb}
                        )
                return us

            # ---- emission schedule ----
            # front: head-0 projections (512-wide for latency) with head-0
            # stats + per-superblock row-max finalize woven in
            for s in (0, 1):
                qk_slab(0, s, width=1, q_act=True)
                stats_group(0, s)
                stats_fin(0, s=s)
                if s == 0:
                    v_chunk(0)
                    v_chunk(1)

            def vp(t0, t1):
                def fn():
                    v_chunk(t0)
                    v_chunk(t1)
                return fn

            def vq(t0):
                def fn():
                    for t in range(t0, t0 + 4):
                        v_chunk(t)
                return fn

            def qk1s(s):
                return lambda: qk_slab(1, s, width=1, q_act=False)

            def qs2(jn, h):
                return lambda: qk_slab(jn, 2 * h, width=2)

            def stg(jn, g):
                return lambda: stats_group(jn, g)

            def qk0s(s):
                return lambda: qk_slab(0, s, width=1, q_act=True)

            def stgf0(s):
                def fn():
                    stats_group(0, s)
                    stats_fin(0, s=s)
                return fn

            def sfin(jn):
                return lambda: stats_fin(jn)

            def p3p(s, db0):
                return lambda: p3_pair(s, db0)

            il_map = {
                0: [qk1s(0), qk0s(2), stgf0(2), vp(2, 3), None,
                    qk1s(1), qk1s(2), vq(4), qk0s(3), stgf0(3),
                    stg(1, 0), stg(1, 1), stg(1, 2), qk1s(3), vq(8),
                    stg(1, 3), sfin(1), vp(12, 13), vp(14, 15), None],
                1: [None, None, qs2(2, 0), qs2(2, 1), None, None, stg(2, 0),
                    None, stg(2, 1), None, stg(2, 2), None, stg(2, 3),
                    sfin(2), None, None, None, None, None, None],
                2: [None, None, qs2(3, 0), qs2(3, 1), None, None, stg(3, 0),
                    None, stg(3, 1), None, stg(3, 2), None, stg(3, 3),
                    sfin(3), None, None, None, None, None, None],
                3: [None, None, None, None, None, None, None,
                    p3p(0, 0), p3p(0, 2), p3p(0, 4), p3p(0, 6),
                    lambda: p3_out(0),
                    p3p(1, 0), p3p(1, 2), p3p(1, 4), p3p(1, 6),
                    lambda: p3_out(1),
                    p3p(2, 0), p3p(2, 2), p3p(2, 4)],
            }

            for j in range(HPC):
                il_list = il_map[j]
                for i, u in enumerate(head_units(j)):
                    step(u, il_list[i])
            # ---- tail: retire the last units with a split normalize so
            # phase 3's last superblock overlaps the final softmax columns.
            outv = outT.rearrange("(g p) t -> p g t", p=128)
            nc.sync.dma_start(
                outv[:, 0:4, 1024:1536], o_sb[2][:, 0:4, :]
            )
            while len(pending) > 2:
                retire()
            while norm_q:
                norm_q.pop(0)()
            p3_pair(2, 6)
            nc.sync.dma_start(
                outv[:, 4:8, 1024:1536], o_sb[2][:, 4:8, :]
            )
            pending[1][0]["close"] = False
            retire()  # z for kbs 12,13 -> cols [0:256] of (3,3) complete
            zps33 = zps_t[(3, 3)]
            r1a = sbp.tile([1, 512], F32, tag="r1", bufs=3, name="r1a")
            nc.vector.reciprocal(r1a[0:1, 0:256], zps33[64:65, 0:256])
            rba = sbp.tile([64, 512], F32, tag="rb", bufs=3, name="rba")
            nc.gpsimd.partition_broadcast(rba[0:64, 0:256], r1a[0:1, 0:256])
            nc.vector.tensor_mul(
                zT_sb[1][64:128, 1536:1792], zps33[0:64, 0:256], rba[0:64, 0:256]
            )
            retire()  # z for kbs 14,15 -> cols [256:512] complete
            r1b = sbp.tile([1, 512], F32, tag="r1", bufs=3, name="r1b")
            nc.vector.reciprocal(r1b[0:1, 0:256], zps33[64:65, 256:512])
            rbb = sbp.tile([64, 512], F32, tag="rb", bufs=3, name="rbb")
            nc.gpsimd.partition_broadcast(rbb[0:64, 0:256], r1b[0:1, 0:256])
            nc.vector.tensor_mul(
                zT_sb[1][64:128, 1792:2048], zps33[0:64, 256:512], rbb[0:64, 0:256]
            )
            del zps_t[(3, 3)]

            o_sb[3] = sbp.tile(
                [128, NDC, 512], BF16, tag="osb", bufs=2, name="osb3"
            )

            def p3_pair_h(db0, h):
                ops = psB.tile([128, 1024], F32, tag="mm")
                for i in range(2):
                    for p in range(2):
                        nc.tensor.matmul(
                            ops[:, i * 512 + h * 256 : i * 512 + (h + 1) * 256],
                            lhsT=(wo_sb[:, p, (db0 + i) * 128 : (db0 + i + 1) * 128]),
                            rhs=(zT_sb[p][:, 1536 + h * 256 : 1792 + h * 256]),
                            start=(p == 0),
                            stop=(p == 1),
                        )
                dst = o_sb[3][:, db0 : db0 + 2, h * 256 : (h + 1) * 256]
                srcv = ops[:, 0:1024].rearrange("p (i w) -> p i w", w=512)[
                    :, :, h * 256 : (h + 1) * 256
                ]
                if db0 % 4 == 0:
                    nc.scalar.activation(dst, srcv, AF.Copy)
                else:
                    nc.vector.tensor_copy(dst, srcv)

            for h in range(2):
                for db0 in (0, 2):
                    p3_pair_h(db0, h)
                nc.sync.dma_start(
                    outv[:, 0:4, 1536 + h * 256 : 1792 + h * 256],
                    o_sb[3][:, 0:4, h * 256 : (h + 1) * 256],
                )
                for db0 in (4, 6):
                    p3_pair_h(db0, h)
                nc.sync.dma_start(
                    outv[:, 4:8, 1536 + h * 256 : 1792 + h * 256],
                    o_sb[3][:, 4:8, h * 256 : (h + 1) * 256],
                )
    nc.compile()
    return nc


_NC = None


def _get_nc():
    global _NC
    if _NC is None:
        _NC = build_nc()
    return _NC


def _make_in_maps(inputs):
    x = np.ascontiguousarray(np.asarray(inputs["residual_stream"], dtype=np.float32))
    W_Q = np.asarray(inputs["W_Q"], dtype=np.float32)
    W_K = np.asarray(inputs["W_K"], dtype=np.float32)
    W_V = np.asarray(inputs["W_V"], dtype=np.float32)
    W_O = np.asarray(inputs["W_output"], dtype=np.float32)

    try:
        import ml_dtypes
        bf16 = ml_dtypes.bfloat16
    except ImportError:
        bf16 = np.float32

    qi = np.arange(128)
    mskS = np.where(qi[None, :] <= qi[:, None], 0.0, -1e9).astype(np.float32)  # [q,k]
    mskT = np.ascontiguousarray(mskS.T)  # [k,q]
    si = np.arange(128)
    mskG = np.zeros((128, NQB, 128), np.float32)
    for qb in range(NQB):
        mskG[:, qb, :] = np.where(
            si[None, :] * (qb + 1) <= 128 * qb + qi[:, None], 0.0, -1e9
        )
    mskG = np.ascontiguousarray(mskG.reshape(128, NQB * 128))

    in_maps = []
    for c in range(8):
        b, hg = c // 4, c % 4
        hs = [4 * hg + j for j in range(HPC)]
        wqk_h = np.concatenate(
            [np.concatenate([W_Q[h] / 8.0, W_K[h]], axis=1) for h in hs], axis=1
        )  # [1024, 512]
        wv_h = np.concatenate([W_V[h] for h in hs], axis=1)  # [1024, 256]
        wo_h = np.stack(
            [np.concatenate([W_O[hs[2 * p]], W_O[hs[2 * p + 1]]], axis=0)
             for p in range(2)]
        )  # [2, 128, 1024]
        in_maps.append(
            {
                "xT": np.ascontiguousarray(x[b].T).astype(np.float16),
                "wqk": np.ascontiguousarray(wqk_h).astype(np.float16),
                "wv": np.ascontiguousarray(wv_h).astype(np.float16),
                "wo": np.ascontiguousarray(wo_h).astype(bf16),
                "ones": np.ones((1, T), np.float16),
            }
        )
    return in_maps


def _postprocess(results, inputs):
    W_O = np.asarray(inputs["W_output"], dtype=np.float32)
    b_V = np.asarray(inputs["b_V"], dtype=np.float32)
    b_out = np.asarray(inputs["b_output"], dtype=np.float32)
    out = np.zeros((2, T, D), dtype=np.float32)
    for c in range(8):
        out[c // 4] += results[c]["outT"].T.astype(np.float32)
    # z = P @ v + b_V (P rows sum to 1) -> fold b_V through W_O on the host
    const = np.einsum("he,hed->d", b_V, W_O) + b_out
    out += const[None, None, :]
    return out


def kernel(**inputs):
    from concourse.bass_utils import run_bass_kernel_spmd

    nc = _get_nc()
    res = run_bass_kernel_spmd(nc, _make_in_maps(inputs), core_ids=list(range(8)))
    return _postprocess(res.results, inputs)


def kernel_traced(**inputs):
    """Returns (output, exec_time_ns or None) using a traced run."""
    from concourse.bass_utils import run_bass_kernel_spmd

    nc = _get_nc()
    res = run_bass_kernel_spmd(
        nc, _make_in_maps(inputs), core_ids=list(range(8)), trace=True
    )
    return _postprocess(res.results, inputs), res.exec_time_ns


# revision 10
# speedup vs baseline: 1.1093x; 1.0153x over previous
"""Multi-head causal attention on 8 TRN2 NeuronCores.

Sharding: data-parallel over batch (2) x tensor-parallel over heads (4 groups
of 4 heads) = 8 cores. Each core emits a partial output projection in
transposed layout; the host sums the 4 partials per batch and folds the
(zero) biases.

Precision: x/W_QK/q/k fp16 (tf32-class mantissa; halves input DMA, full-rate
matmuls at any width), V/P/z/W_O/out bf16, all accumulation f32 in PSUM.
Softmax max-subtraction uses a sampled causal row max (each 128-row q-block
samples its range at stride (qb+1) = 128 samples) with a 40-unit safety
margin; the un-normalized exp weights stay finite in bf16/f32 by
construction (measured emulation rel err 4.5e-3 vs the noisy reference).

Engine plan (GPSIMD cannot touch PSUM on this HW):
  PE   S^T pairs, z accumulation, projections, stats samples, transposes
  ACT  exp over paired S^T blocks (2-bank PSUM tiles), q/k PSUM drains,
       row-max formatting + its DMA queue, half of phase-3 drains
  DVE  v drain, fused diagonal-pair causal masks, stats boundary masks +
       grouped 3-D max-reduces, denominator reciprocal, z normalize
  Pool on-device mask generation (affine_select), denominator broadcast
  SP   batched fp16/bf16 input DMAs, per-superblock output DMAs

Schedule: one global software pipeline over every S-pair unit of every
head. A step emits [2 S^T matmuls, fused diag mask, exp], then one
interleave closure (v chunk / next head's projection slab / next head's
stats group / phase-3 chunk), then retires the z matmuls of the unit four
steps back; normalize (reciprocal+broadcast+multiply) is deferred one
further step so it lands in the mask-free region of the next superblock.
The front consumes x slabs as they stream; the tail splits the last
superblock's normalize and phase 3 in halves to overlap the final DMAs.

CoreSim cost model: 148.2us vs 222.3us baseline (1.50x); rel err 4.5e-3.
"""

import os
import sys

import numpy as np

for _p in ("/opt/trn_rl_repo", "/root/.axon_site/_ro/trn_rl_repo"):
    if os.path.isdir(_p) and _p not in sys.path:
        sys.path.insert(0, _p)

import concourse.bass as bass
from concourse import bacc
import concourse.tile as tile
from concourse import mybir
from concourse.masks import make_identity

F32 = mybir.dt.float32
F16 = mybir.dt.float16
BF16 = mybir.dt.bfloat16
AX = mybir.AxisListType
OP = mybir.AluOpType
AF = mybir.ActivationFunctionType

T = 2048
D = 1024
HPC = 4          # heads per core
DH = 64
NQB = T // 128   # 16
NSB = T // 512   # 4
NDC = D // 128   # 8
MARGIN = 40.0


def build_nc():
    nc = bacc.Bacc("TRN2", target_bir_lowering=False)
    xT = nc.dram_tensor("xT", [D, T], F16, kind="ExternalInput")
    wqk = nc.dram_tensor("wqk", [D, 128 * HPC], F16, kind="ExternalInput")
    wv = nc.dram_tensor("wv", [D, DH * HPC], F16, kind="ExternalInput")
    wo = nc.dram_tensor("wo", [2, 128, D], BF16, kind="ExternalInput")
    ones = nc.dram_tensor("ones", [1, T], F16, kind="ExternalInput")
    outT = nc.dram_tensor("outT", [D, T], BF16, kind="ExternalOutput")

    with tile.TileContext(nc) as tc:
        with (
            tc.tile_pool(name="const", bufs=1) as constp,
            tc.tile_pool(name="big", bufs=1) as bigp,
            tc.tile_pool(name="sb", bufs=3) as sbp,
            tc.tile_pool(name="psB", bufs=3, space="PSUM") as psB,
            tc.tile_pool(name="psZ", bufs=2, space="PSUM") as psZ,
        ):
            # ---- persistent SBUF ----
            xT_sb = bigp.tile([128, NDC, T], F16, tag="xT")
            wqk_sb = bigp.tile([128, NDC, 128 * HPC], F16, tag="wqk")
            wv_sb = bigp.tile([128, NDC, DH * HPC], F16, tag="wv")
            wo_sb = bigp.tile([128, 2, D], BF16, tag="wo")
            mskG_sb = constp.tile([128, NQB, 128], F32, tag="mskG")
            mskT_sb = constp.tile([128, 128], F32, tag="mskT")
            mskT2_sb = constp.tile([128, 2, 128], F32, tag="mskT2")
            ident = constp.tile([128, 128], F32, tag="ident")
            margin = constp.tile([NQB, 1], F32, tag="margin")
            q_sb = [bigp.tile([65, T], F16, tag=f"q{j}", name=f"q{j}") for j in range(HPC)]
            k_sb = [bigp.tile([65, T], F16, tag=f"k{j}", name=f"k{j}") for j in range(HPC)]
            v_sb = bigp.tile([128, NQB, HPC, DH + 1], BF16, tag="v")
            zT_sb = [bigp.tile([128, T], BF16, tag=f"zp{p}", name=f"zp{p}") for p in range(2)]

            xTr = xT.rearrange("(c p) t -> p c t", p=128)
            wqkr = wqk.rearrange("(c p) m -> p c m", p=128)
            nc.sync.dma_start(wqk_sb[:, :, 0:128], wqkr[:, :, 0:128])
            nc.sync.dma_start(xT_sb[:, 0:4, 0:512], xTr[:, 0:4, 0:512])
            nc.sync.dma_start(xT_sb[:, 4:8, 0:512], xTr[:, 4:8, 0:512])
            nc.sync.dma_start(wv_sb[:], wv.rearrange("(c p) m -> p c m", p=128))
            nc.sync.dma_start(k_sb[0][64:65, :], ones[:])
            nc.sync.dma_start(wqk_sb[:, :, 128:256], wqkr[:, :, 128:256])
            nc.sync.dma_start(xT_sb[:, 0:4, 512:1024], xTr[:, 0:4, 512:1024])
            nc.sync.dma_start(xT_sb[:, 4:8, 512:1024], xTr[:, 4:8, 512:1024])
            nc.sync.dma_start(wqk_sb[:, :, 256:512], wqkr[:, :, 256:512])
            for s in (2, 3):
                for c0 in (0, 4):
                    nc.sync.dma_start(
                        xT_sb[:, c0 : c0 + 4, s * 512 : (s + 1) * 512],
                        xTr[:, c0 : c0 + 4, s * 512 : (s + 1) * 512],
                    )
            for j in range(1, HPC):
                nc.sync.dma_start(k_sb[j][64:65, :], ones[:])
            nc.sync.dma_start(wo_sb[:], wo.rearrange("p k d -> k p d"))
            make_identity(nc, ident[:])
            nc.vector.memset(margin[:], -MARGIN)
            nc.gpsimd.memset(v_sb[:, :, :, DH : DH + 1], 1.0)
            # on-device masks (Pool is otherwise idle):
            # mskT[k, q] = 0 if k <= q else -1e9
            nc.gpsimd.memset(mskT_sb[:], 0.0)
            nc.gpsimd.affine_select(
                out=mskT_sb[:], in_=mskT_sb[:], compare_op=OP.is_ge,
                fill=-1e9, base=0, pattern=[[1, 128]], channel_multiplier=-1,
            )
            nc.gpsimd.tensor_copy(mskT2_sb[:, 0, :], mskT_sb[:])
            nc.gpsimd.tensor_copy(mskT2_sb[:, 1, :], mskT_sb[:])
            # mskG block qb: 0 if i*(qb+1) <= 128*qb + p else -1e9
            nc.gpsimd.memset(mskG_sb[:], 0.0)
            for qb in range(NQB):
                nc.gpsimd.affine_select(
                    out=mskG_sb[:, qb, :], in_=mskG_sb[:, qb, :],
                    compare_op=OP.is_ge, fill=-1e9, base=128 * qb,
                    pattern=[[-(qb + 1), 128]], channel_multiplier=1,
                )

            # ---- phase 1a: qT/kT slabs. width = number of 512-superblocks.
            # Copies drain in 512-col chunks so critical DVE ops (diag masks,
            # normalize) never queue behind a >1us copy; q goes via ACT only
            # in the front where ACT is idle.
            def qk_slab(j, s0, width=1, q_act=True):
                ps = psB.tile([128, 1024], F32, tag="mm")
                for h in range(width):
                    for c in range(NDC):
                        nc.tensor.matmul(
                            ps[:, h * 512 : (h + 1) * 512],
                            lhsT=(wqk_sb[:, c, j * 128 : (j + 1) * 128]),
                            rhs=(xT_sb[:, c, (s0 + h) * 512 : (s0 + h + 1) * 512]),
                            start=(c == 0),
                            stop=(c == NDC - 1),
                        )
                for h in range(width):
                    lo, hi = (s0 + h) * 512, (s0 + h + 1) * 512
                    if q_act:
                        nc.scalar.activation(
                            q_sb[j][0:64, lo:hi], ps[0:64, h * 512 : (h + 1) * 512],
                            AF.Copy,
                        )
                    else:
                        nc.vector.tensor_copy(
                            q_sb[j][0:64, lo:hi], ps[0:64, h * 512 : (h + 1) * 512]
                        )
                    if False and q_act and width == 2:
                        nc.scalar.activation(
                            k_sb[j][0:64, lo:hi], ps[64:128, h * 512 : (h + 1) * 512],
                            AF.Copy,
                        )
                    else:
                        nc.vector.tensor_copy(
                            k_sb[j][0:64, lo:hi], ps[64:128, h * 512 : (h + 1) * 512]
                        )

            # ---- phase 1b: v natural [t, 4*65] (ones col 64 for denominators)
            def v_chunk(tb):
                ps = psB.tile([128, 512], F32, tag="mm")
                for c in range(NDC):
                    nc.tensor.matmul(
                        ps[:, 0 : DH * HPC],
                        lhsT=(xT_sb[:, c, tb * 128 : (tb + 1) * 128]),
                        rhs=(wv_sb[:, c, :]),
                        start=(c == 0),
                        stop=(c == NDC - 1),
                    )
                nc.vector.tensor_copy(
                    v_sb[:, tb, :, 0:DH],
                    ps[:, 0 : DH * HPC].rearrange("p (j e) -> p j e", j=HPC),
                )

            # ---- phase 2A: stats — negated causal row max. Each 128-row
            # q-block qb samples its causal range at stride (qb+1), giving a
            # uniform 128 samples; 4 blocks share one PSUM tile, one boundary
            # mask add and one 3-D max-reduce. Margin 40 covers sampling.
            m_all = {}

            def stats_group(j, g, nq=4):
                if g == 0:
                    m_all[j] = sbp.tile(
                        [128, NQB], F32, tag=f"mall{j % 2}", name=f"mall{j}"
                    )
                ma = m_all[j]
                ps = psB.tile([128, 128 * nq], F32, tag="mm")
                for i in range(nq):
                    qb = nq * g + i
                    L = 128 * (qb + 1)
                    if qb == 0:
                        rhs = k_sb[j][0:64, 0:128]
                    else:
                        kr = k_sb[j][0:64, 0:L].rearrange(
                            "p (n st) -> p n st", st=qb + 1
                        )
                        rhs = kr[:, :, 0:1]
                    nc.tensor.matmul(
                        ps[:, i * 128 : (i + 1) * 128],
                        lhsT=(q_sb[j][0:64, qb * 128 : (qb + 1) * 128]),
                        rhs=rhs,
                        start=True,
                        stop=True,
                    )
                W = 128 * nq
                pv = ps[:, 0:W].rearrange("p (n w) -> p n w", w=128)
                gv = mskG_sb[:, nq * g : nq * g + nq, :]
                if g == 0:
                    # first group holds qb0 (full causal mask) + wide edges
                    nc.vector.tensor_tensor(
                        ps[:, 0:W], ps[:, 0:W],
                        gv.rearrange("p n w -> p (n w)"),
                        op=OP.add,
                    )
                else:
                    nc.vector.tensor_tensor(
                        pv[:, :, 96:128], pv[:, :, 96:128], gv[:, :, 96:128],
                        op=OP.add,
                    )
                nc.vector.tensor_reduce(
                    ma[:, nq * g : nq * g + nq], pv[:], axis=AX.X,
                    op=OP.max, negate=True,
                )

            def stats_fin(j, s=None, half=None):
                # transpose -m into row 64 of q' (as -max - MARGIN, fp16).
                # s=None: whole row via the idle SP queue (heads 1-3);
                # else one 512-col chunk via the ACT queue (head 0 warmup).
                if half is not None:
                    lo, n = 8 * half, 8
                else:
                    lo, n = (0, NQB) if s is None else (4 * s, 4)
                pm = psB.tile([128, 512], F32, tag="mm")
                nc.tensor.transpose(
                    pm[0:n, 0:128], m_all[j][:, lo : lo + n], ident[:]
                )
                mT = sbp.tile([NQB, 128], F16, tag="mT", bufs=3)
                nc.scalar.activation(
                    mT[0:n, :], pm[0:n, 0:128], AF.Identity, bias=margin[0:n, :]
                )
                dst = q_sb[j][64:65, lo * 128 : (lo + n) * 128]
                if s is None:
                    nc.sync.dma_start(dst, mT[0:n, :])
                else:
                    nc.scalar.dma_start(dst, mT[0:n, :])

            # ---- phase 3: one [128d, 512t] out^T chunk -> o_sb; DMA per s
            o_sb = {}

            p3_alt = [0]

            def p3_pair(s, db0):
                if db0 == 0:
                    o_sb[s] = sbp.tile(
                        [128, NDC, 512], BF16, tag="osb", bufs=2, name=f"osb{s}"
                    )
                ops = psB.tile([128, 1024], F32, tag="mm")
                for i in range(2):
                    for p in range(2):
                        nc.tensor.matmul(
                            ops[:, i * 512 : (i + 1) * 512],
                            lhsT=(wo_sb[:, p, (db0 + i) * 128 : (db0 + i + 1) * 128]),
                            rhs=(zT_sb[p][:, s * 512 : (s + 1) * 512]),
                            start=(p == 0),
                            stop=(p == 1),
                        )
                dst = o_sb[s][:, db0 : db0 + 2, :].rearrange("p n w -> p (n w)")
                p3_alt[0] ^= 1
                if s == 2 or p3_alt[0]:
                    nc.scalar.activation(dst, ops[:], AF.Copy)
                else:
                    nc.vector.tensor_copy(dst, ops[:])

            def p3_out(s):
                nc.sync.dma_start(
                    outT.rearrange("(g p) t -> p g t", p=128)[
                        :, :, s * 512 : (s + 1) * 512
                    ],
                    o_sb[s][:],
                )

            # ---- phase 2B: the global S-pair pipeline ----------------------
            zps_t = {}
            pending = []
            norm_q = []

            def retire():
                u, pT = pending.pop(0)
                if norm_q:
                    norm_q.pop(0)()
                j, s = u["j"], u["s"]
                nkb = 4 * s + 4
                if (j, s) not in zps_t:
                    zps_t[(j, s)] = psZ.tile(
                        [65, 512], F32, tag="zz", name=f"zps{j}_{s}"
                    )
                zps = zps_t[(j, s)]
                for kb, w, off in u["z"]:
                    nc.tensor.matmul(
                        zps[:, 512 - w : 512],
                        lhsT=v_sb[:, kb, j, :],
                        rhs=pT[:, off : off + w],
                        start=(kb == 0),
                        stop=(kb == nkb - 1),
                        skip_group_check=True,
                    )
                if u["close"]:
                    def mknorm(j=j, s=s, zps=zps):
                        def donorm():
                            r1 = sbp.tile([1, 512], F32, tag="r1", bufs=3)
                            nc.vector.reciprocal(r1[:], zps[64:65, :])
                            rb = sbp.tile([64, 512], F32, tag="rb", bufs=3)
                            nc.gpsimd.partition_broadcast(rb[:], r1[:])
                            p, po = j // 2, 64 * (j % 2)
                            nc.vector.tensor_mul(
                                zT_sb[p][po : po + 64, s * 512 : (s + 1) * 512],
                                zps[0:64, :],
                                rb[:],
                            )
                        return donorm
                    norm_q.append(mknorm())
                    del zps_t[(j, s)]

            def step(u, il):
                j, s, a, b = u["j"], u["s"], u["a"], u["b"]
                wa = 512 if a < 4 * s else 512 - 128 * (a - 4 * s)
                wb = 512 if b < 4 * s else 512 - 128 * (b - 4 * s)
                sps = psB.tile([128, 1024], F32, tag="mm")
                nc.tensor.matmul(
                    sps[:, 0:wa],
                    lhsT=(k_sb[j][0:65, a * 128 : (a + 1) * 128]),
                    rhs=(q_sb[j][0:65, s * 512 + 512 - wa : (s + 1) * 512]),
                    start=True,
                    stop=True,
                )
                nc.tensor.matmul(
                    sps[:, wa : wa + wb],
                    lhsT=(k_sb[j][0:65, b * 128 : (b + 1) * 128]),
                    rhs=(q_sb[j][0:65, s * 512 + 512 - wb : (s + 1) * 512]),
                    start=True,
                    stop=True,
                )
                if a >= 4 * s:
                    # both kbs of a pair are diagonal together: one strided
                    # add covers both [*, 0:128] and [*, wa:wa+128]
                    dv = sps[:, 0 : 2 * wa].rearrange(
                        "p (n w) -> p n w", w=wa
                    )[:, :, 0:128]
                    nc.vector.tensor_tensor(dv, dv, mskT2_sb[:], op=OP.add)
                pT = sbp.tile([128, 1024], BF16, tag="pT", bufs=6)
                nc.scalar.activation(pT[:, 0 : wa + wb], sps[:, 0 : wa + wb], AF.Exp)
                if il is not None:
                    il()
                u["z"] = [(a, wa, 0), (b, wb, wa)]
                pending.append((u, pT))
                if len(pending) > 4:
                    retire()

            def head_units(j):
                us = []
                for s in range(NSB):
                    nkb = 4 * s + 4
                    for a in range(0, nkb, 2):
                        us.append(
                            {"j": j, "s": s, "a": a, "b": a + 1,
                             "close": a + 2 == nkb}
                        )
                return us

            # ---- emission schedule ----
            # front: head-0 projections (512-wide for latency) with head-0
            # stats + per-superblock row-max finalize woven in
            for s in (0, 1):
                qk_slab(0, s, width=1, q_act=True)
                stats_group(0, s)
                stats_fin(0, s=s)
                if s == 0:
                    v_chunk(0)
                    v_chunk(1)

            def vp(t0, t1):
                def fn():
                    v_chunk(t0)
                    v_chunk(t1)
                return fn

            def vq(t0):
                def fn():
                    for t in range(t0, t0 + 4):
                        v_chunk(t)
                return fn

            def qk1s(s):
                return lambda: qk_slab(1, s, width=1, q_act=False)

            def qs2(jn, h):
                return lambda: qk_slab(jn, 2 * h, width=2)

            def stg(jn, g):
                return lambda: stats_group(jn, g)

            def qk0s(s):
                return lambda: qk_slab(0, s, width=1, q_act=True)

            def stgf0(s):
                def fn():
                    stats_group(0, s)
                    stats_fin(0, s=s)
                return fn

            def sfin(jn):
                return lambda: stats_fin(jn)

            def sfh(jn, h):
                return lambda: stats_fin(jn, half=h)

            def p3p(s, db0):
                return lambda: p3_pair(s, db0)

            il_map = {
                0: [qk1s(0), qk0s(2), stgf0(2), vp(2, 3), None,
                    qk1s(1), qk1s(2), vq(4), qk0s(3), stgf0(3),
                    stg(1, 0), stg(1, 1), stg(1, 2), qk1s(3), vq(8),
                    stg(1, 3), sfin(1), vp(12, 13), vp(14, 15), None],
                1: [None, None, qs2(2, 0), qs2(2, 1), None, None, stg(2, 0),
                    None, stg(2, 1), None, stg(2, 2), None, stg(2, 3),
                    sfin(2), None, None, None, None, None, None],
                2: [None, None, qs2(3, 0), qs2(3, 1), None, None, stg(3, 0),
                    None, stg(3, 1), None, stg(3, 2), None, stg(3, 3),
                    sfin(3), None, None, None, None, None, None],
                3: [None, None, None, None, None, None, None,
                    p3p(0, 0), p3p(0, 2), p3p(0, 4), p3p(0, 6),
                    lambda: p3_out(0),
                    p3p(1, 0), p3p(1, 2), p3p(1, 4), p3p(1, 6),
                    lambda: p3_out(1),
                    p3p(2, 0), p3p(2, 2), p3p(2, 4)],
            }

            for j in range(HPC):
                il_list = il_map[j]
                for i, u in enumerate(head_units(j)):
                    step(u, il_list[i])
            # ---- tail: retire the last units with a split normalize so
            # phase 3's last superblock overlaps the final softmax columns.
            outv = outT.rearrange("(g p) t -> p g t", p=128)
            nc.sync.dma_start(
                outv[:, 0:4, 1024:1536], o_sb[2][:, 0:4, :]
            )
            while len(pending) > 2:
                retire()
            while norm_q:
                norm_q.pop(0)()
            pending[1][0]["close"] = False
            retire()  # z for kbs 12,13 -> cols [0:256] of (3,3) complete
            zps33 = zps_t[(3, 3)]
            r1a = sbp.tile([1, 512], F32, tag="r1", bufs=3, name="r1a")
            nc.vector.reciprocal(r1a[0:1, 0:256], zps33[64:65, 0:256])
            rba = sbp.tile([64, 512], F32, tag="rb", bufs=3, name="rba")
            nc.gpsimd.partition_broadcast(rba[0:64, 0:256], r1a[0:1, 0:256])
            nc.vector.tensor_mul(
                zT_sb[1][64:128, 1536:1792], zps33[0:64, 0:256], rba[0:64, 0:256]
            )
            # PE keeps busy on the last s2 projection chunk while the first
            # normalize half and the final exp drain
            p3_pair(2, 6)
            nc.sync.dma_start(
                outv[:, 4:8, 1024:1536], o_sb[2][:, 4:8, :]
            )
            retire()  # z for kbs 14,15 -> cols [256:512] complete
            r1b = sbp.tile([1, 512], F32, tag="r1", bufs=3, name="r1b")
            nc.vector.reciprocal(r1b[0:1, 0:256], zps33[64:65, 256:512])
            rbb = sbp.tile([64, 512], F32, tag="rb", bufs=3, name="rbb")
            nc.gpsimd.partition_broadcast(rbb[0:64, 0:256], r1b[0:1, 0:256])
            nc.vector.tensor_mul(
                zT_sb[1][64:128, 1792:2048], zps33[0:64, 256:512], rbb[0:64, 0:256]
            )
            del zps_t[(3, 3)]

            o_sb[3] = sbp.tile(
                [128, NDC, 512], BF16, tag="osb", bufs=2, name="osb3"
            )

            def p3_pair_h(db0, h):
                ops = psB.tile([128, 1024], F32, tag="mm")
                for i in range(2):
                    for p in range(2):
                        nc.tensor.matmul(
                            ops[:, i * 512 + h * 256 : i * 512 + (h + 1) * 256],
                            lhsT=(wo_sb[:, p, (db0 + i) * 128 : (db0 + i + 1) * 128]),
                            rhs=(zT_sb[p][:, 1536 + h * 256 : 1792 + h * 256]),
                            start=(p == 0),
                            stop=(p == 1),
                        )
                dst = o_sb[3][:, db0 : db0 + 2, h * 256 : (h + 1) * 256]
                srcv = ops[:, 0:1024].rearrange("p (i w) -> p i w", w=512)[
                    :, :, h * 256 : (h + 1) * 256
                ]
                if db0 % 4 == 0:
                    nc.scalar.activation(dst, srcv, AF.Copy)
                else:
                    nc.vector.tensor_copy(dst, srcv)

            for h in range(2):
                for db0 in (0, 2):
                    p3_pair_h(db0, h)
                nc.sync.dma_start(
                    outv[:, 0:4, 1536 + h * 256 : 1792 + h * 256],
                    o_sb[3][:, 0:4, h * 256 : (h + 1) * 256],
                )
                for db0 in (4, 6):
                    p3_pair_h(db0, h)
                nc.sync.dma_start(
                    outv[:, 4:8, 1536 + h * 256 : 1792 + h * 256],
                    o_sb[3][:, 4:8, h * 256 : (h + 1) * 256],
                )
    nc.compile()
    return nc


_NC = None


def _get_nc():
    global _NC
    if _NC is None:
        _NC = build_nc()
    return _NC


def _make_in_maps(inputs):
    x = np.ascontiguousarray(np.asarray(inputs["residual_stream"], dtype=np.float32))
    W_Q = np.asarray(inputs["W_Q"], dtype=np.float32)
    W_K = np.asarray(inputs["W_K"], dtype=np.float32)
    W_V = np.asarray(inputs["W_V"], dtype=np.float32)
    W_O = np.asarray(inputs["W_output"], dtype=np.float32)

    try:
        import ml_dtypes
        bf16 = ml_dtypes.bfloat16
    except ImportError:
        bf16 = np.float32

    qi = np.arange(128)
    mskS = np.where(qi[None, :] <= qi[:, None], 0.0, -1e9).astype(np.float32)  # [q,k]
    mskT = np.ascontiguousarray(mskS.T)  # [k,q]
    si = np.arange(128)
    mskG = np.zeros((128, NQB, 128), np.float32)
    for qb in range(NQB):
        mskG[:, qb, :] = np.where(
            si[None, :] * (qb + 1) <= 128 * qb + qi[:, None], 0.0, -1e9
        )
    mskG = np.ascontiguousarray(mskG.reshape(128, NQB * 128))

    in_maps = []
    for c in range(8):
        b, hg = c // 4, c % 4
        hs = [4 * hg + j for j in range(HPC)]
        wqk_h = np.concatenate(
            [np.concatenate([W_Q[h] / 8.0, W_K[h]], axis=1) for h in hs], axis=1
        )  # [1024, 512]
        wv_h = np.concatenate([W_V[h] for h in hs], axis=1)  # [1024, 256]
        wo_h = np.stack(
            [np.concatenate([W_O[hs[2 * p]], W_O[hs[2 * p + 1]]], axis=0)
             for p in range(2)]
        )  # [2, 128, 1024]
        in_maps.append(
            {
                "xT": np.ascontiguousarray(x[b].T).astype(np.float16),
                "wqk": np.ascontiguousarray(wqk_h).astype(np.float16),
                "wv": np.ascontiguousarray(wv_h).astype(np.float16),
                "wo": np.ascontiguousarray(wo_h).astype(bf16),
                "ones": np.ones((1, T), np.float16),
            }
        )
    return in_maps


def _postprocess(results, inputs):
    W_O = np.asarray(inputs["W_output"], dtype=np.float32)
    b_V = np.asarray(inputs["b_V"], dtype=np.float32)
    b_out = np.asarray(inputs["b_output"], dtype=np.float32)
    out = np.zeros((2, T, D), dtype=np.float32)
    for c in range(8):
        out[c // 4] += results[c]["outT"].T.astype(np.float32)
    # z = P @ v + b_V (P rows sum to 1) -> fold b_V through W_O on the host
    const = np.einsum("he,hed->d", b_V, W_O) + b_out
    out += const[None, None, :]
    return out


def kernel(**inputs):
    from concourse.bass_utils import run_bass_kernel_spmd

    nc = _get_nc()
    res = run_bass_kernel_spmd(nc, _make_in_maps(inputs), core_ids=list(range(8)))
    return _postprocess(res.results, inputs)


def kernel_traced(**inputs):
    """Returns (output, exec_time_ns or None) using a traced run."""
    from concourse.bass_utils import run_bass_kernel_spmd

    nc = _get_nc()
    res = run_bass_kernel_spmd(
        nc, _make_in_maps(inputs), core_ids=list(range(8)), trace=True
    )
    return _postprocess(res.results, inputs), res.exec_time_ns


# revision 11
# speedup vs baseline: 1.1190x; 1.0088x over previous
"""Multi-head causal attention on 8 TRN2 NeuronCores.

Sharding: data-parallel over batch (2) x tensor-parallel over heads (4 groups
of 4 heads) = 8 cores. Each core emits a partial output projection in
transposed layout; the host sums the 4 partials per batch and folds the
(zero) biases.

Precision: x/W_QK/q/k fp16 (tf32-class mantissa; halves input DMA, full-rate
matmuls at any width), V/P/z/W_O/out bf16, all accumulation f32 in PSUM.
Softmax max-subtraction uses a sampled causal row max (each 128-row q-block
samples its causal range with 128 samples for the first 4 blocks and 64
for the rest) with a 55-unit safety margin; the un-normalized exp weights stay finite in bf16/f32 by
construction (measured emulation rel err 4.5e-3 vs the noisy reference).

Engine plan (GPSIMD cannot touch PSUM on this HW):
  PE   S^T pairs, z accumulation, projections, stats samples, transposes
  ACT  exp over paired S^T blocks (2-bank PSUM tiles), q/k PSUM drains,
       row-max formatting + its DMA queue, half of phase-3 drains
  DVE  v drain, fused diagonal-pair causal masks, stats boundary masks +
       grouped 3-D max-reduces, denominator reciprocal, z normalize
  Pool on-device mask generation (affine_select), denominator broadcast
  SP   batched fp16/bf16 input DMAs, per-superblock output DMAs

Schedule: one global software pipeline over every S-pair unit of every
head. A step emits [2 S^T matmuls, fused diag mask, exp], then one
interleave closure (v chunk / next head's projection slab / next head's
stats group / phase-3 chunk), then retires the z matmuls of the unit four
steps back; normalize (reciprocal+broadcast+multiply) is deferred one
further step so it lands in the mask-free region of the next superblock.
The front consumes x slabs as they stream; the tail splits the last
superblock's normalize and phase 3 in halves to overlap the final DMAs.

CoreSim cost model: 145.9us vs 222.3us baseline (1.52x); rel err 4.5e-3.
"""

import os
import sys

import numpy as np

for _p in ("/opt/trn_rl_repo", "/root/.axon_site/_ro/trn_rl_repo"):
    if os.path.isdir(_p) and _p not in sys.path:
        sys.path.insert(0, _p)

import concourse.bass as bass
from concourse import bacc
import concourse.tile as tile
from concourse import mybir
from concourse.masks import make_identity

F32 = mybir.dt.float32
F16 = mybir.dt.float16
BF16 = mybir.dt.bfloat16
AX = mybir.AxisListType
OP = mybir.AluOpType
AF = mybir.ActivationFunctionType

T = 2048
D = 1024
HPC = 4          # heads per core
DH = 64
NQB = T // 128   # 16
NSB = T // 512   # 4
NDC = D // 128   # 8
MARGIN = 55.0


def build_nc():
    nc = bacc.Bacc("TRN2", target_bir_lowering=False)
    xT = nc.dram_tensor("xT", [D, T], F16, kind="ExternalInput")
    wqk = nc.dram_tensor("wqk", [D, 128 * HPC], F16, kind="ExternalInput")
    wv = nc.dram_tensor("wv", [D, DH * HPC], F16, kind="ExternalInput")
    wo = nc.dram_tensor("wo", [2, 128, D], BF16, kind="ExternalInput")
    ones = nc.dram_tensor("ones", [1, T], F16, kind="ExternalInput")
    outT = nc.dram_tensor("outT", [D, T], BF16, kind="ExternalOutput")

    with tile.TileContext(nc) as tc:
        with (
            tc.tile_pool(name="const", bufs=1) as constp,
            tc.tile_pool(name="big", bufs=1) as bigp,
            tc.tile_pool(name="sb", bufs=3) as sbp,
            tc.tile_pool(name="psB", bufs=3, space="PSUM") as psB,
            tc.tile_pool(name="psZ", bufs=2, space="PSUM") as psZ,
        ):
            # ---- persistent SBUF ----
            xT_sb = bigp.tile([128, NDC, T], F16, tag="xT")
            wqk_sb = bigp.tile([128, NDC, 128 * HPC], F16, tag="wqk")
            wv_sb = bigp.tile([128, NDC, DH * HPC], F16, tag="wv")
            wo_sb = bigp.tile([128, 2, D], BF16, tag="wo")
            mskG_sb = constp.tile([128, NQB, 128], F32, tag="mskG")
            mskT_sb = constp.tile([128, 128], F32, tag="mskT")
            mskT2_sb = constp.tile([128, 2, 128], F32, tag="mskT2")
            ident = constp.tile([128, 128], F32, tag="ident")
            margin = constp.tile([NQB, 1], F32, tag="margin")
            q_sb = [bigp.tile([65, T], F16, tag=f"q{j}", name=f"q{j}") for j in range(HPC)]
            k_sb = [bigp.tile([65, T], F16, tag=f"k{j}", name=f"k{j}") for j in range(HPC)]
            v_sb = bigp.tile([128, NQB, HPC, DH + 1], BF16, tag="v")
            zT_sb = [bigp.tile([128, T], BF16, tag=f"zp{p}", name=f"zp{p}") for p in range(2)]

            xTr = xT.rearrange("(c p) t -> p c t", p=128)
            wqkr = wqk.rearrange("(c p) m -> p c m", p=128)
            nc.sync.dma_start(wqk_sb[:, :, 0:128], wqkr[:, :, 0:128])
            nc.sync.dma_start(xT_sb[:, 0:4, 0:512], xTr[:, 0:4, 0:512])
            nc.sync.dma_start(xT_sb[:, 4:8, 0:512], xTr[:, 4:8, 0:512])
            nc.sync.dma_start(wv_sb[:], wv.rearrange("(c p) m -> p c m", p=128))
            nc.sync.dma_start(k_sb[0][64:65, :], ones[:])
            nc.sync.dma_start(wqk_sb[:, :, 128:256], wqkr[:, :, 128:256])
            nc.sync.dma_start(xT_sb[:, 0:4, 512:1024], xTr[:, 0:4, 512:1024])
            nc.sync.dma_start(xT_sb[:, 4:8, 512:1024], xTr[:, 4:8, 512:1024])
            nc.sync.dma_start(wqk_sb[:, :, 256:512], wqkr[:, :, 256:512])
            for s in (2, 3):
                for c0 in (0, 4):
                    nc.sync.dma_start(
                        xT_sb[:, c0 : c0 + 4, s * 512 : (s + 1) * 512],
                        xTr[:, c0 : c0 + 4, s * 512 : (s + 1) * 512],
                    )
            for j in range(1, HPC):
                nc.sync.dma_start(k_sb[j][64:65, :], ones[:])
            nc.sync.dma_start(wo_sb[:], wo.rearrange("p k d -> k p d"))
            make_identity(nc, ident[:])
            nc.vector.memset(margin[:], -MARGIN)
            nc.gpsimd.memset(v_sb[:, :, :, DH : DH + 1], 1.0)
            # on-device masks (Pool is otherwise idle):
            # mskT[k, q] = 0 if k <= q else -1e9
            nc.gpsimd.memset(mskT_sb[:], 0.0)
            nc.gpsimd.affine_select(
                out=mskT_sb[:], in_=mskT_sb[:], compare_op=OP.is_ge,
                fill=-1e9, base=0, pattern=[[1, 128]], channel_multiplier=-1,
            )
            nc.gpsimd.tensor_copy(mskT2_sb[:, 0, :], mskT_sb[:])
            nc.gpsimd.tensor_copy(mskT2_sb[:, 1, :], mskT_sb[:])
            # mskG block qb: 0 if i*(qb+1) <= 128*qb + p else -1e9
            nc.gpsimd.memset(mskG_sb[:], 0.0)
            for qb in range(NQB):
                if qb < 4:
                    nc.gpsimd.affine_select(
                        out=mskG_sb[:, qb, :], in_=mskG_sb[:, qb, :],
                        compare_op=OP.is_ge, fill=-1e9, base=128 * qb,
                        pattern=[[-(qb + 1), 128]], channel_multiplier=1,
                    )
                else:
                    nc.gpsimd.affine_select(
                        out=mskG_sb[:, qb, 0:64], in_=mskG_sb[:, qb, 0:64],
                        compare_op=OP.is_ge, fill=-1e9, base=128 * qb,
                        pattern=[[-2 * (qb + 1), 64]], channel_multiplier=1,
                    )

            # ---- phase 1a: qT/kT slabs. width = number of 512-superblocks.
            # Copies drain in 512-col chunks so critical DVE ops (diag masks,
            # normalize) never queue behind a >1us copy; q goes via ACT only
            # in the front where ACT is idle.
            def qk_slab(j, s0, width=1, q_act=True):
                ps = psB.tile([128, 1024], F32, tag="mm")
                for h in range(width):
                    for c in range(NDC):
                        nc.tensor.matmul(
                            ps[:, h * 512 : (h + 1) * 512],
                            lhsT=(wqk_sb[:, c, j * 128 : (j + 1) * 128]),
                            rhs=(xT_sb[:, c, (s0 + h) * 512 : (s0 + h + 1) * 512]),
                            start=(c == 0),
                            stop=(c == NDC - 1),
                        )
                for h in range(width):
                    lo, hi = (s0 + h) * 512, (s0 + h + 1) * 512
                    if q_act:
                        nc.scalar.activation(
                            q_sb[j][0:64, lo:hi], ps[0:64, h * 512 : (h + 1) * 512],
                            AF.Copy,
                        )
                    else:
                        nc.vector.tensor_copy(
                            q_sb[j][0:64, lo:hi], ps[0:64, h * 512 : (h + 1) * 512]
                        )
                    if False and q_act and width == 2:
                        nc.scalar.activation(
                            k_sb[j][0:64, lo:hi], ps[64:128, h * 512 : (h + 1) * 512],
                            AF.Copy,
                        )
                    else:
                        nc.vector.tensor_copy(
                            k_sb[j][0:64, lo:hi], ps[64:128, h * 512 : (h + 1) * 512]
                        )

            # ---- phase 1b: v natural [t, 4*65] (ones col 64 for denominators)
            def v_chunk(tb):
                ps = psB.tile([128, 512], F32, tag="mm")
                for c in range(NDC):
                    nc.tensor.matmul(
                        ps[:, 0 : DH * HPC],
                        lhsT=(xT_sb[:, c, tb * 128 : (tb + 1) * 128]),
                        rhs=(wv_sb[:, c, :]),
                        start=(c == 0),
                        stop=(c == NDC - 1),
                    )
                nc.vector.tensor_copy(
                    v_sb[:, tb, :, 0:DH],
                    ps[:, 0 : DH * HPC].rearrange("p (j e) -> p j e", j=HPC),
                )

            # ---- phase 2A: stats — negated causal row max. Each 128-row
            # q-block qb samples its causal range at stride (qb+1), giving a
            # uniform 128 samples; 4 blocks share one PSUM tile, one boundary
            # mask add and one 3-D max-reduce. Margin 40 covers sampling.
            m_all = {}

            def stats_group(j, g, nq=4):
                if g == 0:
                    m_all[j] = sbp.tile(
                        [128, NQB], F32, tag=f"mall{j % 2}", name=f"mall{j}"
                    )
                ma = m_all[j]
                sw = 128 if g == 0 else 64  # sample slot width
                ps = psB.tile([128, 512], F32, tag="mm")
                for i in range(nq):
                    qb = nq * g + i
                    L = 128 * (qb + 1)
                    if qb == 0:
                        rhs = k_sb[j][0:64, 0:128]
                    else:
                        st = (qb + 1) * (128 // sw)
                        kr = k_sb[j][0:64, 0:L].rearrange(
                            "p (n st) -> p n st", st=st
                        )
                        rhs = kr[:, 0:sw, 0:1]
                    nc.tensor.matmul(
                        ps[:, i * sw : (i + 1) * sw],
                        lhsT=(q_sb[j][0:64, qb * 128 : (qb + 1) * 128]),
                        rhs=rhs,
                        start=True,
                        stop=True,
                    )
                pv = ps[:, 0 : nq * sw].rearrange("p (n w) -> p n w", w=sw)
                gv = mskG_sb[:, nq * g : nq * g + nq, 0:sw]
                if g == 0:
                    # first group holds qb0 (full causal mask) + wide edges
                    nc.vector.tensor_tensor(
                        ps[:, 0 : nq * sw], ps[:, 0 : nq * sw],
                        gv.rearrange("p n w -> p (n w)"),
                        op=OP.add,
                    )
                else:
                    nc.vector.tensor_tensor(
                        pv[:, :, sw - 32 : sw], pv[:, :, sw - 32 : sw],
                        gv[:, :, sw - 32 : sw],
                        op=OP.add,
                    )
                nc.vector.tensor_reduce(
                    ma[:, nq * g : nq * g + nq], pv[:], axis=AX.X,
                    op=OP.max, negate=True,
                )

            def stats_fin(j, s=None, half=None):
                # transpose -m into row 64 of q' (as -max - MARGIN, fp16).
                # s=None: whole row via the idle SP queue (heads 1-3);
                # else one 512-col chunk via the ACT queue (head 0 warmup).
                if half is not None:
                    lo, n = 8 * half, 8
                else:
                    lo, n = (0, NQB) if s is None else (4 * s, 4)
                pm = psB.tile([128, 512], F32, tag="mm")
                nc.tensor.transpose(
                    pm[0:n, 0:128], m_all[j][:, lo : lo + n], ident[:]
                )
                mT = sbp.tile([NQB, 128], F16, tag="mT", bufs=3)
                nc.vector.tensor_scalar_add(mT[0:n, :], pm[0:n, 0:128], -MARGIN)
                dst = q_sb[j][64:65, lo * 128 : (lo + n) * 128]
                if s is None:
                    nc.sync.dma_start(dst, mT[0:n, :])
                else:
                    nc.scalar.dma_start(dst, mT[0:n, :])

            # ---- phase 3: one [128d, 512t] out^T chunk -> o_sb; DMA per s
            o_sb = {}

            p3_alt = [0]

            def p3_pair(s, db0):
                if db0 == 0:
                    o_sb[s] = sbp.tile(
                        [128, NDC, 512], BF16, tag="osb", bufs=2, name=f"osb{s}"
                    )
                ops = psB.tile([128, 1024], F32, tag="mm")
                for i in range(2):
                    for p in range(2):
                        nc.tensor.matmul(
                            ops[:, i * 512 : (i + 1) * 512],
                            lhsT=(wo_sb[:, p, (db0 + i) * 128 : (db0 + i + 1) * 128]),
                            rhs=(zT_sb[p][:, s * 512 : (s + 1) * 512]),
                            start=(p == 0),
                            stop=(p == 1),
                        )
                dst = o_sb[s][:, db0 : db0 + 2, :].rearrange("p n w -> p (n w)")
                p3_alt[0] ^= 1
                if s == 2 or p3_alt[0]:
                    nc.scalar.activation(dst, ops[:], AF.Copy)
                else:
                    nc.vector.tensor_copy(dst, ops[:])

            def p3_out(s):
                nc.sync.dma_start(
                    outT.rearrange("(g p) t -> p g t", p=128)[
                        :, :, s * 512 : (s + 1) * 512
                    ],
                    o_sb[s][:],
                )

            # ---- phase 2B: the global S-pair pipeline ----------------------
            zps_t = {}
            pending = []
            norm_q = []

            def retire():
                u, pT = pending.pop(0)
                if norm_q:
                    norm_q.pop(0)()
                j, s = u["j"], u["s"]
                nkb = 4 * s + 4
                if (j, s) not in zps_t:
                    zps_t[(j, s)] = psZ.tile(
                        [65, 512], F32, tag="zz", name=f"zps{j}_{s}"
                    )
                zps = zps_t[(j, s)]
                for kb, w, off in u["z"]:
                    nc.tensor.matmul(
                        zps[:, 512 - w : 512],
                        lhsT=v_sb[:, kb, j, :],
                        rhs=pT[:, off : off + w],
                        start=(kb == 0),
                        stop=(kb == nkb - 1),
                        skip_group_check=True,
                    )
                if u["close"]:
                    def mknorm(j=j, s=s, zps=zps):
                        def donorm():
                            r1 = sbp.tile([1, 512], F32, tag="r1", bufs=3)
                            nc.vector.reciprocal(r1[:], zps[64:65, :])
                            rb = sbp.tile([64, 512], F32, tag="rb", bufs=3)
                            nc.gpsimd.partition_broadcast(rb[:], r1[:])
                            p, po = j // 2, 64 * (j % 2)
                            nc.vector.tensor_mul(
                                zT_sb[p][po : po + 64, s * 512 : (s + 1) * 512],
                                zps[0:64, :],
                                rb[:],
                            )
                        return donorm
                    norm_q.append(mknorm())
                    del zps_t[(j, s)]

            def step(u, il):
                j, s, a, b = u["j"], u["s"], u["a"], u["b"]
                wa = 512 if a < 4 * s else 512 - 128 * (a - 4 * s)
                wb = 512 if b < 4 * s else 512 - 128 * (b - 4 * s)
                sps = psB.tile([128, 1024], F32, tag="mm")
                nc.tensor.matmul(
                    sps[:, 0:wa],
                    lhsT=(k_sb[j][0:65, a * 128 : (a + 1) * 128]),
                    rhs=(q_sb[j][0:65, s * 512 + 512 - wa : (s + 1) * 512]),
                    start=True,
                    stop=True,
                )
                nc.tensor.matmul(
                    sps[:, wa : wa + wb],
                    lhsT=(k_sb[j][0:65, b * 128 : (b + 1) * 128]),
                    rhs=(q_sb[j][0:65, s * 512 + 512 - wb : (s + 1) * 512]),
                    start=True,
                    stop=True,
                )
                if a >= 4 * s:
                    # both kbs of a pair are diagonal together: one strided
                    # add covers both [*, 0:128] and [*, wa:wa+128]
                    dv = sps[:, 0 : 2 * wa].rearrange(
                        "p (n w) -> p n w", w=wa
                    )[:, :, 0:128]
                    nc.vector.tensor_tensor(dv, dv, mskT2_sb[:], op=OP.add)
                pT = sbp.tile([128, 1024], BF16, tag="pT", bufs=6)
                nc.scalar.activation(pT[:, 0 : wa + wb], sps[:, 0 : wa + wb], AF.Exp)
                if il is not None:
                    il()
                u["z"] = [(a, wa, 0), (b, wb, wa)]
                pending.append((u, pT))
                if len(pending) > 4:
                    retire()

            def head_units(j):
                us = []
                for s in range(NSB):
                    nkb = 4 * s + 4
                    for a in range(0, nkb, 2):
                        us.append(
                            {"j": j, "s": s, "a": a, "b": a + 1,
                             "close": a + 2 == nkb}
                        )
                return us

            # ---- emission schedule ----
            # front: head-0 projections (512-wide for latency) with head-0
            # stats + per-superblock row-max finalize woven in
            for s in (0, 1):
                qk_slab(0, s, width=1, q_act=True)
                stats_group(0, s)
                stats_fin(0, s=s)
                if s == 0:
                    v_chunk(0)
                    v_chunk(1)

            def vp(t0, t1):
                def fn():
                    v_chunk(t0)
                    v_chunk(t1)
                return fn

            def vq(t0):
                def fn():
                    for t in range(t0, t0 + 4):
                        v_chunk(t)
                return fn

            def qk1s(s):
                return lambda: qk_slab(1, s, width=1, q_act=False)

            def qs2(jn, h):
                return lambda: qk_slab(jn, 2 * h, width=2)

            def stg(jn, g):
                return lambda: stats_group(jn, g)

            def qk0s(s):
                return lambda: qk_slab(0, s, width=1, q_act=True)

            def stgf0(s):
                def fn():
                    stats_group(0, s)
                    stats_fin(0, s=s)
                return fn

            def sfin(jn):
                return lambda: stats_fin(jn)

            def sfh(jn, h):
                return lambda: stats_fin(jn, half=h)

            def p3p(s, db0):
                return lambda: p3_pair(s, db0)

            il_map = {
                0: [qk1s(0), qk0s(2), stgf0(2), vp(2, 3), None,
                    qk1s(1), qk1s(2), vq(4), qk0s(3), stgf0(3),
                    stg(1, 0), stg(1, 1), stg(1, 2), qk1s(3), vq(8),
                    stg(1, 3), sfin(1), vp(12, 13), vp(14, 15), None],
                1: [None, None, qs2(2, 0), qs2(2, 1), None, None, stg(2, 0),
                    None, stg(2, 1), None, stg(2, 2), None, stg(2, 3),
                    sfin(2), None, None, None, None, None, None],
                2: [None, None, qs2(3, 0), qs2(3, 1), None, None, stg(3, 0),
                    None, stg(3, 1), None, stg(3, 2), None, stg(3, 3),
                    sfin(3), None, None, None, None, None, None],
                3: [None, None, None, None, None, None, None,
                    p3p(0, 0), p3p(0, 2), p3p(0, 4), p3p(0, 6),
                    lambda: p3_out(0),
                    p3p(1, 0), p3p(1, 2), p3p(1, 4), p3p(1, 6),
                    lambda: p3_out(1),
                    p3p(2, 0), p3p(2, 2), p3p(2, 4)],
            }

            for j in range(HPC):
                il_list = il_map[j]
                for i, u in enumerate(head_units(j)):
                    step(u, il_list[i])
            # ---- tail: retire the last units with a split normalize so
            # phase 3's last superblock overlaps the final softmax columns.
            outv = outT.rearrange("(g p) t -> p g t", p=128)
            nc.sync.dma_start(
                outv[:, 0:4, 1024:1536], o_sb[2][:, 0:4, :]
            )
            while len(pending) > 2:
                retire()
            while norm_q:
                norm_q.pop(0)()
            pending[1][0]["close"] = False
            retire()  # z for kbs 12,13 -> cols [0:256] of (3,3) complete
            zps33 = zps_t[(3, 3)]
            r1a = sbp.tile([1, 512], F32, tag="r1", bufs=3, name="r1a")
            nc.vector.reciprocal(r1a[0:1, 0:256], zps33[64:65, 0:256])
            rba = sbp.tile([64, 512], F32, tag="rb", bufs=3, name="rba")
            nc.gpsimd.partition_broadcast(rba[0:64, 0:256], r1a[0:1, 0:256])
            nc.vector.tensor_mul(
                zT_sb[1][64:128, 1536:1792], zps33[0:64, 0:256], rba[0:64, 0:256]
            )
            # PE keeps busy on the last s2 projection chunk while the first
            # normalize half and the final exp drain
            p3_pair(2, 6)
            nc.sync.dma_start(
                outv[:, 4:8, 1024:1536], o_sb[2][:, 4:8, :]
            )
            retire()  # z for kbs 14,15 -> cols [256:512] complete
            r1b = sbp.tile([1, 512], F32, tag="r1", bufs=3, name="r1b")
            nc.vector.reciprocal(r1b[0:1, 0:256], zps33[64:65, 256:512])
            rbb = sbp.tile([64, 512], F32, tag="rb", bufs=3, name="rbb")
            nc.gpsimd.partition_broadcast(rbb[0:64, 0:256], r1b[0:1, 0:256])
            nc.vector.tensor_mul(
                zT_sb[1][64:128, 1792:2048], zps33[0:64, 256:512], rbb[0:64, 0:256]
            )
            del zps_t[(3, 3)]

            o_sb[3] = sbp.tile(
                [128, NDC, 512], BF16, tag="osb", bufs=2, name="osb3"
            )

            def p3_pair_h(db0, h):
                ops = psB.tile([128, 1024], F32, tag="mm")
                for i in range(2):
                    for p in range(2):
                        nc.tensor.matmul(
                            ops[:, i * 512 + h * 256 : i * 512 + (h + 1) * 256],
                            lhsT=(wo_sb[:, p, (db0 + i) * 128 : (db0 + i + 1) * 128]),
                            rhs=(zT_sb[p][:, 1536 + h * 256 : 1792 + h * 256]),
                            start=(p == 0),
                            stop=(p == 1),
                        )
                dst = o_sb[3][:, db0 : db0 + 2, h * 256 : (h + 1) * 256]
                srcv = ops[:, 0:1024].rearrange("p (i w) -> p i w", w=512)[
                    :, :, h * 256 : (h + 1) * 256
                ]
                if db0 % 4 == 0:
                    nc.scalar.activation(dst, srcv, AF.Copy)
                else:
                    nc.vector.tensor_copy(dst, srcv)

            for h in range(2):
                for db0 in (0, 2):
                    p3_pair_h(db0, h)
                nc.sync.dma_start(
                    outv[:, 0:4, 1536 + h * 256 : 1792 + h * 256],
                    o_sb[3][:, 0:4, h * 256 : (h + 1) * 256],
                )
                for db0 in (4, 6):
                    p3_pair_h(db0, h)
                nc.sync.dma_start(
                    outv[:, 4:8, 1536 + h * 256 : 1792 + h * 256],
                    o_sb[3][:, 4:8, h * 256 : (h + 1) * 256],
                )
    nc.compile()
    return nc


_NC = None


def _get_nc():
    global _NC
    if _NC is None:
        _NC = build_nc()
    return _NC


def _make_in_maps(inputs):
    x = np.ascontiguousarray(np.asarray(inputs["residual_stream"], dtype=np.float32))
    W_Q = np.asarray(inputs["W_Q"], dtype=np.float32)
    W_K = np.asarray(inputs["W_K"], dtype=np.float32)
    W_V = np.asarray(inputs["W_V"], dtype=np.float32)
    W_O = np.asarray(inputs["W_output"], dtype=np.float32)

    try:
        import ml_dtypes
        bf16 = ml_dtypes.bfloat16
    except ImportError:
        bf16 = np.float32

    qi = np.arange(128)
    mskS = np.where(qi[None, :] <= qi[:, None], 0.0, -1e9).astype(np.float32)  # [q,k]
    mskT = np.ascontiguousarray(mskS.T)  # [k,q]
    si = np.arange(128)
    mskG = np.zeros((128, NQB, 128), np.float32)
    for qb in range(NQB):
        mskG[:, qb, :] = np.where(
            si[None, :] * (qb + 1) <= 128 * qb + qi[:, None], 0.0, -1e9
        )
    mskG = np.ascontiguousarray(mskG.reshape(128, NQB * 128))

    in_maps = []
    for c in range(8):
        b, hg = c // 4, c % 4
        hs = [4 * hg + j for j in range(HPC)]
        wqk_h = np.concatenate(
            [np.concatenate([W_Q[h] / 8.0, W_K[h]], axis=1) for h in hs], axis=1
        )  # [1024, 512]
        wv_h = np.concatenate([W_V[h] for h in hs], axis=1)  # [1024, 256]
        wo_h = np.stack(
            [np.concatenate([W_O[hs[2 * p]], W_O[hs[2 * p + 1]]], axis=0)
             for p in range(2)]
        )  # [2, 128, 1024]
        in_maps.append(
            {
                "xT": np.ascontiguousarray(x[b].T).astype(np.float16),
                "wqk": np.ascontiguousarray(wqk_h).astype(np.float16),
                "wv": np.ascontiguousarray(wv_h).astype(np.float16),
                "wo": np.ascontiguousarray(wo_h).astype(bf16),
                "ones": np.ones((1, T), np.float16),
            }
        )
    return in_maps


def _postprocess(results, inputs):
    W_O = np.asarray(inputs["W_output"], dtype=np.float32)
    b_V = np.asarray(inputs["b_V"], dtype=np.float32)
    b_out = np.asarray(inputs["b_output"], dtype=np.float32)
    out = np.zeros((2, T, D), dtype=np.float32)
    for c in range(8):
        out[c // 4] += results[c]["outT"].T.astype(np.float32)
    # z = P @ v + b_V (P rows sum to 1) -> fold b_V through W_O on the host
    const = np.einsum("he,hed->d", b_V, W_O) + b_out
    out += const[None, None, :]
    return out


def kernel(**inputs):
    from concourse.bass_utils import run_bass_kernel_spmd

    nc = _get_nc()
    res = run_bass_kernel_spmd(nc, _make_in_maps(inputs), core_ids=list(range(8)))
    return _postprocess(res.results, inputs)


def kernel_traced(**inputs):
    """Returns (output, exec_time_ns or None) using a traced run."""
    from concourse.bass_utils import run_bass_kernel_spmd

    nc = _get_nc()
    res = run_bass_kernel_spmd(
        nc, _make_in_maps(inputs), core_ids=list(range(8)), trace=True
    )
    return _postprocess(res.results, inputs), res.exec_time_ns
